# revision 38
# baseline (speedup 1.0000x reference)

# CRGCN multi-behavior GCN forward loss on 8 Trainium2 NeuronCores.
#
# Strategy (graph/data parallel, dest-node sharding):
#  - Nodes (users+items, 200000 -> padded 200704) are sharded row-wise across
#    8 cores (25088 = 196*128 nodes/core). Edges are partitioned by the shard
#    of their destination (col) node on the host, bucketed by (128-dest tile,
#    source bucket of 28672 rows) and padded so every 128-edge chunk maps to
#    one dest tile and one source bucket. The chunk schedule is the max over
#    cores so a single SPMD program fits all 8 cores.
#  - Per behavior each core holds a bf16 table T2 = [dinv*total | total]
#    ([200704, 128], 256B rows) for ALL nodes, produced by AllGather of
#    per-shard slabs. Message pass: dma_gather (int16 in-bucket indices) of
#    T2 rows for edge sources; a 0/1 one-hot (edge x dest-in-tile) built on
#    DVE from edge cols; PE matmul contracts edges, accumulating
#    S^T[feat, dest] = sum_e dinv[r_e]*total[r_e] x onehot in PSUM per dest
#    tile; then S @ W, *dinv[d], +b, l2-normalize, residual-accumulate into
#    the SBUF-resident fp32 total shard.
#  - deg (in-degree) is a one-hot x ones matmul (bf16, exact), per behavior,
#    from the same col data.
#  - BPR loss: batch rows sharded across cores; u/pos/neg rows fetched with
#    per-partition indirect DMA from the raw-total half of T2; dots +
#    softplus(-d) (relu + log1p poly) on-device; partials AllGathered so all
#    cores emit the identical final scalar.

import sys

sys.path.insert(0, "/opt/trn_rl_repo")

import dataclasses
import numpy as np

# ---------------- problem constants (hardcoded; kernel.py is standalone) ---
N_USERS = 100000
N_ITEMS = 100000
N_NODES = 200000
EMBED = 64
N_BEH = 3
BATCH = 4096
REG_WEIGHT = 1e-4
NCORES = 8

FULL_CFG = dict(
    ncores=NCORES,
    embed=EMBED,
    nbeh=N_BEH,
    shard=25088,          # 196 * 128
    nt=196,               # dest tiles per shard
    wt=128,               # T2 row width in bf16 elems (256B)
    nbuck=7,              # source buckets
    bucket=28672,         # rows per bucket (7 * 28672 = 200704)
    wtiles=8,             # dest tiles per gather window
    g=32,                 # chunks per one-hot build group
    flush=14,             # tiles per T2 staging flush (196 = 14*14)
    batch=BATCH,
    batch_per_core=BATCH // NCORES,   # 512
    n_nodes=N_NODES,
    reg_weight=REG_WEIGHT,
)


# ---------------------------------------------------------------------------
# Host-side preprocessing
# ---------------------------------------------------------------------------
def make_schedule_and_arrays(edges, cfg):
    """edges: [NB, 2, E]. Builds the (window, bucket, tile)-ordered common
    chunk schedule and the per-core col/idx arrays."""
    ncores = cfg["ncores"]
    NT = cfg["nt"]
    NB = cfg["nbeh"]
    NBK = cfg["nbuck"]
    BUCK = cfg["bucket"]
    WT = cfg["wtiles"]
    NW = (NT + WT - 1) // WT

    sched = {"C": [], "cells": [], "tiles": [], "windows": [],
             "tile_cstart": []}
    cols_arr = [[None] * NB for _ in range(ncores)]
    idx_arr = [[None] * NB for _ in range(ncores)]
    dinv_arr = np.zeros((ncores, NB, 128, NT), dtype=np.float32)

    n_nodes = cfg["n_nodes"]
    shard = cfg["shard"]
    for b in range(NB):
        row = np.asarray(edges[b, 0], dtype=np.int64)
        col = np.asarray(edges[b, 1], dtype=np.int64)
        # host-side in-degree -> dinv per core shard, [128, NT] layout
        deg = np.bincount(col, minlength=n_nodes).astype(np.float32)
        dinv_g = np.where(deg > 0,
                          1.0 / np.sqrt(np.maximum(deg, 1.0)),
                          0.0).astype(np.float32)
        dinv_pad = np.zeros(ncores * shard, dtype=np.float32)
        dinv_pad[:n_nodes] = dinv_g
        for s in range(ncores):
            dinv_arr[s, b] = dinv_pad[s * shard:(s + 1) * shard].reshape(
                NT, 128).T
        gt = col >> 7                       # global dest tile
        s_of = gt // NT                     # owning core
        t_of = gt - s_of * NT               # local dest tile
        # permuted table layout: bucket k holds piece k (PIECE local rows)
        # of every core's shard, so AllGather k is per-rank contiguous:
        # pos(n) = beta*BUCK + (n//SH)*PIECE + (n%SH)%PIECE,
        # beta = (n%SH)//PIECE
        PIECE = BUCK // ncores
        r_loc = row % shard
        beta = r_loc // PIECE               # source bucket (= piece id)
        # per (core, tile, bucket) counts
        cellkey = (s_of * NT + t_of) * NBK + beta
        cnt = np.bincount(cellkey, minlength=ncores * NT * NBK).reshape(
            ncores, NT, NBK)
        K_cell = -(-cnt.max(axis=0) // 128)           # [NT, NBK]
        empty_t = K_cell.sum(axis=1) == 0
        K_cell[empty_t, 0] = 1

        # gather order: (window, bucket, tile); consumption order:
        # (window, tile, bucket). Chunks get positions in both orders.
        C = int(K_cell.sum())
        cell_start = {}      # gather-order chunk start per cell
        cell_cstart = {}     # consumption-order chunk start per cell
        pos = 0
        for w in range(NW):
            ts = range(w * WT, min((w + 1) * WT, NT))
            for be in range(NBK):
                for t in ts:
                    if K_cell[t, be]:
                        cell_start[(t, be)] = pos
                        pos += int(K_cell[t, be])
        assert pos == C
        cpos = 0
        tile_cstart = np.zeros(NT + 1, dtype=np.int64)
        for w in range(NW):
            ts = range(w * WT, min((w + 1) * WT, NT))
            for t in ts:
                tile_cstart[t] = cpos
                for be in range(NBK):
                    if K_cell[t, be]:
                        cell_cstart[(t, be)] = cpos
                        cpos += int(K_cell[t, be])
        tile_cstart[NT] = cpos
        assert cpos == C

        # per-tile consumption: ordered chunk positions + total K per tile
        tiles = []
        for t in range(NT):
            plist = []
            for be in range(NBK):
                if K_cell[t, be]:
                    st = cell_start[(t, be)]
                    plist.extend(range(st, st + int(K_cell[t, be])))
            tiles.append(plist)

        # per-window gather segments: (bucket, pos_start, n_chunks)
        windows = []
        for w in range(NW):
            ts = range(w * WT, min((w + 1) * WT, NT))
            segs = []
            for be in range(NBK):
                n = int(sum(K_cell[t, be] for t in ts))
                if n:
                    st = min(cell_start[(t, be)] for t in ts
                             if K_cell[t, be])
                    segs.append((be, st, n))
            windows.append(segs)

        sched["C"].append(C)
        sched["cells"].append((K_cell, cell_start))
        sched["tiles"].append(tiles)
        sched["windows"].append(windows)
        sched["tile_cstart"].append(tile_cstart)

        # ------------- per-core arrays -------------
        starts_np = np.zeros((NT, NBK), dtype=np.int64)
        for (t, be), st in cell_start.items():
            starts_np[t, be] = st
        cstarts_np = np.zeros((NT, NBK), dtype=np.int64)
        for (t, be), st in cell_cstart.items():
            cstarts_np[t, be] = st
        for s in range(ncores):
            colv = np.full(C * 128, 128.0, dtype=np.float32)
            rowv = np.zeros(C * 128, dtype=np.int64)   # in-bucket idx
            sel = s_of == s
            r_s = row[sel]
            c_s = col[sel]
            t_s = t_of[sel]
            be_s = beta[sel]
            key = t_s * NBK + be_s
            order = np.argsort(key, kind="stable")
            r_s, c_s, t_s, be_s, key = (r_s[order], c_s[order], t_s[order],
                                        be_s[order], key[order])
            seg_start = np.searchsorted(key, np.arange(NT * NBK))
            within = np.arange(len(key)) - seg_start[key]
            dst = starts_np[t_s, be_s] * 128 + within
            cdst = cstarts_np[t_s, be_s] * 128 + within
            colv[cdst] = (c_s & 127).astype(np.float32)
            rowv[dst] = (r_s // shard) * (BUCK // ncores) \
                + (r_s % shard) % (BUCK // ncores)
            import ml_dtypes as _md
            cols_arr[s][b] = np.ascontiguousarray(
                colv.reshape(C, 128).T).astype(_md.bfloat16)   # [128, C]
            # idx16: [128, C*8]; gather element i -> [i%16 (+16k), off+i//16]
            iv = rowv.reshape(C * 128)
            i16 = np.zeros((16, C * 8), dtype=np.int16)
            ii = np.arange(C * 128)
            i16[ii % 16, ii // 16] = iv.astype(np.int16)
            idx_arr[s][b] = np.ascontiguousarray(np.tile(i16, (8, 1)))

    return sched, cols_arr, idx_arr, dinv_arr


def make_inputs_per_core(inputs, cfg, sched_arrays):
    import ml_dtypes

    ncores = cfg["ncores"]
    SH = cfg["shard"]
    E = cfg["embed"]
    NB = cfg["nbeh"]
    BPC = cfg["batch_per_core"]
    BJ = BPC // 128
    n_nodes = cfg["n_nodes"]
    n_users = n_nodes // 2

    sched, cols_arr, idx_arr, dinv_arr = sched_arrays

    user_emb = np.asarray(inputs["user_emb"], dtype=np.float32)
    item_emb = np.asarray(inputs["item_emb"], dtype=np.float32)
    gcn_weight = np.asarray(inputs["gcn_weight"], dtype=np.float32)
    gcn_bias = np.asarray(inputs["gcn_bias"], dtype=np.float32)
    batch_data = np.asarray(inputs["batch_data"], dtype=np.int64)

    total0 = np.concatenate([user_emb, item_emb], axis=0)

    G = cfg["g"]
    iotar = np.repeat(np.arange(128, dtype=np.float32), G)[None, :].astype(
        ml_dtypes.bfloat16)                       # [1, 128*G], j//G values
    w_bf = gcn_weight.astype(ml_dtypes.bfloat16)
    bb = np.tile(gcn_bias[:, None, :], (1, 128, 1)).astype(np.float32)

    in_maps = []
    for s in range(ncores):
        lo = s * SH
        hi = min((s + 1) * SH, n_nodes)
        init_shard = np.zeros((SH, E), dtype=np.float32)
        if hi > lo:
            init_shard[: hi - lo] = total0[lo:hi]

        PIECE = cfg["bucket"] // ncores

        def pos(n):
            # node id -> permuted table row (see make_schedule_and_arrays)
            r = n % SH
            return (r // PIECE) * cfg["bucket"] + (n // SH) * PIECE \
                + r % PIECE

        bidx = np.zeros((NB * 3, 128, BJ), dtype=np.int32)
        rs = slice(s * BPC, (s + 1) * BPC)
        for b in range(NB):
            u = pos(batch_data[rs, b, 0].astype(np.int32))
            p = pos(batch_data[rs, b, 1].astype(np.int32) + n_users)
            n = pos(batch_data[rs, b, 2].astype(np.int32) + n_users)
            for k, v in enumerate((u, p, n)):
                bidx[b * 3 + k] = v.reshape(BJ, 128).T

        m = {
            "init_shard": init_shard,
            "iotar_in": iotar,
            "w_in": w_bf,
            "bb_in": bb,
            "bidx_in": bidx,
            "dinv_in": dinv_arr[s],
        }
        for b in range(NB):
            m[f"col{b}"] = cols_arr[s][b]
            m[f"idx{b}"] = idx_arr[s][b]
        in_maps.append(m)
    return in_maps


# ---------------------------------------------------------------------------
# Device program
# ---------------------------------------------------------------------------
def build_program(cfg, sched, sim=False):
    from concourse import bass, bacc, mybir, tile

    dt = mybir.dt
    AF = mybir.ActivationFunctionType
    ALU = mybir.AluOpType

    ncores = cfg["ncores"]
    NT = cfg["nt"]
    SH = cfg["shard"]
    NTOT = SH * ncores
    E = cfg["embed"]
    WT = cfg["wt"]            # 128 table cols
    NBK = cfg["nbuck"]
    BUCK = cfg["bucket"]
    WTL = cfg["wtiles"]
    G = cfg["g"]
    FLUSH = cfg["flush"]
    BPC = cfg["batch_per_core"]
    BJ = BPC // 128
    NB = cfg["nbeh"]
    NV = NB + 1
    NW = (NT + WTL - 1) // WTL

    C = sched["C"]
    tiles_md = sched["tiles"]
    windows_md = sched["windows"]
    cstart_md = sched["tile_cstart"]

    # max chunks in any window (for the staging tile size)
    wch_max = 0
    for b in range(NB):
        for w in range(NW):
            wch = sum(n for (_, _, n) in windows_md[b][w])
            wch_max = max(wch_max, wch)

    def bc(ap, where, n):
        newap = list(ap.ap)
        newap.insert(where, [0, n])
        return dataclasses.replace(ap, ap=newap)

    nc = bacc.Bacc("TRN2", target_bir_lowering=False, debug=False,
                   num_devices=1 if sim is True else ncores,
                   num_swdge_queues=4)

    def all_gather(src_tile, dst_tile, nrep):
        # sim mode: stand in for the collective with local HBM->HBM copies
        # of the same receive volume so TimelineSim can run (single-core,
        # no collectives) with comparable DMA load + dependencies.
        if sim is True:
            n = src_tile.shape[0]
            for r in range(nrep):
                nc.sync.dma_start(out=dst_tile[r * n:(r + 1) * n, :],
                                  in_=src_tile[:])
        else:
            nc.gpsimd.collective_compute(
                "AllGather", mybir.AluOpType.bypass,
                replica_groups=[list(range(nrep))],
                ins=[src_tile[:].opt()], outs=[dst_tile[:].opt()])

    f32, bf16, i32, i16 = dt.float32, dt.bfloat16, dt.int32, dt.int16
    shared = "Local"

    init_in = nc.dram_tensor("init_shard", [SH, E], f32,
                             kind="ExternalInput").ap()
    iotar_in = nc.dram_tensor("iotar_in", [1, 128 * G], bf16,
                              kind="ExternalInput").ap()
    dinv_in = nc.dram_tensor("dinv_in", [NB, 128, NT], f32,
                             kind="ExternalInput").ap()
    w_in = nc.dram_tensor("w_in", [NB, E, E], bf16, kind="ExternalInput").ap()
    bb_in = nc.dram_tensor("bb_in", [NB, 128, E], f32,
                           kind="ExternalInput").ap()
    bidx_in = nc.dram_tensor("bidx_in", [NB * 3, 128, BJ], i32,
                             kind="ExternalInput").ap()
    col_in = [nc.dram_tensor(f"col{b}", [128, C[b]], bf16,
                             kind="ExternalInput").ap() for b in range(NB)]
    idx_in = [nc.dram_tensor(f"idx{b}", [128, C[b] * 8], i16,
                             kind="ExternalInput").ap() for b in range(NB)]
    loss_out = nc.dram_tensor("loss", [1, 1], f32, kind="ExternalOutput").ap()

    with tile.TileContext(nc) as tc:
        with (
            tc.tile_pool(name="dram", bufs=1, space="DRAM") as dpool,
            tc.tile_pool(name="pers", bufs=1) as pers,
            tc.tile_pool(name="work", bufs=2) as work,
            tc.tile_pool(name="small", bufs=4) as small,
            tc.tile_pool(name="ppx", bufs=3, space="PSUM") as ppx,
            tc.tile_pool(name="ppy", bufs=2, space="PSUM") as ppy,
        ):
            t2s = [dpool.tile([SH, WT], bf16, tag=f"t2s{v}",
                              name=f"t2s{v}") for v in range(NV)]
            t2f = [dpool.tile([NTOT, WT], bf16, tag=f"t2f{v}",
                              name=f"t2f{v}", addr_space=shared)
                   for v in range(NV)]
            lag_i = dpool.tile([1, 2], f32, tag="lag_i", name="lag_i")
            lag_o = dpool.tile([ncores, 2], f32, tag="lag_o", name="lag_o",
                               addr_space=shared)

            tot = pers.tile([128, NT * E], f32, tag="tot", name="tot")
            irep = pers.tile([128, 128 * G], bf16, tag="irep", name="irep")
            wsb = pers.tile([E, NB * E], bf16, tag="wsb", name="wsb")
            bbsb = pers.tile([128, NB * E], f32, tag="bbsb", name="bbsb")
            bidx = pers.tile([128, NB * 3 * BJ], i32, tag="bidx", name="bidx")
            dinvsb = pers.tile([128, NB * NT], f32, tag="dinvsb",
                               name="dinvsb")
            dinv3z = pers.tile([128, 1], f32, tag="dinv3z", name="dinv3z")
            onesf = pers.tile([128, 1], f32, tag="onesf", name="onesf")
            racc = pers.tile([128, 16], f32, tag="racc", name="racc")
            blacc = pers.tile([128, NB], f32, tag="blacc", name="blacc")

            nc.sync.dma_start(
                out=irep[:].rearrange("p (a x) -> p a x", a=1),
                in_=bc(iotar_in, 0, 128))
            nc.sync.dma_start(
                out=dinvsb[:].rearrange("p (b t) -> p b t", b=NB),
                in_=dinv_in.rearrange("b p t -> p b t"))
            nc.sync.dma_start(
                out=wsb[:].rearrange("k (b e) -> k b e", b=NB),
                in_=w_in.rearrange("b k e -> k b e"))
            nc.sync.dma_start(
                out=bbsb[:].rearrange("p (b e) -> p b e", b=NB),
                in_=bb_in.rearrange("b p e -> p b e"))
            nc.sync.dma_start(
                out=bidx[:].rearrange("p (a j) -> p a j", a=NB * 3),
                in_=bidx_in.rearrange("a p j -> p a j"))
            nc.sync.dma_start(
                out=tot[:].rearrange("p (t e) -> p t e", e=E),
                in_=init_in.rearrange("(t p) e -> p t e", p=128))
            nc.vector.memset(onesf[:], 1.0)
            nc.vector.memset(dinv3z[:], 0.0)
            epsb = pers.tile([128, 1], f32, tag="epsb", name="epsb")
            nc.vector.memset(epsb[:], 1e-24)

            def dinv_ap(v, t):
                # per-partition dinv scalar for (behavior v, dest tile t)
                if v < NB:
                    return dinvsb[:, v * NT + t:v * NT + t + 1]
                return dinv3z[:, 0:1]

            # reg term: sum of squares of the initial embeddings
            NREG = (NT * E + 1023) // 1024
            sqd = pers.tile([128, 1024], f32, tag="sqd", name="sqd")
            for i in range(NREG):
                sl = slice(i * 1024, min((i + 1) * 1024, NT * E))
                nc.scalar.activation(out=sqd[:, : sl.stop - sl.start],
                                     in_=tot[:, sl], func=AF.Square,
                                     accum_out=racc[:, i:i + 1])

            # ------- lazy consumption-ordered one-hot group builder -------
            class IndBuilder:
                """Builds one-hot groups for consumption positions
                [c0, c0+wch) on demand, in order, so only a few groups are
                live at once. Layout [p, d, g] (g innermost) so every
                operand of the is_equal has stride-1 innermost dims and the
                DVE runs in its 2x perf mode."""

                def __init__(self, cs, c0, wch):
                    self.cs, self.c0, self.wch = cs, c0, wch
                    self.groups = {}

                def get(self, cpos):
                    rel = cpos - self.c0
                    g0 = (rel // G) * G
                    if g0 not in self.groups:
                        gw = min(G, self.wch - g0)
                        ind = work.tile([128, 128 * G], dt.bfloat16,
                                        tag="ind", name="ind", bufs=3)
                        iv = ind[:].rearrange("p (d g) -> p d g", g=G)
                        nc.vector.tensor_tensor(
                            out=iv[:, :, :gw],
                            in0=irep[:].rearrange(
                                "p (d g) -> p d g", g=G)[:, :, :gw],
                            in1=bc(self.cs[:, g0:g0 + gw], 1, 128),
                            op=ALU.is_equal)
                        self.groups[g0] = iv
                    return self.groups[g0], rel - g0

            # ------------- T2 staging + per-piece AllGather -------------
            # Table v is built tile-by-tile (fused into main_pass(v-1)'s
            # post_tile stream); every 2 flushes completes one PIECE of the
            # local slab and fires that piece's AllGather, so collectives
            # overlap the remaining compute of the producing behavior.
            PIECE = BUCK // ncores

            def ag_piece(v, k):
                src = t2s[v][k * PIECE:(k + 1) * PIECE, :]
                if sim == "noag":
                    return
                if sim:
                    for r in range(ncores):
                        o = k * BUCK + r * PIECE
                        nc.sync.dma_start(out=t2f[v][o:o + PIECE, :],
                                          in_=src)
                else:
                    nc.gpsimd.collective_compute(
                        "AllGather", mybir.AluOpType.bypass,
                        replica_groups=[list(range(ncores))],
                        ins=[src.opt()],
                        outs=[t2f[v][k * BUCK:(k + 1) * BUCK, :].opt()])

            class Stager:
                def __init__(self, v):
                    self.v = v
                    self.s65 = None

                def stage(self, t):
                    i = t % FLUSH
                    if i == 0:
                        self.s65 = work.tile([128, FLUSH * WT], bf16,
                                             tag="s65", name="s65")
                    totsl = tot[:, t * E:(t + 1) * E]
                    nc.vector.tensor_scalar(
                        out=self.s65[:, i * WT:i * WT + E], in0=totsl,
                        scalar1=dinv_ap(self.v, t), scalar2=None,
                        op0=ALU.mult)
                    nc.scalar.copy(
                        out=self.s65[:, i * WT + E:i * WT + 2 * E],
                        in_=totsl)
                    if i == FLUSH - 1:
                        tf = t - i
                        nc.sync.dma_start(
                            out=t2s[self.v][:].rearrange(
                                "(t p) w -> p t w", p=128)[:, tf:t + 1, :],
                            in_=self.s65[:, :FLUSH * WT].rearrange(
                                "p (t w) -> p t w", w=WT))

            def assemble(v):
                st = Stager(v)
                for t in range(NT):
                    st.stage(t)

            # ---------------- main pass ----------------
            self_q = [0]
            NQ = 4

            def ag_block(v):
                # all 7 piece triggers up-front: their input flushes
                # completed during the previous pass, so these don't stall
                # the in-order Pool queue, and the transfers stream ahead
                # of the bucket-ordered gathers that consume them.
                for k in range(NBK):
                    ag_piece(v, k)

            def main_pass(b):
                ag_block(b)
                stg = Stager(b + 1)
                for w in range(NW):
                    segs = windows_md[b][w]
                    g0 = min(st for (_, st, _) in segs)
                    wch = sum(n for (_, _, n) in segs)
                    t0w = w * WTL
                    c0 = int(cstart_md[b][t0w])
                    cs = small.tile([128, wch_max], bf16, tag="cs", name="cs",
                                    bufs=3)
                    nc.sync.dma_start(out=cs[:, :wch],
                                      in_=col_in[b][:, c0:c0 + wch])
                    ixs = small.tile([128, wch_max * 8], i16, tag="ixs",
                                     name="ixs", bufs=2)
                    nc.sync.dma_start(out=ixs[:, :wch * 8],
                                      in_=idx_in[b][:, g0 * 8:(g0 + wch) * 8])
                    gat = work.tile([128, wch_max * 128], bf16, tag="gat",
                                    name="gat")
                    gv = gat[:].rearrange("p (c e) -> p c e", e=128)
                    for (be, st, n) in segs:
                        for o in range(0, n, 8):
                            m = min(8, n - o)
                            so = st - g0 + o
                            nc.gpsimd.dma_gather(
                                out_ap=gv[:, so:so + m, :],
                                in_ap=t2f[b][be * BUCK:(be + 1) * BUCK, :],
                                idxs_ap=ixs[:, so * 8:(so + m) * 8],
                                num_idxs=m * 128,
                                num_idxs_reg=m * 128,
                                elem_size=WT,
                                queue_num=self_q[0] % NQ)
                            self_q[0] += 1
                    bld = IndBuilder(cs, c0, wch)
                    for t in range(t0w, min(t0w + WTL, NT)):
                        plist = tiles_md[b][t]
                        xt_ps = ppx.tile([E, 128], f32, tag="xt", name="xt")
                        for j, pos in enumerate(plist):
                            iv, r = bld.get(int(cstart_md[b][t]) + j)
                            nc.tensor.matmul(
                                out=xt_ps[:],
                                lhsT=gv[:, pos - g0, 0:E],
                                rhs=iv[:, :, r],
                                start=(j == 0), stop=(j == len(plist) - 1))
                        post_tile(b, t, xt_ps)
                        stg.stage(t)

            def post_tile(b, t, xt_ps):
                xts = small.tile([E, 128], bf16, tag="xts", name="xts")
                nc.scalar.copy(out=xts[:], in_=xt_ps[:])
                y_ps = ppy.tile([128, E], f32, tag="y", name="y")
                nc.tensor.matmul(out=y_ps[:], lhsT=xts[:],
                                 rhs=wsb[:, b * E:(b + 1) * E],
                                 start=True, stop=True)
                z = small.tile([128, E], f32, tag="z", name="z")
                ss = small.tile([128, 1], f32, tag="ss", name="ss")
                # z = y*dinv_col + bias
                nc.vector.scalar_tensor_tensor(
                    out=z[:], in0=y_ps[:], scalar=dinv_ap(b, t),
                    in1=bbsb[:, b * E:(b + 1) * E],
                    op0=ALU.mult, op1=ALU.add)
                sq = small.tile([128, E], f32, tag="sq", name="sq")
                nc.scalar.activation(out=sq[:], in_=z[:], func=AF.Square,
                                     accum_out=ss[:])
                # sqrt(ss + 1e-24) ~= max(sqrt(ss), 1e-12)
                nc.scalar.activation(out=ss[:], in_=ss[:], func=AF.Sqrt,
                                     bias=epsb[:, 0:1])
                rin = small.tile([128, 1], f32, tag="rin", name="rin")
                nc.vector.reciprocal(out=rin[:], in_=ss[:])
                totsl = tot[:, t * E:(t + 1) * E]
                # tot += z * rin
                nc.vector.scalar_tensor_tensor(
                    out=totsl, in0=z[:], scalar=rin[:, 0:1], in1=totsl,
                    op0=ALU.mult, op1=ALU.add)

            # ---------------- loss ----------------
            LOG1P_C = [2.4139025189026897e-09, 0.9999996692324197,
                       -0.499988759640371, 0.3331669190104936,
                       -0.2486582066434577, 0.19337637102999028,
                       -0.14517645896753417, 0.09470379566439587,
                       -0.04713346504062944, 0.015145372148722138,
                       -0.002288060381570317]

            def loss_pass(b):
                gs = []
                for k in range(3):
                    gk = small.tile([128, BJ * WT], bf16, tag=f"bg{k}",
                                    name=f"bg{k}")
                    gkv = gk[:].rearrange("p (j w) -> p j w", w=WT)
                    for j in range(BJ):
                        o = (b * 3 + k) * BJ + j
                        nc.gpsimd.indirect_dma_start(
                            out=gkv[:, j, :],
                            out_offset=None,
                            in_=t2f[b + 1][:],
                            in_offset=bass.IndirectOffsetOnAxis(
                                ap=bidx[:, o:o + 1], axis=0))
                    gs.append(gkv)
                prod = small.tile([128, BJ * E], f32, tag="prod", name="prod")
                pv = prod[:].rearrange("p (j e) -> p j e", e=E)
                sco = small.tile([128, 2 * BJ], f32, tag="sco", name="sco")
                for k in range(2):
                    nc.vector.tensor_tensor(out=pv, in0=gs[0][:, :, E:2 * E],
                                            in1=gs[k + 1][:, :, E:2 * E],
                                            op=ALU.mult)
                    nc.vector.tensor_reduce(
                        out=sco[:, k * BJ:(k + 1) * BJ], in_=pv,
                        axis=mybir.AxisListType.X, op=ALU.add)
                dd = small.tile([128, BJ], f32, tag="dd", name="dd")
                nc.vector.tensor_tensor(out=dd[:], in0=sco[:, 0:BJ],
                                        in1=sco[:, BJ:2 * BJ],
                                        op=ALU.subtract)
                aab = small.tile([128, BJ], f32, tag="aab", name="aab")
                nc.vector.tensor_scalar(out=aab[:], in0=dd[:], scalar1=-1.0,
                                        scalar2=None, op0=ALU.mult)
                nc.vector.tensor_tensor(out=aab[:], in0=aab[:], in1=dd[:],
                                        op=ALU.max)
                zex = small.tile([128, BJ], f32, tag="zex", name="zex")
                nc.scalar.activation(out=zex[:], in_=aab[:], func=AF.Exp,
                                     scale=-1.0)
                pol = small.tile([128, BJ], f32, tag="pol", name="pol")
                nc.vector.tensor_scalar(out=pol[:], in0=zex[:],
                                        scalar1=LOG1P_C[10],
                                        scalar2=LOG1P_C[9],
                                        op0=ALU.mult, op1=ALU.add)
                for k in range(8, -1, -1):
                    nc.vector.tensor_tensor(out=pol[:], in0=pol[:],
                                            in1=zex[:], op=ALU.mult)
                    nc.vector.tensor_scalar(out=pol[:], in0=pol[:],
                                            scalar1=LOG1P_C[k], scalar2=None,
                                            op0=ALU.add)
                nc.vector.tensor_scalar(out=dd[:], in0=dd[:], scalar1=-1.0,
                                        scalar2=0.0, op0=ALU.mult,
                                        op1=ALU.max)
                nc.vector.tensor_tensor(out=pol[:], in0=pol[:], in1=dd[:],
                                        op=ALU.add)
                nc.vector.tensor_reduce(out=blacc[:, b:b + 1], in_=pol[:],
                                        axis=mybir.AxisListType.X,
                                        op=ALU.add)

            # ================= program =================
            assemble(0)       # stages table 0 (no AGs)
            main_pass(0)      # AG(0) block, then gathers; stages table 1
            main_pass(1)      # AG(1) block first; stages table 2
            loss_pass(0)      # table 1 complete by now
            main_pass(2)      # AG(2) block first; stages table 3
            loss_pass(1)
            ag_block(3)
            loss_pass(2)

            # ---------------- final combine ----------------
            pack = small.tile([128, 2], f32, tag="pack", name="pack")
            nc.vector.tensor_reduce(out=pack[:, 0:1], in_=blacc[:],
                                    axis=mybir.AxisListType.X, op=ALU.add)
            nc.vector.tensor_reduce(out=pack[:, 1:2], in_=racc[:, :NREG],
                                    axis=mybir.AxisListType.X, op=ALU.add)
            fin_ps = ppy.tile([1, 2], f32, tag="fin", name="fin", bufs=1)
            nc.tensor.matmul(out=fin_ps[:], lhsT=onesf[:], rhs=pack[:],
                             start=True, stop=True)
            fin = small.tile([1, 2], f32, tag="fins", name="fins")
            nc.vector.tensor_copy(out=fin[:], in_=fin_ps[:])
            nc.sync.dma_start(out=lag_i[:], in_=fin[:])
            all_gather(lag_i, lag_o, ncores)
            lsb = small.tile([1, 2 * ncores], f32, tag="lsb", name="lsb")
            nc.sync.dma_start(
                out=lsb[:],
                in_=lag_o[:].rearrange("(o a) b -> o (a b)", o=1))
            bl = small.tile([1, 2], f32, tag="bl", name="bl")
            lv = lsb[:].rearrange("p (a b) -> p a b", b=2)
            nc.vector.tensor_reduce(out=bl[:, 0:1], in_=lv[:, :, 0:1],
                                    axis=mybir.AxisListType.XY, op=ALU.add)
            nc.vector.tensor_reduce(out=bl[:, 1:2], in_=lv[:, :, 1:2],
                                    axis=mybir.AxisListType.XY, op=ALU.add)
            res = small.tile([1, 1], f32, tag="res", name="res")
            nc.vector.tensor_scalar(out=res[:], in0=bl[:, 1:2],
                                    scalar1=cfg["reg_weight"] * 0.5,
                                    scalar2=None, op0=ALU.mult)
            nc.vector.tensor_tensor(out=res[:], in0=res[:], in1=bl[:, 0:1],
                                    op=ALU.add)
            nc.vector.tensor_scalar(out=res[:], in0=res[:],
                                    scalar1=1.0 / cfg["batch"],
                                    scalar2=None, op0=ALU.mult)
            nc.sync.dma_start(out=loss_out, in_=res[:])

    nc.compile()
    return nc


# ---------------------------------------------------------------------------
# Entry point
# ---------------------------------------------------------------------------
LAST_RESULTS = None


def kernel(**inputs) -> np.ndarray:
    global LAST_RESULTS
    cfg = FULL_CFG
    edges = np.asarray(inputs["edges"])
    arrs = make_schedule_and_arrays(edges, cfg)
    sched = arrs[0]
    in_maps = make_inputs_per_core(inputs, cfg, arrs)
    nc = build_program(cfg, sched)

    import os
    os.environ["BASS_NEVER_TRACE"] = "1"  # axon NTFF hook absent here
    from concourse import bass_utils
    res = bass_utils.run_bass_kernel_spmd(
        nc, in_maps, core_ids=list(range(cfg["ncores"])))
    LAST_RESULTS = res
    out = res.results[0]["loss"]
    return np.float32(out.reshape(-1)[0])



# revision 43
# speedup vs baseline: 1.0095x; 1.0095x over previous

# CRGCN multi-behavior GCN forward loss on 8 Trainium2 NeuronCores.
#
# Strategy (graph/data parallel, dest-node sharding):
#  - Nodes (users+items, 200000 -> padded 200704) are sharded row-wise across
#    8 cores (25088 = 196*128 nodes/core). Edges are partitioned by the shard
#    of their destination (col) node on the host, bucketed by (128-dest tile,
#    source bucket of 28672 rows) and padded so every 128-edge chunk maps to
#    one dest tile and one source bucket. The chunk schedule is the max over
#    cores so a single SPMD program fits all 8 cores.
#  - Per behavior each core holds a bf16 table T2 = [dinv*total | total]
#    ([200704, 128], 256B rows) for ALL nodes, produced by AllGather of
#    per-shard slabs. Message pass: dma_gather (int16 in-bucket indices) of
#    T2 rows for edge sources; a 0/1 one-hot (edge x dest-in-tile) built on
#    DVE from edge cols; PE matmul contracts edges, accumulating
#    S^T[feat, dest] = sum_e dinv[r_e]*total[r_e] x onehot in PSUM per dest
#    tile; then S @ W, *dinv[d], +b, l2-normalize, residual-accumulate into
#    the SBUF-resident fp32 total shard.
#  - deg (in-degree) is a one-hot x ones matmul (bf16, exact), per behavior,
#    from the same col data.
#  - BPR loss: batch rows sharded across cores; u/pos/neg rows fetched with
#    per-partition indirect DMA from the raw-total half of T2; dots +
#    softplus(-d) (relu + log1p poly) on-device; partials AllGathered so all
#    cores emit the identical final scalar.

import sys

sys.path.insert(0, "/opt/trn_rl_repo")

import dataclasses
import numpy as np

# ---------------- problem constants (hardcoded; kernel.py is standalone) ---
N_USERS = 100000
N_ITEMS = 100000
N_NODES = 200000
EMBED = 64
N_BEH = 3
BATCH = 4096
REG_WEIGHT = 1e-4
NCORES = 8

FULL_CFG = dict(
    ncores=NCORES,
    embed=EMBED,
    nbeh=N_BEH,
    shard=25088,          # 196 * 128
    nt=196,               # dest tiles per shard
    wt=128,               # T2 row width in bf16 elems (256B)
    nbuck=7,              # source buckets
    bucket=28672,         # rows per bucket (7 * 28672 = 200704)
    wtiles=8,             # dest tiles per gather window
    g=32,                 # chunks per one-hot build group
    flush=14,             # tiles per T2 staging flush (196 = 14*14)
    batch=BATCH,
    batch_per_core=BATCH // NCORES,   # 512
    n_nodes=N_NODES,
    reg_weight=REG_WEIGHT,
)


# ---------------------------------------------------------------------------
# Host-side preprocessing
# ---------------------------------------------------------------------------
def make_schedule_and_arrays(edges, cfg):
    """edges: [NB, 2, E]. Builds the (window, bucket, tile)-ordered common
    chunk schedule and the per-core col/idx arrays."""
    ncores = cfg["ncores"]
    NT = cfg["nt"]
    NB = cfg["nbeh"]
    NBK = cfg["nbuck"]
    BUCK = cfg["bucket"]
    WT = cfg["wtiles"]
    NW = (NT + WT - 1) // WT

    sched = {"C": [], "cells": [], "tiles": [], "windows": [],
             "tile_cstart": []}
    cols_arr = [[None] * NB for _ in range(ncores)]
    idx_arr = [[None] * NB for _ in range(ncores)]
    dinv_arr = np.zeros((ncores, NB, 128, NT), dtype=np.float32)

    n_nodes = cfg["n_nodes"]
    shard = cfg["shard"]
    for b in range(NB):
        row = np.asarray(edges[b, 0], dtype=np.int64)
        col = np.asarray(edges[b, 1], dtype=np.int64)
        # host-side in-degree -> dinv per core shard, [128, NT] layout
        deg = np.bincount(col, minlength=n_nodes).astype(np.float32)
        dinv_g = np.where(deg > 0,
                          1.0 / np.sqrt(np.maximum(deg, 1.0)),
                          0.0).astype(np.float32)
        dinv_pad = np.zeros(ncores * shard, dtype=np.float32)
        dinv_pad[:n_nodes] = dinv_g
        for s in range(ncores):
            dinv_arr[s, b] = dinv_pad[s * shard:(s + 1) * shard].reshape(
                NT, 128).T
        gt = col >> 7                       # global dest tile
        s_of = gt // NT                     # owning core
        t_of = gt - s_of * NT               # local dest tile
        # permuted table layout: bucket k holds piece k (PIECE local rows)
        # of every core's shard, so AllGather k is per-rank contiguous:
        # pos(n) = beta*BUCK + (n//SH)*PIECE + (n%SH)%PIECE,
        # beta = (n%SH)//PIECE
        PIECE = BUCK // ncores
        r_loc = row % shard
        beta = r_loc // PIECE               # source bucket (= piece id)
        # per (core, tile, bucket) counts
        cellkey = (s_of * NT + t_of) * NBK + beta
        cnt = np.bincount(cellkey, minlength=ncores * NT * NBK).reshape(
            ncores, NT, NBK)
        K_cell = -(-cnt.max(axis=0) // 128)           # [NT, NBK]
        empty_t = K_cell.sum(axis=1) == 0
        K_cell[empty_t, 0] = 1

        # gather order: (window, bucket, tile); consumption order:
        # (window, tile, bucket). Chunks get positions in both orders.
        C = int(K_cell.sum())
        cell_start = {}      # gather-order chunk start per cell
        cell_cstart = {}     # consumption-order chunk start per cell
        pos = 0
        for w in range(NW):
            ts = range(w * WT, min((w + 1) * WT, NT))
            for be in range(NBK):
                for t in ts:
                    if K_cell[t, be]:
                        cell_start[(t, be)] = pos
                        pos += int(K_cell[t, be])
        assert pos == C
        cpos = 0
        tile_cstart = np.zeros(NT + 1, dtype=np.int64)
        for w in range(NW):
            ts = range(w * WT, min((w + 1) * WT, NT))
            for t in ts:
                tile_cstart[t] = cpos
                for be in range(NBK):
                    if K_cell[t, be]:
                        cell_cstart[(t, be)] = cpos
                        cpos += int(K_cell[t, be])
        tile_cstart[NT] = cpos
        assert cpos == C

        # per-tile consumption: ordered chunk positions + total K per tile
        tiles = []
        for t in range(NT):
            plist = []
            for be in range(NBK):
                if K_cell[t, be]:
                    st = cell_start[(t, be)]
                    plist.extend(range(st, st + int(K_cell[t, be])))
            tiles.append(plist)

        # per-window gather segments: (bucket, pos_start, n_chunks)
        windows = []
        for w in range(NW):
            ts = range(w * WT, min((w + 1) * WT, NT))
            segs = []
            for be in range(NBK):
                n = int(sum(K_cell[t, be] for t in ts))
                if n:
                    st = min(cell_start[(t, be)] for t in ts
                             if K_cell[t, be])
                    segs.append((be, st, n))
            windows.append(segs)

        sched["C"].append(C)
        sched["cells"].append((K_cell, cell_start))
        sched["tiles"].append(tiles)
        sched["windows"].append(windows)
        sched["tile_cstart"].append(tile_cstart)

        # ------------- per-core arrays -------------
        starts_np = np.zeros((NT, NBK), dtype=np.int64)
        for (t, be), st in cell_start.items():
            starts_np[t, be] = st
        cstarts_np = np.zeros((NT, NBK), dtype=np.int64)
        for (t, be), st in cell_cstart.items():
            cstarts_np[t, be] = st
        for s in range(ncores):
            colv = np.full(C * 128, 128.0, dtype=np.float32)
            rowv = np.zeros(C * 128, dtype=np.int64)   # in-bucket idx
            sel = s_of == s
            r_s = row[sel]
            c_s = col[sel]
            t_s = t_of[sel]
            be_s = beta[sel]
            key = t_s * NBK + be_s
            order = np.argsort(key, kind="stable")
            r_s, c_s, t_s, be_s, key = (r_s[order], c_s[order], t_s[order],
                                        be_s[order], key[order])
            seg_start = np.searchsorted(key, np.arange(NT * NBK))
            within = np.arange(len(key)) - seg_start[key]
            dst = starts_np[t_s, be_s] * 128 + within
            cdst = cstarts_np[t_s, be_s] * 128 + within
            colv[cdst] = (c_s & 127).astype(np.float32)
            rowv[dst] = (r_s // shard) * (BUCK // ncores) \
                + (r_s % shard) % (BUCK // ncores)
            import ml_dtypes as _md
            cols_arr[s][b] = np.ascontiguousarray(
                colv.reshape(C, 128).T).astype(_md.bfloat16)   # [128, C]
            # idx16: [128, C*8]; gather element i -> [i%16 (+16k), off+i//16]
            iv = rowv.reshape(C * 128)
            i16 = np.zeros((16, C * 8), dtype=np.int16)
            ii = np.arange(C * 128)
            i16[ii % 16, ii // 16] = iv.astype(np.int16)
            idx_arr[s][b] = np.ascontiguousarray(np.tile(i16, (8, 1)))

    return sched, cols_arr, idx_arr, dinv_arr


def make_inputs_per_core(inputs, cfg, sched_arrays):
    import ml_dtypes

    ncores = cfg["ncores"]
    SH = cfg["shard"]
    E = cfg["embed"]
    NB = cfg["nbeh"]
    BPC = cfg["batch_per_core"]
    BJ = BPC // 128
    n_nodes = cfg["n_nodes"]
    n_users = n_nodes // 2

    sched, cols_arr, idx_arr, dinv_arr = sched_arrays

    user_emb = np.asarray(inputs["user_emb"], dtype=np.float32)
    item_emb = np.asarray(inputs["item_emb"], dtype=np.float32)
    gcn_weight = np.asarray(inputs["gcn_weight"], dtype=np.float32)
    gcn_bias = np.asarray(inputs["gcn_bias"], dtype=np.float32)
    batch_data = np.asarray(inputs["batch_data"], dtype=np.int64)

    total0 = np.concatenate([user_emb, item_emb], axis=0)

    G = cfg["g"]
    iotar = np.repeat(np.arange(128, dtype=np.float32), G)[None, :].astype(
        ml_dtypes.bfloat16)                       # [1, 128*G], j//G values
    w_bf = gcn_weight.astype(ml_dtypes.bfloat16)
    bb = np.tile(gcn_bias[:, None, :], (1, 128, 1)).astype(np.float32)

    in_maps = []
    for s in range(ncores):
        lo = s * SH
        hi = min((s + 1) * SH, n_nodes)
        init_shard = np.zeros((SH, E), dtype=np.float32)
        if hi > lo:
            init_shard[: hi - lo] = total0[lo:hi]

        PIECE = cfg["bucket"] // ncores

        def pos(n):
            # node id -> permuted table row (see make_schedule_and_arrays)
            r = n % SH
            return (r // PIECE) * cfg["bucket"] + (n // SH) * PIECE \
                + r % PIECE

        bidx = np.zeros((NB * 3, 128, BJ), dtype=np.int32)
        rs = slice(s * BPC, (s + 1) * BPC)
        for b in range(NB):
            u = pos(batch_data[rs, b, 0].astype(np.int32))
            p = pos(batch_data[rs, b, 1].astype(np.int32) + n_users)
            n = pos(batch_data[rs, b, 2].astype(np.int32) + n_users)
            for k, v in enumerate((u, p, n)):
                bidx[b * 3 + k] = v.reshape(BJ, 128).T

        m = {
            "init_shard": init_shard,
            "iotar_in": iotar,
            "w_in": w_bf,
            "bb_in": bb,
            "bidx_in": bidx,
            "dinv_in": dinv_arr[s],
        }
        for b in range(NB):
            m[f"col{b}"] = cols_arr[s][b]
            m[f"idx{b}"] = idx_arr[s][b]
        in_maps.append(m)
    return in_maps


# ---------------------------------------------------------------------------
# Device program
# ---------------------------------------------------------------------------
def build_program(cfg, sched, sim=False):
    from concourse import bass, bacc, mybir, tile

    dt = mybir.dt
    AF = mybir.ActivationFunctionType
    ALU = mybir.AluOpType

    ncores = cfg["ncores"]
    NT = cfg["nt"]
    SH = cfg["shard"]
    NTOT = SH * ncores
    E = cfg["embed"]
    WT = cfg["wt"]            # 128 table cols
    NBK = cfg["nbuck"]
    BUCK = cfg["bucket"]
    WTL = cfg["wtiles"]
    G = cfg["g"]
    FLUSH = cfg["flush"]
    BPC = cfg["batch_per_core"]
    BJ = BPC // 128
    NB = cfg["nbeh"]
    NV = NB + 1
    NW = (NT + WTL - 1) // WTL

    C = sched["C"]
    tiles_md = sched["tiles"]
    windows_md = sched["windows"]
    cstart_md = sched["tile_cstart"]

    # max chunks in any window (for the staging tile size)
    wch_max = 0
    for b in range(NB):
        for w in range(NW):
            wch = sum(n for (_, _, n) in windows_md[b][w])
            wch_max = max(wch_max, wch)

    def bc(ap, where, n):
        newap = list(ap.ap)
        newap.insert(where, [0, n])
        return dataclasses.replace(ap, ap=newap)

    nc = bacc.Bacc("TRN2", target_bir_lowering=False, debug=False,
                   num_devices=1 if sim is True else ncores,
                   num_swdge_queues=4)

    def all_gather(src_tile, dst_tile, nrep):
        # sim mode: stand in for the collective with local HBM->HBM copies
        # of the same receive volume so TimelineSim can run (single-core,
        # no collectives) with comparable DMA load + dependencies.
        if sim is True:
            n = src_tile.shape[0]
            for r in range(nrep):
                nc.sync.dma_start(out=dst_tile[r * n:(r + 1) * n, :],
                                  in_=src_tile[:])
        else:
            nc.gpsimd.collective_compute(
                "AllGather", mybir.AluOpType.bypass,
                replica_groups=[list(range(nrep))],
                ins=[src_tile[:].opt()], outs=[dst_tile[:].opt()])

    f32, bf16, i32, i16 = dt.float32, dt.bfloat16, dt.int32, dt.int16
    shared = "Local"

    init_in = nc.dram_tensor("init_shard", [SH, E], f32,
                             kind="ExternalInput").ap()
    iotar_in = nc.dram_tensor("iotar_in", [1, 128 * G], bf16,
                              kind="ExternalInput").ap()
    dinv_in = nc.dram_tensor("dinv_in", [NB, 128, NT], f32,
                             kind="ExternalInput").ap()
    w_in = nc.dram_tensor("w_in", [NB, E, E], bf16, kind="ExternalInput").ap()
    bb_in = nc.dram_tensor("bb_in", [NB, 128, E], f32,
                           kind="ExternalInput").ap()
    bidx_in = nc.dram_tensor("bidx_in", [NB * 3, 128, BJ], i32,
                             kind="ExternalInput").ap()
    col_in = [nc.dram_tensor(f"col{b}", [128, C[b]], bf16,
                             kind="ExternalInput").ap() for b in range(NB)]
    idx_in = [nc.dram_tensor(f"idx{b}", [128, C[b] * 8], i16,
                             kind="ExternalInput").ap() for b in range(NB)]
    loss_out = nc.dram_tensor("loss", [1, 1], f32, kind="ExternalOutput").ap()

    with tile.TileContext(nc) as tc:
        with (
            tc.tile_pool(name="dram", bufs=1, space="DRAM") as dpool,
            tc.tile_pool(name="pers", bufs=1) as pers,
            tc.tile_pool(name="work", bufs=2) as work,
            tc.tile_pool(name="small", bufs=4) as small,
            tc.tile_pool(name="ppx", bufs=3, space="PSUM") as ppx,
            tc.tile_pool(name="ppy", bufs=2, space="PSUM") as ppy,
        ):
            t2s = [dpool.tile([SH, WT], bf16, tag=f"t2s{v}",
                              name=f"t2s{v}") for v in range(NV)]
            t2f = [dpool.tile([NTOT, WT], bf16, tag=f"t2f{v}",
                              name=f"t2f{v}", addr_space=shared)
                   for v in range(NV)]
            lag_i = dpool.tile([1, 2], f32, tag="lag_i", name="lag_i")
            lag_o = dpool.tile([ncores, 2], f32, tag="lag_o", name="lag_o",
                               addr_space=shared)

            tot = pers.tile([128, NT * E], f32, tag="tot", name="tot")
            irep = pers.tile([128, 128 * G], bf16, tag="irep", name="irep")
            wsb = pers.tile([E, NB * E], bf16, tag="wsb", name="wsb")
            bbsb = pers.tile([128, NB * E], f32, tag="bbsb", name="bbsb")
            bidx = pers.tile([128, NB * 3 * BJ], i32, tag="bidx", name="bidx")
            dinvsb = pers.tile([128, NB * NT], f32, tag="dinvsb",
                               name="dinvsb")
            dinv3z = pers.tile([128, 1], f32, tag="dinv3z", name="dinv3z")
            onesf = pers.tile([128, 1], f32, tag="onesf", name="onesf")
            racc = pers.tile([128, 16], f32, tag="racc", name="racc")
            blacc = pers.tile([128, NB], f32, tag="blacc", name="blacc")

            nc.sync.dma_start(
                out=irep[:].rearrange("p (a x) -> p a x", a=1),
                in_=bc(iotar_in, 0, 128))
            nc.sync.dma_start(
                out=dinvsb[:].rearrange("p (b t) -> p b t", b=NB),
                in_=dinv_in.rearrange("b p t -> p b t"))
            nc.sync.dma_start(
                out=wsb[:].rearrange("k (b e) -> k b e", b=NB),
                in_=w_in.rearrange("b k e -> k b e"))
            nc.sync.dma_start(
                out=bbsb[:].rearrange("p (b e) -> p b e", b=NB),
                in_=bb_in.rearrange("b p e -> p b e"))
            nc.sync.dma_start(
                out=bidx[:].rearrange("p (a j) -> p a j", a=NB * 3),
                in_=bidx_in.rearrange("a p j -> p a j"))
            nc.sync.dma_start(
                out=tot[:].rearrange("p (t e) -> p t e", e=E),
                in_=init_in.rearrange("(t p) e -> p t e", p=128))
            nc.vector.memset(onesf[:], 1.0)
            nc.vector.memset(dinv3z[:], 0.0)
            epsb = pers.tile([128, 1], f32, tag="epsb", name="epsb")
            nc.vector.memset(epsb[:], 1e-24)

            def dinv_ap(v, t):
                # per-partition dinv scalar for (behavior v, dest tile t)
                if v < NB:
                    return dinvsb[:, v * NT + t:v * NT + t + 1]
                return dinv3z[:, 0:1]

            # reg term: sum of squares of the initial embeddings
            NREG = (NT * E + 1023) // 1024
            sqd = pers.tile([128, 1024], f32, tag="sqd", name="sqd")
            for i in range(NREG):
                sl = slice(i * 1024, min((i + 1) * 1024, NT * E))
                nc.scalar.activation(out=sqd[:, : sl.stop - sl.start],
                                     in_=tot[:, sl], func=AF.Square,
                                     accum_out=racc[:, i:i + 1])

            # ------- lazy consumption-ordered one-hot group builder -------
            class IndBuilder:
                """Builds one-hot groups for consumption positions
                [c0, c0+wch) on demand, in order, so only a few groups are
                live at once. Layout [p, d, g] (g innermost) so every
                operand of the is_equal has stride-1 innermost dims and the
                DVE runs in its 2x perf mode."""

                def __init__(self, cs, c0, wch):
                    self.cs, self.c0, self.wch = cs, c0, wch
                    self.groups = {}

                def get(self, cpos):
                    rel = cpos - self.c0
                    g0 = (rel // G) * G
                    if g0 not in self.groups:
                        gw = min(G, self.wch - g0)
                        ind = work.tile([128, 128 * G], dt.bfloat16,
                                        tag="ind", name="ind", bufs=3)
                        iv = ind[:].rearrange("p (d g) -> p d g", g=G)
                        nc.vector.tensor_tensor(
                            out=iv[:, :, :gw],
                            in0=irep[:].rearrange(
                                "p (d g) -> p d g", g=G)[:, :, :gw],
                            in1=bc(self.cs[:, g0:g0 + gw], 1, 128),
                            op=ALU.is_equal)
                        self.groups[g0] = iv
                    return self.groups[g0], rel - g0

            # ------------- T2 staging + per-piece AllGather -------------
            # Table v is built tile-by-tile (fused into main_pass(v-1)'s
            # post_tile stream); every 2 flushes completes one PIECE of the
            # local slab and fires that piece's AllGather, so collectives
            # overlap the remaining compute of the producing behavior.
            PIECE = BUCK // ncores

            def ag_piece(v, k):
                src = t2s[v][k * PIECE:(k + 1) * PIECE, :]
                if sim == "noag":
                    return
                if sim:
                    for r in range(ncores):
                        o = k * BUCK + r * PIECE
                        nc.sync.dma_start(out=t2f[v][o:o + PIECE, :],
                                          in_=src)
                else:
                    nc.gpsimd.collective_compute(
                        "AllGather", mybir.AluOpType.bypass,
                        replica_groups=[list(range(ncores))],
                        ins=[src.opt()],
                        outs=[t2f[v][k * BUCK:(k + 1) * BUCK, :].opt()])

            class Stager:
                def __init__(self, v):
                    self.v = v
                    self.s65 = None

                def stage(self, t):
                    i = t % FLUSH
                    if i == 0:
                        self.s65 = work.tile([128, FLUSH * WT], bf16,
                                             tag="s65", name="s65")
                    totsl = tot[:, t * E:(t + 1) * E]
                    nc.vector.tensor_scalar(
                        out=self.s65[:, i * WT:i * WT + E], in0=totsl,
                        scalar1=dinv_ap(self.v, t), scalar2=None,
                        op0=ALU.mult)
                    nc.scalar.copy(
                        out=self.s65[:, i * WT + E:i * WT + 2 * E],
                        in_=totsl)
                    if i == FLUSH - 1:
                        tf = t - i
                        nc.sync.dma_start(
                            out=t2s[self.v][:].rearrange(
                                "(t p) w -> p t w", p=128)[:, tf:t + 1, :],
                            in_=self.s65[:, :FLUSH * WT].rearrange(
                                "p (t w) -> p t w", w=WT))

            def assemble(v):
                st = Stager(v)
                for t in range(NT):
                    st.stage(t)

            # ---------------- main pass ----------------
            self_q = [0]
            NQ = 4

            def ag_block(v):
                # all 7 piece triggers up-front: their input flushes
                # completed during the previous pass, so these don't stall
                # the in-order Pool queue, and the transfers stream ahead
                # of the bucket-ordered gathers that consume them.
                for k in range(NBK):
                    ag_piece(v, k)

            def main_pass(b):
                ag_block(b)
                stg = Stager(b + 1)
                for w in range(NW):
                    segs = windows_md[b][w]
                    g0 = min(st for (_, st, _) in segs)
                    wch = sum(n for (_, _, n) in segs)
                    t0w = w * WTL
                    c0 = int(cstart_md[b][t0w])
                    cs = small.tile([128, wch_max], bf16, tag="cs", name="cs",
                                    bufs=3)
                    nc.sync.dma_start(out=cs[:, :wch],
                                      in_=col_in[b][:, c0:c0 + wch])
                    ixs = small.tile([128, wch_max * 8], i16, tag="ixs",
                                     name="ixs", bufs=2)
                    nc.sync.dma_start(out=ixs[:, :wch * 8],
                                      in_=idx_in[b][:, g0 * 8:(g0 + wch) * 8])
                    gat = work.tile([128, wch_max * 128], bf16, tag="gat",
                                    name="gat")
                    gv = gat[:].rearrange("p (c e) -> p c e", e=128)
                    for (be, st, n) in segs:
                        for o in range(0, n, 8):
                            if sim == "nogather":
                                break
                            m = min(8, n - o)
                            so = st - g0 + o
                            nc.gpsimd.dma_gather(
                                out_ap=gv[:, so:so + m, :],
                                in_ap=t2f[b][be * BUCK:(be + 1) * BUCK, :],
                                idxs_ap=ixs[:, so * 8:(so + m) * 8],
                                num_idxs=m * 128,
                                num_idxs_reg=m * 128,
                                elem_size=WT,
                                queue_num=self_q[0] % NQ)
                            self_q[0] += 1
                    bld = IndBuilder(cs, c0, wch)
                    for t in range(t0w, min(t0w + WTL, NT)):
                        plist = tiles_md[b][t]
                        xt_ps = ppx.tile([E, 128], f32, tag="xt", name="xt")
                        for j, pos in enumerate(plist):
                            iv, r = bld.get(int(cstart_md[b][t]) + j)
                            nc.tensor.matmul(
                                out=xt_ps[:],
                                lhsT=gv[:, pos - g0, 0:E],
                                rhs=iv[:, :, r],
                                start=(j == 0), stop=(j == len(plist) - 1))
                        post_tile(b, t, xt_ps)
                        stg.stage(t)

            def post_tile(b, t, xt_ps):
                xts = small.tile([E, 128], bf16, tag="xts", name="xts")
                nc.scalar.copy(out=xts[:], in_=xt_ps[:])
                y_ps = ppy.tile([128, E], f32, tag="y", name="y")
                nc.tensor.matmul(out=y_ps[:], lhsT=xts[:],
                                 rhs=wsb[:, b * E:(b + 1) * E],
                                 start=True, stop=True)
                z = small.tile([128, E], f32, tag="z", name="z")
                ss = small.tile([128, 1], f32, tag="ss", name="ss")
                # z = y*dinv_col + bias
                nc.vector.scalar_tensor_tensor(
                    out=z[:], in0=y_ps[:], scalar=dinv_ap(b, t),
                    in1=bbsb[:, b * E:(b + 1) * E],
                    op0=ALU.mult, op1=ALU.add)
                sq = small.tile([128, E], f32, tag="sq", name="sq")
                nc.scalar.activation(out=sq[:], in_=z[:], func=AF.Square,
                                     accum_out=ss[:])
                # sqrt(ss + 1e-24) ~= max(sqrt(ss), 1e-12)
                nc.scalar.activation(out=ss[:], in_=ss[:], func=AF.Sqrt,
                                     bias=epsb[:, 0:1])
                rin = small.tile([128, 1], f32, tag="rin", name="rin")
                nc.vector.reciprocal(out=rin[:], in_=ss[:])
                totsl = tot[:, t * E:(t + 1) * E]
                # tot += z * rin
                nc.vector.scalar_tensor_tensor(
                    out=totsl, in0=z[:], scalar=rin[:, 0:1], in1=totsl,
                    op0=ALU.mult, op1=ALU.add)

            # ---------------- loss ----------------
            LOG1P_C = [2.4139025189026897e-09, 0.9999996692324197,
                       -0.499988759640371, 0.3331669190104936,
                       -0.2486582066434577, 0.19337637102999028,
                       -0.14517645896753417, 0.09470379566439587,
                       -0.04713346504062944, 0.015145372148722138,
                       -0.002288060381570317]

            def loss_pass(b):
                gs = []
                for k in range(3):
                    gk = small.tile([128, BJ * WT], bf16, tag=f"bg{k}",
                                    name=f"bg{k}")
                    gkv = gk[:].rearrange("p (j w) -> p j w", w=WT)
                    for j in range(BJ):
                        o = (b * 3 + k) * BJ + j
                        nc.gpsimd.indirect_dma_start(
                            out=gkv[:, j, :],
                            out_offset=None,
                            in_=t2f[b + 1][:],
                            in_offset=bass.IndirectOffsetOnAxis(
                                ap=bidx[:, o:o + 1], axis=0))
                    gs.append(gkv)
                prod = small.tile([128, BJ * E], f32, tag="prod", name="prod")
                pv = prod[:].rearrange("p (j e) -> p j e", e=E)
                sco = small.tile([128, 2 * BJ], f32, tag="sco", name="sco")
                for k in range(2):
                    nc.vector.tensor_tensor(out=pv, in0=gs[0][:, :, E:2 * E],
                                            in1=gs[k + 1][:, :, E:2 * E],
                                            op=ALU.mult)
                    nc.vector.tensor_reduce(
                        out=sco[:, k * BJ:(k + 1) * BJ], in_=pv,
                        axis=mybir.AxisListType.X, op=ALU.add)
                dd = small.tile([128, BJ], f32, tag="dd", name="dd")
                nc.vector.tensor_tensor(out=dd[:], in0=sco[:, 0:BJ],
                                        in1=sco[:, BJ:2 * BJ],
                                        op=ALU.subtract)
                aab = small.tile([128, BJ], f32, tag="aab", name="aab")
                nc.vector.tensor_scalar(out=aab[:], in0=dd[:], scalar1=-1.0,
                                        scalar2=None, op0=ALU.mult)
                nc.vector.tensor_tensor(out=aab[:], in0=aab[:], in1=dd[:],
                                        op=ALU.max)
                zex = small.tile([128, BJ], f32, tag="zex", name="zex")
                nc.scalar.activation(out=zex[:], in_=aab[:], func=AF.Exp,
                                     scale=-1.0)
                pol = small.tile([128, BJ], f32, tag="pol", name="pol")
                nc.vector.tensor_scalar(out=pol[:], in0=zex[:],
                                        scalar1=LOG1P_C[10],
                                        scalar2=LOG1P_C[9],
                                        op0=ALU.mult, op1=ALU.add)
                for k in range(8, -1, -1):
                    nc.vector.tensor_tensor(out=pol[:], in0=pol[:],
                                            in1=zex[:], op=ALU.mult)
                    nc.vector.tensor_scalar(out=pol[:], in0=pol[:],
                                            scalar1=LOG1P_C[k], scalar2=None,
                                            op0=ALU.add)
                nc.vector.tensor_scalar(out=dd[:], in0=dd[:], scalar1=-1.0,
                                        scalar2=0.0, op0=ALU.mult,
                                        op1=ALU.max)
                nc.vector.tensor_tensor(out=pol[:], in0=pol[:], in1=dd[:],
                                        op=ALU.add)
                nc.vector.tensor_reduce(out=blacc[:, b:b + 1], in_=pol[:],
                                        axis=mybir.AxisListType.X,
                                        op=ALU.add)

            # ================= program =================
            assemble(0)       # stages table 0 (no AGs)
            main_pass(0)      # AG(0) block, then gathers; stages table 1
            main_pass(1)      # AG(1) block first; stages table 2
            loss_pass(0)      # table 1 complete by now
            main_pass(2)      # AG(2) block first; stages table 3
            loss_pass(1)
            ag_block(3)
            loss_pass(2)

            # ---------------- final combine ----------------
            pack = small.tile([128, 2], f32, tag="pack", name="pack")
            nc.vector.tensor_reduce(out=pack[:, 0:1], in_=blacc[:],
                                    axis=mybir.AxisListType.X, op=ALU.add)
            nc.vector.tensor_reduce(out=pack[:, 1:2], in_=racc[:, :NREG],
                                    axis=mybir.AxisListType.X, op=ALU.add)
            fin_ps = ppy.tile([1, 2], f32, tag="fin", name="fin", bufs=1)
            nc.tensor.matmul(out=fin_ps[:], lhsT=onesf[:], rhs=pack[:],
                             start=True, stop=True)
            fin = small.tile([1, 2], f32, tag="fins", name="fins")
            nc.vector.tensor_copy(out=fin[:], in_=fin_ps[:])
            nc.sync.dma_start(out=lag_i[:], in_=fin[:])
            all_gather(lag_i, lag_o, ncores)
            lsb = small.tile([1, 2 * ncores], f32, tag="lsb", name="lsb")
            nc.sync.dma_start(
                out=lsb[:],
                in_=lag_o[:].rearrange("(o a) b -> o (a b)", o=1))
            bl = small.tile([1, 2], f32, tag="bl", name="bl")
            lv = lsb[:].rearrange("p (a b) -> p a b", b=2)
            nc.vector.tensor_reduce(out=bl[:, 0:1], in_=lv[:, :, 0:1],
                                    axis=mybir.AxisListType.XY, op=ALU.add)
            nc.vector.tensor_reduce(out=bl[:, 1:2], in_=lv[:, :, 1:2],
                                    axis=mybir.AxisListType.XY, op=ALU.add)
            res = small.tile([1, 1], f32, tag="res", name="res")
            nc.vector.tensor_scalar(out=res[:], in0=bl[:, 1:2],
                                    scalar1=cfg["reg_weight"] * 0.5,
                                    scalar2=None, op0=ALU.mult)
            nc.vector.tensor_tensor(out=res[:], in0=res[:], in1=bl[:, 0:1],
                                    op=ALU.add)
            nc.vector.tensor_scalar(out=res[:], in0=res[:],
                                    scalar1=1.0 / cfg["batch"],
                                    scalar2=None, op0=ALU.mult)
            nc.sync.dma_start(out=loss_out, in_=res[:])

    nc.compile()
    return nc


# ---------------------------------------------------------------------------
# Entry point
# ---------------------------------------------------------------------------
LAST_RESULTS = None


def kernel(**inputs) -> np.ndarray:
    global LAST_RESULTS
    cfg = FULL_CFG
    edges = np.asarray(inputs["edges"])
    arrs = make_schedule_and_arrays(edges, cfg)
    sched = arrs[0]
    in_maps = make_inputs_per_core(inputs, cfg, arrs)
    nc = build_program(cfg, sched)

    import os
    os.environ["BASS_NEVER_TRACE"] = "1"  # axon NTFF hook absent here
    from concourse import bass_utils
    res = bass_utils.run_bass_kernel_spmd(
        nc, in_maps, core_ids=list(range(cfg["ncores"])))
    LAST_RESULTS = res
    out = res.results[0]["loss"]
    return np.float32(out.reshape(-1)[0])



# revision 47
# speedup vs baseline: 1.1271x; 1.1165x over previous

# CRGCN multi-behavior GCN forward loss on 8 Trainium2 NeuronCores.
#
# Strategy (graph/data parallel, dest-node sharding):
#  - Nodes (users+items, 200000 -> padded 200704) are sharded row-wise across
#    8 cores (25088 = 196*128 nodes/core). Edges are partitioned by the shard
#    of their destination (col) node on the host, bucketed by (128-dest tile,
#    source bucket of 28672 rows) and padded so every 128-edge chunk maps to
#    one dest tile and one source bucket. The chunk schedule is the max over
#    cores so a single SPMD program fits all 8 cores.
#  - Per behavior each core holds a bf16 table T2 = [dinv*total | total]
#    ([200704, 128], 256B rows) for ALL nodes, produced by AllGather of
#    per-shard slabs. Message pass: dma_gather (int16 in-bucket indices) of
#    T2 rows for edge sources; a 0/1 one-hot (edge x dest-in-tile) built on
#    DVE from edge cols; PE matmul contracts edges, accumulating
#    S^T[feat, dest] = sum_e dinv[r_e]*total[r_e] x onehot in PSUM per dest
#    tile; then S @ W, *dinv[d], +b, l2-normalize, residual-accumulate into
#    the SBUF-resident fp32 total shard.
#  - deg (in-degree) is a one-hot x ones matmul (bf16, exact), per behavior,
#    from the same col data.
#  - BPR loss: batch rows sharded across cores; u/pos/neg rows fetched with
#    per-partition indirect DMA from the raw-total half of T2; dots +
#    softplus(-d) (relu + log1p poly) on-device; partials AllGathered so all
#    cores emit the identical final scalar.

import sys

sys.path.insert(0, "/opt/trn_rl_repo")

import dataclasses
import numpy as np

# ---------------- problem constants (hardcoded; kernel.py is standalone) ---
N_USERS = 100000
N_ITEMS = 100000
N_NODES = 200000
EMBED = 64
N_BEH = 3
BATCH = 4096
REG_WEIGHT = 1e-4
NCORES = 8

FULL_CFG = dict(
    ncores=NCORES,
    embed=EMBED,
    nbeh=N_BEH,
    shard=25088,          # 196 * 128
    nt=196,               # dest tiles per shard
    wt=128,               # T2 row width in bf16 elems (256B)
    nbuck=7,              # source buckets
    bucket=28672,         # rows per bucket (7 * 28672 = 200704)
    wtiles=8,             # dest tiles per gather window
    g=32,                 # chunks per one-hot build group
    flush=14,             # tiles per T2 staging flush (196 = 14*14)
    batch=BATCH,
    batch_per_core=BATCH // NCORES,   # 512
    n_nodes=N_NODES,
    reg_weight=REG_WEIGHT,
)


# ---------------------------------------------------------------------------
# Host-side preprocessing
# ---------------------------------------------------------------------------
def make_schedule_and_arrays(edges, cfg):
    """edges: [NB, 2, E]. Builds the (window, bucket, tile)-ordered common
    chunk schedule and the per-core col/idx arrays."""
    ncores = cfg["ncores"]
    NT = cfg["nt"]
    NB = cfg["nbeh"]
    NBK = cfg["nbuck"]
    BUCK = cfg["bucket"]
    WT = cfg["wtiles"]
    NW = (NT + WT - 1) // WT

    sched = {"C": [], "cells": [], "tiles": [], "windows": [],
             "tile_cstart": []}
    cols_arr = [[None] * NB for _ in range(ncores)]
    idx_arr = [[None] * NB for _ in range(ncores)]
    dinv_arr = np.zeros((ncores, NB, 128, NT), dtype=np.float32)

    n_nodes = cfg["n_nodes"]
    shard = cfg["shard"]
    for b in range(NB):
        row = np.asarray(edges[b, 0], dtype=np.int64)
        col = np.asarray(edges[b, 1], dtype=np.int64)
        # host-side in-degree -> dinv per core shard, [128, NT] layout
        deg = np.bincount(col, minlength=n_nodes).astype(np.float32)
        dinv_g = np.where(deg > 0,
                          1.0 / np.sqrt(np.maximum(deg, 1.0)),
                          0.0).astype(np.float32)
        dinv_pad = np.zeros(ncores * shard, dtype=np.float32)
        dinv_pad[:n_nodes] = dinv_g
        for s in range(ncores):
            dinv_arr[s, b] = dinv_pad[s * shard:(s + 1) * shard].reshape(
                NT, 128).T
        gt = col >> 7                       # global dest tile
        s_of = gt // NT                     # owning core
        t_of = gt - s_of * NT               # local dest tile
        # permuted table layout: bucket k holds piece k (PIECE local rows)
        # of every core's shard, so AllGather k is per-rank contiguous:
        # pos(n) = beta*BUCK + (n//SH)*PIECE + (n%SH)%PIECE,
        # beta = (n%SH)//PIECE
        PIECE = BUCK // ncores
        r_loc = row % shard
        beta = r_loc // PIECE               # source bucket (= piece id)
        # per (core, tile, bucket) counts
        cellkey = (s_of * NT + t_of) * NBK + beta
        cnt = np.bincount(cellkey, minlength=ncores * NT * NBK).reshape(
            ncores, NT, NBK)
        K_cell = -(-cnt.max(axis=0) // 128)           # [NT, NBK]
        empty_t = K_cell.sum(axis=1) == 0
        K_cell[empty_t, 0] = 1

        # gather order: (window, bucket, tile); consumption order:
        # (window, tile, bucket). Chunks get positions in both orders.
        C = int(K_cell.sum())
        cell_start = {}      # gather-order chunk start per cell
        cell_cstart = {}     # consumption-order chunk start per cell
        pos = 0
        for w in range(NW):
            ts = range(w * WT, min((w + 1) * WT, NT))
            for be in range(NBK):
                for t in ts:
                    if K_cell[t, be]:
                        cell_start[(t, be)] = pos
                        pos += int(K_cell[t, be])
        assert pos == C
        cpos = 0
        tile_cstart = np.zeros(NT + 1, dtype=np.int64)
        for w in range(NW):
            ts = range(w * WT, min((w + 1) * WT, NT))
            for t in ts:
                tile_cstart[t] = cpos
                for be in range(NBK):
                    if K_cell[t, be]:
                        cell_cstart[(t, be)] = cpos
                        cpos += int(K_cell[t, be])
        tile_cstart[NT] = cpos
        assert cpos == C

        # per-tile consumption: ordered chunk positions + total K per tile
        tiles = []
        for t in range(NT):
            plist = []
            for be in range(NBK):
                if K_cell[t, be]:
                    st = cell_start[(t, be)]
                    plist.extend(range(st, st + int(K_cell[t, be])))
            tiles.append(plist)

        # per-window gather segments: (bucket, pos_start, n_chunks)
        windows = []
        for w in range(NW):
            ts = range(w * WT, min((w + 1) * WT, NT))
            segs = []
            for be in range(NBK):
                n = int(sum(K_cell[t, be] for t in ts))
                if n:
                    st = min(cell_start[(t, be)] for t in ts
                             if K_cell[t, be])
                    segs.append((be, st, n))
            windows.append(segs)

        sched["C"].append(C)
        sched["cells"].append((K_cell, cell_start))
        sched["tiles"].append(tiles)
        sched["windows"].append(windows)
        sched["tile_cstart"].append(tile_cstart)

        # ------------- per-core arrays -------------
        starts_np = np.zeros((NT, NBK), dtype=np.int64)
        for (t, be), st in cell_start.items():
            starts_np[t, be] = st
        cstarts_np = np.zeros((NT, NBK), dtype=np.int64)
        for (t, be), st in cell_cstart.items():
            cstarts_np[t, be] = st
        for s in range(ncores):
            colv = np.full(C * 128, 128.0, dtype=np.float32)
            rowv = np.zeros(C * 128, dtype=np.int64)   # in-bucket idx
            sel = s_of == s
            r_s = row[sel]
            c_s = col[sel]
            t_s = t_of[sel]
            be_s = beta[sel]
            key = t_s * NBK + be_s
            order = np.argsort(key, kind="stable")
            r_s, c_s, t_s, be_s, key = (r_s[order], c_s[order], t_s[order],
                                        be_s[order], key[order])
            seg_start = np.searchsorted(key, np.arange(NT * NBK))
            within = np.arange(len(key)) - seg_start[key]
            dst = starts_np[t_s, be_s] * 128 + within
            cdst = cstarts_np[t_s, be_s] * 128 + within
            colv[cdst] = (c_s & 127).astype(np.float32)
            rowv[dst] = (r_s // shard) * (BUCK // ncores) \
                + (r_s % shard) % (BUCK // ncores)
            import ml_dtypes as _md
            cols_arr[s][b] = np.ascontiguousarray(
                colv.reshape(C, 128).T).astype(_md.bfloat16)   # [128, C]
            # idx16: [128, C*8]; gather element i -> [i%16 (+16k), off+i//16]
            iv = rowv.reshape(C * 128)
            i16 = np.zeros((16, C * 8), dtype=np.int16)
            ii = np.arange(C * 128)
            i16[ii % 16, ii // 16] = iv.astype(np.int16)
            idx_arr[s][b] = np.ascontiguousarray(np.tile(i16, (8, 1)))

    return sched, cols_arr, idx_arr, dinv_arr


def make_inputs_per_core(inputs, cfg, sched_arrays):
    import ml_dtypes

    ncores = cfg["ncores"]
    SH = cfg["shard"]
    E = cfg["embed"]
    NB = cfg["nbeh"]
    BPC = cfg["batch_per_core"]
    BJ = BPC // 128
    n_nodes = cfg["n_nodes"]
    n_users = n_nodes // 2

    sched, cols_arr, idx_arr, dinv_arr = sched_arrays

    user_emb = np.asarray(inputs["user_emb"], dtype=np.float32)
    item_emb = np.asarray(inputs["item_emb"], dtype=np.float32)
    gcn_weight = np.asarray(inputs["gcn_weight"], dtype=np.float32)
    gcn_bias = np.asarray(inputs["gcn_bias"], dtype=np.float32)
    batch_data = np.asarray(inputs["batch_data"], dtype=np.int64)

    total0 = np.concatenate([user_emb, item_emb], axis=0)

    G = cfg["g"]
    iotar = np.repeat(np.arange(128, dtype=np.float32), G)[None, :].astype(
        ml_dtypes.bfloat16)                       # [1, 128*G], j//G values
    w_bf = gcn_weight.astype(ml_dtypes.bfloat16)
    bb = np.tile(gcn_bias[:, None, :], (1, 128, 1)).astype(np.float32)

    in_maps = []
    for s in range(ncores):
        lo = s * SH
        hi = min((s + 1) * SH, n_nodes)
        init_shard = np.zeros((SH, E), dtype=np.float32)
        if hi > lo:
            init_shard[: hi - lo] = total0[lo:hi]

        PIECE = cfg["bucket"] // ncores

        def pos(n):
            # node id -> permuted table row (see make_schedule_and_arrays)
            r = n % SH
            return (r // PIECE) * cfg["bucket"] + (n // SH) * PIECE \
                + r % PIECE

        bidx = np.zeros((NB * 3, 128, BJ), dtype=np.int32)
        rs = slice(s * BPC, (s + 1) * BPC)
        for b in range(NB):
            u = pos(batch_data[rs, b, 0].astype(np.int32))
            p = pos(batch_data[rs, b, 1].astype(np.int32) + n_users)
            n = pos(batch_data[rs, b, 2].astype(np.int32) + n_users)
            for k, v in enumerate((u, p, n)):
                bidx[b * 3 + k] = v.reshape(BJ, 128).T

        m = {
            "init_shard": init_shard,
            "iotar_in": iotar,
            "w_in": w_bf,
            "bb_in": bb,
            "bidx_in": bidx,
            "dinv_in": dinv_arr[s],
        }
        for b in range(NB):
            m[f"col{b}"] = cols_arr[s][b]
            m[f"idx{b}"] = idx_arr[s][b]
        in_maps.append(m)
    return in_maps


# ---------------------------------------------------------------------------
# Device program
# ---------------------------------------------------------------------------
def build_program(cfg, sched, sim=False):
    from concourse import bass, bacc, mybir, tile

    dt = mybir.dt
    AF = mybir.ActivationFunctionType
    ALU = mybir.AluOpType

    ncores = cfg["ncores"]
    NT = cfg["nt"]
    SH = cfg["shard"]
    NTOT = SH * ncores
    E = cfg["embed"]
    WT = cfg["wt"]            # 128 table cols
    NBK = cfg["nbuck"]
    BUCK = cfg["bucket"]
    WTL = cfg["wtiles"]
    G = cfg["g"]
    FLUSH = cfg["flush"]
    BPC = cfg["batch_per_core"]
    BJ = BPC // 128
    NB = cfg["nbeh"]
    NV = NB + 1
    NW = (NT + WTL - 1) // WTL

    C = sched["C"]
    tiles_md = sched["tiles"]
    windows_md = sched["windows"]
    cstart_md = sched["tile_cstart"]

    # max chunks in any window (for the staging tile size)
    wch_max = 0
    for b in range(NB):
        for w in range(NW):
            wch = sum(n for (_, _, n) in windows_md[b][w])
            wch_max = max(wch_max, wch)

    def bc(ap, where, n):
        newap = list(ap.ap)
        newap.insert(where, [0, n])
        return dataclasses.replace(ap, ap=newap)

    nc = bacc.Bacc("TRN2", target_bir_lowering=False, debug=False,
                   num_devices=1 if sim is True else ncores,
                   num_swdge_queues=4)

    def all_gather(src_tile, dst_tile, nrep):
        # sim mode: stand in for the collective with local HBM->HBM copies
        # of the same receive volume so TimelineSim can run (single-core,
        # no collectives) with comparable DMA load + dependencies.
        if sim is True:
            n = src_tile.shape[0]
            for r in range(nrep):
                nc.sync.dma_start(out=dst_tile[r * n:(r + 1) * n, :],
                                  in_=src_tile[:])
        else:
            nc.gpsimd.collective_compute(
                "AllGather", mybir.AluOpType.bypass,
                replica_groups=[list(range(nrep))],
                ins=[src_tile[:].opt()], outs=[dst_tile[:].opt()])

    f32, bf16, i32, i16 = dt.float32, dt.bfloat16, dt.int32, dt.int16
    shared = "Local"

    init_in = nc.dram_tensor("init_shard", [SH, E], f32,
                             kind="ExternalInput").ap()
    iotar_in = nc.dram_tensor("iotar_in", [1, 128 * G], bf16,
                              kind="ExternalInput").ap()
    dinv_in = nc.dram_tensor("dinv_in", [NB, 128, NT], f32,
                             kind="ExternalInput").ap()
    w_in = nc.dram_tensor("w_in", [NB, E, E], bf16, kind="ExternalInput").ap()
    bb_in = nc.dram_tensor("bb_in", [NB, 128, E], f32,
                           kind="ExternalInput").ap()
    bidx_in = nc.dram_tensor("bidx_in", [NB * 3, 128, BJ], i32,
                             kind="ExternalInput").ap()
    col_in = [nc.dram_tensor(f"col{b}", [128, C[b]], bf16,
                             kind="ExternalInput").ap() for b in range(NB)]
    idx_in = [nc.dram_tensor(f"idx{b}", [128, C[b] * 8], i16,
                             kind="ExternalInput").ap() for b in range(NB)]
    loss_out = nc.dram_tensor("loss", [1, 1], f32, kind="ExternalOutput").ap()

    with tile.TileContext(nc) as tc:
        with (
            tc.tile_pool(name="dram", bufs=1, space="DRAM") as dpool,
            tc.tile_pool(name="pers", bufs=1) as pers,
            tc.tile_pool(name="work", bufs=2) as work,
            tc.tile_pool(name="small", bufs=4) as small,
            tc.tile_pool(name="ppx", bufs=3, space="PSUM") as ppx,
            tc.tile_pool(name="ppy", bufs=2, space="PSUM") as ppy,
        ):
            t2s = [dpool.tile([SH, WT], bf16, tag=f"t2s{v}",
                              name=f"t2s{v}") for v in range(NV)]
            t2f = [dpool.tile([NTOT, WT], bf16, tag=f"t2f{v}",
                              name=f"t2f{v}", addr_space=shared)
                   for v in range(NV)]
            lag_i = dpool.tile([1, 2], f32, tag="lag_i", name="lag_i")
            lag_o = dpool.tile([ncores, 2], f32, tag="lag_o", name="lag_o",
                               addr_space=shared)

            tot = pers.tile([128, NT * E], f32, tag="tot", name="tot")
            irep = pers.tile([128, 128 * G], bf16, tag="irep", name="irep")
            wsb = pers.tile([E, NB * E], bf16, tag="wsb", name="wsb")
            bbsb = pers.tile([128, NB * E], f32, tag="bbsb", name="bbsb")
            bidx = pers.tile([128, NB * 3 * BJ], i32, tag="bidx", name="bidx")
            dinvsb = pers.tile([128, NB * NT], f32, tag="dinvsb",
                               name="dinvsb")
            dinv3z = pers.tile([128, 1], f32, tag="dinv3z", name="dinv3z")
            onesf = pers.tile([128, 1], f32, tag="onesf", name="onesf")
            racc = pers.tile([128, 16], f32, tag="racc", name="racc")
            blacc = pers.tile([128, NB], f32, tag="blacc", name="blacc")

            nc.sync.dma_start(
                out=irep[:].rearrange("p (a x) -> p a x", a=1),
                in_=bc(iotar_in, 0, 128))
            nc.sync.dma_start(
                out=dinvsb[:].rearrange("p (b t) -> p b t", b=NB),
                in_=dinv_in.rearrange("b p t -> p b t"))
            nc.sync.dma_start(
                out=wsb[:].rearrange("k (b e) -> k b e", b=NB),
                in_=w_in.rearrange("b k e -> k b e"))
            nc.sync.dma_start(
                out=bbsb[:].rearrange("p (b e) -> p b e", b=NB),
                in_=bb_in.rearrange("b p e -> p b e"))
            nc.sync.dma_start(
                out=bidx[:].rearrange("p (a j) -> p a j", a=NB * 3),
                in_=bidx_in.rearrange("a p j -> p a j"))
            nc.sync.dma_start(
                out=tot[:].rearrange("p (t e) -> p t e", e=E),
                in_=init_in.rearrange("(t p) e -> p t e", p=128))
            nc.vector.memset(onesf[:], 1.0)
            nc.vector.memset(dinv3z[:], 0.0)
            epsb = pers.tile([128, 1], f32, tag="epsb", name="epsb")
            nc.vector.memset(epsb[:], 1e-24)

            def dinv_ap(v, t):
                # per-partition dinv scalar for (behavior v, dest tile t)
                if v < NB:
                    return dinvsb[:, v * NT + t:v * NT + t + 1]
                return dinv3z[:, 0:1]

            # reg term: sum of squares of the initial embeddings
            NREG = (NT * E + 1023) // 1024
            sqd = pers.tile([128, 1024], f32, tag="sqd", name="sqd")
            for i in range(NREG):
                sl = slice(i * 1024, min((i + 1) * 1024, NT * E))
                nc.scalar.activation(out=sqd[:, : sl.stop - sl.start],
                                     in_=tot[:, sl], func=AF.Square,
                                     accum_out=racc[:, i:i + 1])

            # ------- lazy consumption-ordered one-hot group builder -------
            class IndBuilder:
                """Builds one-hot groups for consumption positions
                [c0, c0+wch) on demand, in order, so only a few groups are
                live at once. Layout [p, d, g] (g innermost) so every
                operand of the is_equal has stride-1 innermost dims and the
                DVE runs in its 2x perf mode."""

                def __init__(self, cs, c0, wch):
                    self.cs, self.c0, self.wch = cs, c0, wch
                    self.groups = {}

                def get(self, cpos):
                    rel = cpos - self.c0
                    g0 = (rel // G) * G
                    if g0 not in self.groups:
                        gw = min(G, self.wch - g0)
                        ind = work.tile([128, 128 * G], dt.bfloat16,
                                        tag="ind", name="ind", bufs=3)
                        iv = ind[:].rearrange("p (d g) -> p d g", g=G)
                        nc.vector.tensor_tensor(
                            out=iv[:, :, :gw],
                            in0=irep[:].rearrange(
                                "p (d g) -> p d g", g=G)[:, :, :gw],
                            in1=bc(self.cs[:, g0:g0 + gw], 1, 128),
                            op=ALU.is_equal)
                        self.groups[g0] = iv
                    return self.groups[g0], rel - g0

            # ------------- T2 staging + per-piece AllGather -------------
            # Table v is built tile-by-tile (fused into main_pass(v-1)'s
            # post_tile stream); every 2 flushes completes one PIECE of the
            # local slab and fires that piece's AllGather, so collectives
            # overlap the remaining compute of the producing behavior.
            PIECE = BUCK // ncores

            def ag_piece(v, k):
                src = t2s[v][k * PIECE:(k + 1) * PIECE, :]
                if sim == "noag":
                    return
                if sim:
                    for r in range(ncores):
                        o = k * BUCK + r * PIECE
                        nc.sync.dma_start(out=t2f[v][o:o + PIECE, :],
                                          in_=src)
                else:
                    nc.gpsimd.collective_compute(
                        "AllGather", mybir.AluOpType.bypass,
                        replica_groups=[list(range(ncores))],
                        ins=[src.opt()],
                        outs=[t2f[v][k * BUCK:(k + 1) * BUCK, :].opt()])

            class Stager:
                def __init__(self, v):
                    self.v = v
                    self.s65 = None

                def stage(self, t):
                    i = t % FLUSH
                    if i == 0:
                        self.s65 = work.tile([128, FLUSH * WT], bf16,
                                             tag="s65", name="s65")
                    totsl = tot[:, t * E:(t + 1) * E]
                    nc.vector.tensor_scalar(
                        out=self.s65[:, i * WT:i * WT + E], in0=totsl,
                        scalar1=dinv_ap(self.v, t), scalar2=None,
                        op0=ALU.mult)
                    nc.scalar.copy(
                        out=self.s65[:, i * WT + E:i * WT + 2 * E],
                        in_=totsl)
                    if i == FLUSH - 1:
                        tf = t - i
                        nc.sync.dma_start(
                            out=t2s[self.v][:].rearrange(
                                "(t p) w -> p t w", p=128)[:, tf:t + 1, :],
                            in_=self.s65[:, :FLUSH * WT].rearrange(
                                "p (t w) -> p t w", w=WT))

            def assemble(v):
                st = Stager(v)
                for t in range(NT):
                    st.stage(t)

            # ---------------- main pass ----------------
            self_q = [0]
            NQ = 4

            def ag_block(v):
                # all 7 piece triggers up-front: their input flushes
                # completed during the previous pass, so these don't stall
                # the in-order Pool queue, and the transfers stream ahead
                # of the bucket-ordered gathers that consume them.
                for k in range(NBK):
                    ag_piece(v, k)

            def main_pass(b):
                ag_block(b)
                stg = Stager(b + 1)
                for w in range(NW):
                    segs = windows_md[b][w]
                    g0 = min(st for (_, st, _) in segs)
                    wch = sum(n for (_, _, n) in segs)
                    t0w = w * WTL
                    c0 = int(cstart_md[b][t0w])
                    cs = small.tile([128, wch_max], bf16, tag="cs", name="cs",
                                    bufs=3)
                    nc.sync.dma_start(out=cs[:, :wch],
                                      in_=col_in[b][:, c0:c0 + wch])
                    ixs = small.tile([128, wch_max * 8], i16, tag="ixs",
                                     name="ixs", bufs=2)
                    nc.sync.dma_start(out=ixs[:, :wch * 8],
                                      in_=idx_in[b][:, g0 * 8:(g0 + wch) * 8])
                    gat = work.tile([128, wch_max * 128], bf16, tag="gat",
                                    name="gat")
                    gv = gat[:].rearrange("p (c e) -> p c e", e=128)
                    for (be, st, n) in segs:
                        for o in range(0, n, 8):
                            if sim == "nogather":
                                break
                            m = min(8, n - o)
                            so = st - g0 + o
                            nc.gpsimd.dma_gather(
                                out_ap=gv[:, so:so + m, :],
                                in_ap=t2f[b][be * BUCK:(be + 1) * BUCK, :],
                                idxs_ap=ixs[:, so * 8:(so + m) * 8],
                                num_idxs=m * 128,
                                num_idxs_reg=m * 128,
                                elem_size=WT,
                                queue_num=self_q[0] % NQ)
                            self_q[0] += 1
                    bld = IndBuilder(cs, c0, wch)
                    for t in range(t0w, min(t0w + WTL, NT)):
                        plist = tiles_md[b][t]
                        xt_ps = ppx.tile([E, 128], f32, tag="xt", name="xt")
                        for j, pos in enumerate(plist):
                            iv, r = bld.get(int(cstart_md[b][t]) + j)
                            nc.tensor.matmul(
                                out=xt_ps[:],
                                lhsT=gv[:, pos - g0, 0:E],
                                rhs=iv[:, :, r],
                                start=(j == 0), stop=(j == len(plist) - 1))
                        post_tile(b, t, xt_ps)
                        stg.stage(t)

            def post_tile(b, t, xt_ps):
                xts = small.tile([E, 128], bf16, tag="xts", name="xts")
                nc.scalar.copy(out=xts[:], in_=xt_ps[:])
                y_ps = ppy.tile([128, E], f32, tag="y", name="y")
                nc.tensor.matmul(out=y_ps[:], lhsT=xts[:],
                                 rhs=wsb[:, b * E:(b + 1) * E],
                                 start=True, stop=True)
                z = small.tile([128, E], f32, tag="z", name="z")
                ss = small.tile([128, 1], f32, tag="ss", name="ss")
                # z = y*dinv_col + bias
                nc.vector.scalar_tensor_tensor(
                    out=z[:], in0=y_ps[:], scalar=dinv_ap(b, t),
                    in1=bbsb[:, b * E:(b + 1) * E],
                    op0=ALU.mult, op1=ALU.add)
                sq = small.tile([128, E], f32, tag="sq", name="sq")
                nc.scalar.activation(out=sq[:], in_=z[:], func=AF.Square,
                                     accum_out=ss[:])
                # sqrt(ss + 1e-24) ~= max(sqrt(ss), 1e-12)
                nc.scalar.activation(out=ss[:], in_=ss[:], func=AF.Sqrt,
                                     bias=epsb[:, 0:1])
                rin = small.tile([128, 1], f32, tag="rin", name="rin")
                nc.vector.reciprocal(out=rin[:], in_=ss[:])
                totsl = tot[:, t * E:(t + 1) * E]
                # tot += z * rin
                nc.vector.scalar_tensor_tensor(
                    out=totsl, in0=z[:], scalar=rin[:, 0:1], in1=totsl,
                    op0=ALU.mult, op1=ALU.add)

            # ---------------- loss ----------------
            LOG1P_C = [2.4139025189026897e-09, 0.9999996692324197,
                       -0.499988759640371, 0.3331669190104936,
                       -0.2486582066434577, 0.19337637102999028,
                       -0.14517645896753417, 0.09470379566439587,
                       -0.04713346504062944, 0.015145372148722138,
                       -0.002288060381570317]

            def loss_pass(b):
                gs = []
                for k in range(3):
                    gk = small.tile([128, BJ * WT], bf16, tag=f"bg{k}",
                                    name=f"bg{k}")
                    gkv = gk[:].rearrange("p (j w) -> p j w", w=WT)
                    for j in range(BJ):
                        o = (b * 3 + k) * BJ + j
                        nc.gpsimd.indirect_dma_start(
                            out=gkv[:, j, :],
                            out_offset=None,
                            in_=t2f[b + 1][:],
                            in_offset=bass.IndirectOffsetOnAxis(
                                ap=bidx[:, o:o + 1], axis=0))
                    gs.append(gkv)
                prod = small.tile([128, BJ * E], f32, tag="prod", name="prod")
                pv = prod[:].rearrange("p (j e) -> p j e", e=E)
                sco = small.tile([128, 2 * BJ], f32, tag="sco", name="sco")
                for k in range(2):
                    nc.vector.tensor_tensor(out=pv, in0=gs[0][:, :, E:2 * E],
                                            in1=gs[k + 1][:, :, E:2 * E],
                                            op=ALU.mult)
                    nc.vector.tensor_reduce(
                        out=sco[:, k * BJ:(k + 1) * BJ], in_=pv,
                        axis=mybir.AxisListType.X, op=ALU.add)
                dd = small.tile([128, BJ], f32, tag="dd", name="dd")
                nc.vector.tensor_tensor(out=dd[:], in0=sco[:, 0:BJ],
                                        in1=sco[:, BJ:2 * BJ],
                                        op=ALU.subtract)
                aab = small.tile([128, BJ], f32, tag="aab", name="aab")
                nc.vector.tensor_scalar(out=aab[:], in0=dd[:], scalar1=-1.0,
                                        scalar2=None, op0=ALU.mult)
                nc.vector.tensor_tensor(out=aab[:], in0=aab[:], in1=dd[:],
                                        op=ALU.max)
                zex = small.tile([128, BJ], f32, tag="zex", name="zex")
                nc.scalar.activation(out=zex[:], in_=aab[:], func=AF.Exp,
                                     scale=-1.0)
                pol = small.tile([128, BJ], f32, tag="pol", name="pol")
                nc.vector.tensor_scalar(out=pol[:], in0=zex[:],
                                        scalar1=LOG1P_C[10],
                                        scalar2=LOG1P_C[9],
                                        op0=ALU.mult, op1=ALU.add)
                for k in range(8, -1, -1):
                    nc.vector.tensor_tensor(out=pol[:], in0=pol[:],
                                            in1=zex[:], op=ALU.mult)
                    nc.vector.tensor_scalar(out=pol[:], in0=pol[:],
                                            scalar1=LOG1P_C[k], scalar2=None,
                                            op0=ALU.add)
                nc.vector.tensor_scalar(out=dd[:], in0=dd[:], scalar1=-1.0,
                                        scalar2=0.0, op0=ALU.mult,
                                        op1=ALU.max)
                nc.vector.tensor_tensor(out=pol[:], in0=pol[:], in1=dd[:],
                                        op=ALU.add)
                nc.vector.tensor_reduce(out=blacc[:, b:b + 1], in_=pol[:],
                                        axis=mybir.AxisListType.X,
                                        op=ALU.add)

            # ================= program =================
            assemble(0)       # stages table 0 (no AGs)
            main_pass(0)      # AG(0) block, then gathers; stages table 1
            main_pass(1)      # AG(1) block first; stages table 2
            loss_pass(0)      # table 1 complete by now
            main_pass(2)      # AG(2) block first; stages table 3
            loss_pass(1)
            ag_block(3)
            loss_pass(2)

            # ---------------- final combine ----------------
            pack = small.tile([128, 2], f32, tag="pack", name="pack")
            nc.vector.tensor_reduce(out=pack[:, 0:1], in_=blacc[:],
                                    axis=mybir.AxisListType.X, op=ALU.add)
            nc.vector.tensor_reduce(out=pack[:, 1:2], in_=racc[:, :NREG],
                                    axis=mybir.AxisListType.X, op=ALU.add)
            fin_ps = ppy.tile([1, 2], f32, tag="fin", name="fin", bufs=1)
            nc.tensor.matmul(out=fin_ps[:], lhsT=onesf[:], rhs=pack[:],
                             start=True, stop=True)
            fin = small.tile([1, 2], f32, tag="fins", name="fins")
            nc.vector.tensor_copy(out=fin[:], in_=fin_ps[:])
            nc.sync.dma_start(out=lag_i[:], in_=fin[:])
            all_gather(lag_i, lag_o, ncores)
            lsb = small.tile([1, 2 * ncores], f32, tag="lsb", name="lsb")
            nc.sync.dma_start(
                out=lsb[:],
                in_=lag_o[:].rearrange("(o a) b -> o (a b)", o=1))
            bl = small.tile([1, 2], f32, tag="bl", name="bl")
            lv = lsb[:].rearrange("p (a b) -> p a b", b=2)
            nc.vector.tensor_reduce(out=bl[:, 0:1], in_=lv[:, :, 0:1],
                                    axis=mybir.AxisListType.XY, op=ALU.add)
            nc.vector.tensor_reduce(out=bl[:, 1:2], in_=lv[:, :, 1:2],
                                    axis=mybir.AxisListType.XY, op=ALU.add)
            res = small.tile([1, 1], f32, tag="res", name="res")
            nc.vector.tensor_scalar(out=res[:], in0=bl[:, 1:2],
                                    scalar1=cfg["reg_weight"] * 0.5,
                                    scalar2=None, op0=ALU.mult)
            nc.vector.tensor_tensor(out=res[:], in0=res[:], in1=bl[:, 0:1],
                                    op=ALU.add)
            nc.vector.tensor_scalar(out=res[:], in0=res[:],
                                    scalar1=1.0 / cfg["batch"],
                                    scalar2=None, op0=ALU.mult)
            nc.sync.dma_start(out=loss_out, in_=res[:])

    nc.compile()
    return nc


# ---------------------------------------------------------------------------
# Entry point
# ---------------------------------------------------------------------------
LAST_RESULTS = None


def kernel(**inputs) -> np.ndarray:
    global LAST_RESULTS
    cfg = FULL_CFG
    edges = np.asarray(inputs["edges"])
    arrs = make_schedule_and_arrays(edges, cfg)
    sched = arrs[0]
    in_maps = make_inputs_per_core(inputs, cfg, arrs)
    nc = build_program(cfg, sched)

    import os
    os.environ["BASS_NEVER_TRACE"] = "1"  # axon NTFF hook absent here
    from concourse import bass_utils
    res = bass_utils.run_bass_kernel_spmd(
        nc, in_maps, core_ids=list(range(cfg["ncores"])))
    LAST_RESULTS = res
    out = res.results[0]["loss"]
    return np.float32(out.reshape(-1)[0])



# revision 50
# speedup vs baseline: 1.1281x; 1.0009x over previous

# CRGCN multi-behavior GCN forward loss on 8 Trainium2 NeuronCores.
#
# Strategy (graph/data parallel, dest-node sharding):
#  - Nodes (users+items, 200000 -> padded 200704) are sharded row-wise across
#    8 cores (25088 = 196*128 nodes/core). Edges are partitioned by the shard
#    of their destination (col) node on the host, bucketed by (128-dest tile,
#    source bucket of 28672 rows) and padded so every 128-edge chunk maps to
#    one dest tile and one source bucket. The chunk schedule is the max over
#    cores so a single SPMD program fits all 8 cores.
#  - Per behavior each core holds a bf16 table T2 = [dinv*total | total]
#    ([200704, 128], 256B rows) for ALL nodes, produced by AllGather of
#    per-shard slabs. Message pass: dma_gather (int16 in-bucket indices) of
#    T2 rows for edge sources; a 0/1 one-hot (edge x dest-in-tile) built on
#    DVE from edge cols; PE matmul contracts edges, accumulating
#    S^T[feat, dest] = sum_e dinv[r_e]*total[r_e] x onehot in PSUM per dest
#    tile; then S @ W, *dinv[d], +b, l2-normalize, residual-accumulate into
#    the SBUF-resident fp32 total shard.
#  - deg (in-degree) is a one-hot x ones matmul (bf16, exact), per behavior,
#    from the same col data.
#  - BPR loss: batch rows sharded across cores; u/pos/neg rows fetched with
#    per-partition indirect DMA from the raw-total half of T2; dots +
#    softplus(-d) (relu + log1p poly) on-device; partials AllGathered so all
#    cores emit the identical final scalar.

import sys

sys.path.insert(0, "/opt/trn_rl_repo")

import dataclasses
import numpy as np

# ---------------- problem constants (hardcoded; kernel.py is standalone) ---
N_USERS = 100000
N_ITEMS = 100000
N_NODES = 200000
EMBED = 64
N_BEH = 3
BATCH = 4096
REG_WEIGHT = 1e-4
NCORES = 8

FULL_CFG = dict(
    ncores=NCORES,
    embed=EMBED,
    nbeh=N_BEH,
    shard=25088,          # 196 * 128
    nt=196,               # dest tiles per shard
    wt=128,               # T2 row width in bf16 elems (256B)
    nbuck=7,              # source buckets
    bucket=28672,         # rows per bucket (7 * 28672 = 200704)
    wtiles=8,             # dest tiles per gather window
    g=32,                 # chunks per one-hot build group
    flush=14,             # tiles per T2 staging flush (196 = 14*14)
    batch=BATCH,
    batch_per_core=BATCH // NCORES,   # 512
    n_nodes=N_NODES,
    reg_weight=REG_WEIGHT,
)


# ---------------------------------------------------------------------------
# Host-side preprocessing
# ---------------------------------------------------------------------------
def make_schedule_and_arrays(edges, cfg):
    """edges: [NB, 2, E]. Builds the (window, bucket, tile)-ordered common
    chunk schedule and the per-core col/idx arrays."""
    ncores = cfg["ncores"]
    NT = cfg["nt"]
    NB = cfg["nbeh"]
    NBK = cfg["nbuck"]
    BUCK = cfg["bucket"]
    WT = cfg["wtiles"]
    NW = (NT + WT - 1) // WT

    sched = {"C": [], "cells": [], "tiles": [], "windows": [],
             "tile_cstart": []}
    cols_arr = [[None] * NB for _ in range(ncores)]
    idx_arr = [[None] * NB for _ in range(ncores)]
    dinv_arr = np.zeros((ncores, NB, 128, NT), dtype=np.float32)

    n_nodes = cfg["n_nodes"]
    shard = cfg["shard"]
    for b in range(NB):
        row = np.asarray(edges[b, 0], dtype=np.int64)
        col = np.asarray(edges[b, 1], dtype=np.int64)
        # host-side in-degree -> dinv per core shard, [128, NT] layout
        deg = np.bincount(col, minlength=n_nodes).astype(np.float32)
        dinv_g = np.where(deg > 0,
                          1.0 / np.sqrt(np.maximum(deg, 1.0)),
                          0.0).astype(np.float32)
        dinv_pad = np.zeros(ncores * shard, dtype=np.float32)
        dinv_pad[:n_nodes] = dinv_g
        for s in range(ncores):
            dinv_arr[s, b] = dinv_pad[s * shard:(s + 1) * shard].reshape(
                NT, 128).T
        gt = col >> 7                       # global dest tile
        s_of = gt // NT                     # owning core
        t_of = gt - s_of * NT               # local dest tile
        # permuted table layout: bucket k holds piece k (PIECE local rows)
        # of every core's shard, so AllGather k is per-rank contiguous:
        # pos(n) = beta*BUCK + (n//SH)*PIECE + (n%SH)%PIECE,
        # beta = (n%SH)//PIECE
        PIECE = BUCK // ncores
        r_loc = row % shard
        beta = r_loc // PIECE               # source bucket (= piece id)
        # per (core, tile, bucket) counts
        cellkey = (s_of * NT + t_of) * NBK + beta
        cnt = np.bincount(cellkey, minlength=ncores * NT * NBK).reshape(
            ncores, NT, NBK)
        K_cell = -(-cnt.max(axis=0) // 128)           # [NT, NBK]
        empty_t = K_cell.sum(axis=1) == 0
        K_cell[empty_t, 0] = 1

        # gather order: (window, bucket, tile); consumption order:
        # (window, tile, bucket). Chunks get positions in both orders.
        C = int(K_cell.sum())
        cell_start = {}      # gather-order chunk start per cell
        cell_cstart = {}     # consumption-order chunk start per cell
        pos = 0
        for w in range(NW):
            ts = range(w * WT, min((w + 1) * WT, NT))
            for be in range(NBK):
                for t in ts:
                    if K_cell[t, be]:
                        cell_start[(t, be)] = pos
                        pos += int(K_cell[t, be])
        assert pos == C
        cpos = 0
        tile_cstart = np.zeros(NT + 1, dtype=np.int64)
        for w in range(NW):
            ts = range(w * WT, min((w + 1) * WT, NT))
            for t in ts:
                tile_cstart[t] = cpos
                for be in range(NBK):
                    if K_cell[t, be]:
                        cell_cstart[(t, be)] = cpos
                        cpos += int(K_cell[t, be])
        tile_cstart[NT] = cpos
        assert cpos == C

        # per-tile consumption: ordered chunk positions + total K per tile
        tiles = []
        for t in range(NT):
            plist = []
            for be in range(NBK):
                if K_cell[t, be]:
                    st = cell_start[(t, be)]
                    plist.extend(range(st, st + int(K_cell[t, be])))
            tiles.append(plist)

        # per-window gather segments: (bucket, pos_start, n_chunks)
        windows = []
        for w in range(NW):
            ts = range(w * WT, min((w + 1) * WT, NT))
            segs = []
            for be in range(NBK):
                n = int(sum(K_cell[t, be] for t in ts))
                if n:
                    st = min(cell_start[(t, be)] for t in ts
                             if K_cell[t, be])
                    segs.append((be, st, n))
            windows.append(segs)

        sched["C"].append(C)
        sched["cells"].append((K_cell, cell_start))
        sched["tiles"].append(tiles)
        sched["windows"].append(windows)
        sched["tile_cstart"].append(tile_cstart)

        # ------------- per-core arrays -------------
        starts_np = np.zeros((NT, NBK), dtype=np.int64)
        for (t, be), st in cell_start.items():
            starts_np[t, be] = st
        cstarts_np = np.zeros((NT, NBK), dtype=np.int64)
        for (t, be), st in cell_cstart.items():
            cstarts_np[t, be] = st
        for s in range(ncores):
            colv = np.full(C * 128, 128.0, dtype=np.float32)
            rowv = np.zeros(C * 128, dtype=np.int64)   # in-bucket idx
            sel = s_of == s
            r_s = row[sel]
            c_s = col[sel]
            t_s = t_of[sel]
            be_s = beta[sel]
            key = t_s * NBK + be_s
            order = np.argsort(key, kind="stable")
            r_s, c_s, t_s, be_s, key = (r_s[order], c_s[order], t_s[order],
                                        be_s[order], key[order])
            seg_start = np.searchsorted(key, np.arange(NT * NBK))
            within = np.arange(len(key)) - seg_start[key]
            dst = starts_np[t_s, be_s] * 128 + within
            cdst = cstarts_np[t_s, be_s] * 128 + within
            colv[cdst] = (c_s & 127).astype(np.float32)
            rowv[dst] = (r_s // shard) * (BUCK // ncores) \
                + (r_s % shard) % (BUCK // ncores)
            cols_arr[s][b] = np.ascontiguousarray(
                colv.reshape(C, 128).T).astype(np.uint8)   # [128, C]
            # idx16: [128, C*8]; gather element i -> [i%16 (+16k), off+i//16]
            iv = rowv.reshape(C * 128)
            i16 = np.zeros((16, C * 8), dtype=np.int16)
            ii = np.arange(C * 128)
            i16[ii % 16, ii // 16] = iv.astype(np.int16)
            idx_arr[s][b] = np.ascontiguousarray(i16)   # [16, C*8]

    return sched, cols_arr, idx_arr, dinv_arr


def make_inputs_per_core(inputs, cfg, sched_arrays):
    import ml_dtypes

    ncores = cfg["ncores"]
    SH = cfg["shard"]
    E = cfg["embed"]
    NB = cfg["nbeh"]
    BPC = cfg["batch_per_core"]
    BJ = BPC // 128
    n_nodes = cfg["n_nodes"]
    n_users = n_nodes // 2

    sched, cols_arr, idx_arr, dinv_arr = sched_arrays

    user_emb = np.asarray(inputs["user_emb"], dtype=np.float32)
    item_emb = np.asarray(inputs["item_emb"], dtype=np.float32)
    gcn_weight = np.asarray(inputs["gcn_weight"], dtype=np.float32)
    gcn_bias = np.asarray(inputs["gcn_bias"], dtype=np.float32)
    batch_data = np.asarray(inputs["batch_data"], dtype=np.int64)

    total0 = np.concatenate([user_emb, item_emb], axis=0)

    G = cfg["g"]
    iotar = np.repeat(np.arange(128, dtype=np.float32), G)[None, :].astype(
        ml_dtypes.bfloat16)                       # [1, 128*G], j//G values
    w_bf = gcn_weight.astype(ml_dtypes.bfloat16)
    bb = np.tile(gcn_bias[:, None, :], (1, 128, 1)).astype(np.float32)

    in_maps = []
    for s in range(ncores):
        lo = s * SH
        hi = min((s + 1) * SH, n_nodes)
        init_shard = np.zeros((SH, E), dtype=ml_dtypes.bfloat16)
        if hi > lo:
            init_shard[: hi - lo] = total0[lo:hi].astype(ml_dtypes.bfloat16)

        PIECE = cfg["bucket"] // ncores

        def pos(n):
            # node id -> permuted table row (see make_schedule_and_arrays)
            r = n % SH
            return (r // PIECE) * cfg["bucket"] + (n // SH) * PIECE \
                + r % PIECE

        bidx = np.zeros((NB * 3, 128, BJ), dtype=np.int32)
        rs = slice(s * BPC, (s + 1) * BPC)
        for b in range(NB):
            u = pos(batch_data[rs, b, 0].astype(np.int32))
            p = pos(batch_data[rs, b, 1].astype(np.int32) + n_users)
            n = pos(batch_data[rs, b, 2].astype(np.int32) + n_users)
            for k, v in enumerate((u, p, n)):
                bidx[b * 3 + k] = v.reshape(BJ, 128).T

        m = {
            "init_shard": init_shard,
            "iotar_in": iotar,
            "w_in": w_bf,
            "bb_in": bb,
            "bidx_in": bidx,
            "dinv_in": dinv_arr[s],
        }
        for b in range(NB):
            m[f"col{b}"] = cols_arr[s][b]
            m[f"idx{b}"] = idx_arr[s][b]
        in_maps.append(m)
    return in_maps


# ---------------------------------------------------------------------------
# Device program
# ---------------------------------------------------------------------------
def build_program(cfg, sched, sim=False):
    from concourse import bass, bacc, mybir, tile

    dt = mybir.dt
    AF = mybir.ActivationFunctionType
    ALU = mybir.AluOpType

    ncores = cfg["ncores"]
    NT = cfg["nt"]
    SH = cfg["shard"]
    NTOT = SH * ncores
    E = cfg["embed"]
    WT = cfg["wt"]            # 128 table cols
    NBK = cfg["nbuck"]
    BUCK = cfg["bucket"]
    WTL = cfg["wtiles"]
    G = cfg["g"]
    FLUSH = cfg["flush"]
    BPC = cfg["batch_per_core"]
    BJ = BPC // 128
    NB = cfg["nbeh"]
    NV = NB + 1
    NW = (NT + WTL - 1) // WTL

    C = sched["C"]
    tiles_md = sched["tiles"]
    windows_md = sched["windows"]
    cstart_md = sched["tile_cstart"]

    # max chunks in any window (for the staging tile size)
    wch_max = 0
    for b in range(NB):
        for w in range(NW):
            wch = sum(n for (_, _, n) in windows_md[b][w])
            wch_max = max(wch_max, wch)

    def bc(ap, where, n):
        newap = list(ap.ap)
        newap.insert(where, [0, n])
        return dataclasses.replace(ap, ap=newap)

    nc = bacc.Bacc("TRN2", target_bir_lowering=False, debug=False,
                   num_devices=1 if sim is True else ncores,
                   num_swdge_queues=4)

    def all_gather(src_tile, dst_tile, nrep):
        # sim mode: stand in for the collective with local HBM->HBM copies
        # of the same receive volume so TimelineSim can run (single-core,
        # no collectives) with comparable DMA load + dependencies.
        if sim is True:
            n = src_tile.shape[0]
            for r in range(nrep):
                nc.sync.dma_start(out=dst_tile[r * n:(r + 1) * n, :],
                                  in_=src_tile[:])
        else:
            nc.gpsimd.collective_compute(
                "AllGather", mybir.AluOpType.bypass,
                replica_groups=[list(range(nrep))],
                ins=[src_tile[:].opt()], outs=[dst_tile[:].opt()])

    f32, bf16, i32, i16 = dt.float32, dt.bfloat16, dt.int32, dt.int16
    shared = "Local"

    init_in = nc.dram_tensor("init_shard", [SH, E], bf16,
                             kind="ExternalInput").ap()
    iotar_in = nc.dram_tensor("iotar_in", [1, 128 * G], bf16,
                              kind="ExternalInput").ap()
    dinv_in = nc.dram_tensor("dinv_in", [NB, 128, NT], f32,
                             kind="ExternalInput").ap()
    w_in = nc.dram_tensor("w_in", [NB, E, E], bf16, kind="ExternalInput").ap()
    bb_in = nc.dram_tensor("bb_in", [NB, 128, E], f32,
                           kind="ExternalInput").ap()
    bidx_in = nc.dram_tensor("bidx_in", [NB * 3, 128, BJ], i32,
                             kind="ExternalInput").ap()
    col_in = [nc.dram_tensor(f"col{b}", [128, C[b]], dt.uint8,
                             kind="ExternalInput").ap() for b in range(NB)]
    idx16_in = [nc.dram_tensor(f"idx{b}", [16, C[b] * 8], i16,
                              kind="ExternalInput").ap() for b in range(NB)]
    loss_out = nc.dram_tensor("loss", [1, 1], f32, kind="ExternalOutput").ap()

    with tile.TileContext(nc) as tc:
        with (
            tc.tile_pool(name="dram", bufs=1, space="DRAM") as dpool,
            tc.tile_pool(name="pers", bufs=1) as pers,
            tc.tile_pool(name="work", bufs=2) as work,
            tc.tile_pool(name="small", bufs=4) as small,
            tc.tile_pool(name="ppx", bufs=3, space="PSUM") as ppx,
            tc.tile_pool(name="ppy", bufs=2, space="PSUM") as ppy,
        ):
            t2s = [dpool.tile([SH, WT], bf16, tag=f"t2s{v}",
                              name=f"t2s{v}") for v in range(NV)]
            idxr = [dpool.tile([128, C[b] * 8], i16, tag=f"idxr{b}",
                               name=f"idxr{b}") for b in range(NB)]
            t2f = [dpool.tile([NTOT, WT], bf16, tag=f"t2f{v}",
                              name=f"t2f{v}", addr_space=shared)
                   for v in range(NV)]
            lag_i = dpool.tile([1, 2], f32, tag="lag_i", name="lag_i")
            lag_o = dpool.tile([ncores, 2], f32, tag="lag_o", name="lag_o",
                               addr_space=shared)

            tot = pers.tile([128, NT * E], f32, tag="tot", name="tot")
            irep = pers.tile([128, 128 * G], bf16, tag="irep", name="irep")
            wsb = pers.tile([E, NB * E], bf16, tag="wsb", name="wsb")
            bbsb = pers.tile([128, NB * E], f32, tag="bbsb", name="bbsb")
            bidx = pers.tile([128, NB * 3 * BJ], i32, tag="bidx", name="bidx")
            dinvsb = pers.tile([128, NB * NT], f32, tag="dinvsb",
                               name="dinvsb")
            dinv3z = pers.tile([128, 1], f32, tag="dinv3z", name="dinv3z")
            onesf = pers.tile([128, 1], f32, tag="onesf", name="onesf")
            racc = pers.tile([128, 16], f32, tag="racc", name="racc")
            blacc = pers.tile([128, NB], f32, tag="blacc", name="blacc")

            nc.sync.dma_start(
                out=irep[:].rearrange("p (a x) -> p a x", a=1),
                in_=bc(iotar_in, 0, 128))
            nc.sync.dma_start(
                out=dinvsb[:].rearrange("p (b t) -> p b t", b=NB),
                in_=dinv_in.rearrange("b p t -> p b t"))
            nc.sync.dma_start(
                out=wsb[:].rearrange("k (b e) -> k b e", b=NB),
                in_=w_in.rearrange("b k e -> k b e"))
            nc.sync.dma_start(
                out=bbsb[:].rearrange("p (b e) -> p b e", b=NB),
                in_=bb_in.rearrange("b p e -> p b e"))
            nc.sync.dma_start(
                out=bidx[:].rearrange("p (a j) -> p a j", a=NB * 3),
                in_=bidx_in.rearrange("a p j -> p a j"))
            for b in range(NB):
                nc.sync.dma_start(
                    out=idxr[b][:].rearrange("(a q) x -> a q x", a=8),
                    in_=bc(idx16_in[b], 0, 8))
            totb = work.tile([128, NT * E], bf16, tag="totb", name="totb",
                             bufs=1)
            nc.sync.dma_start(
                out=totb[:].rearrange("p (t e) -> p t e", e=E),
                in_=init_in.rearrange("(t p) e -> p t e", p=128))
            nc.vector.tensor_copy(out=tot[:], in_=totb[:])
            nc.vector.memset(onesf[:], 1.0)
            nc.vector.memset(dinv3z[:], 0.0)
            epsb = pers.tile([128, 1], f32, tag="epsb", name="epsb")
            nc.vector.memset(epsb[:], 1e-24)

            def dinv_ap(v, t):
                # per-partition dinv scalar for (behavior v, dest tile t)
                if v < NB:
                    return dinvsb[:, v * NT + t:v * NT + t + 1]
                return dinv3z[:, 0:1]

            # reg term: sum of squares of the initial embeddings
            NREG = (NT * E + 1023) // 1024
            sqd = pers.tile([128, 1024], f32, tag="sqd", name="sqd")
            for i in range(NREG):
                sl = slice(i * 1024, min((i + 1) * 1024, NT * E))
                nc.scalar.activation(out=sqd[:, : sl.stop - sl.start],
                                     in_=tot[:, sl], func=AF.Square,
                                     accum_out=racc[:, i:i + 1])

            # ------- lazy consumption-ordered one-hot group builder -------
            class IndBuilder:
                """Builds one-hot groups for consumption positions
                [c0, c0+wch) on demand, in order, so only a few groups are
                live at once. Layout [p, d, g] (g innermost) so every
                operand of the is_equal has stride-1 innermost dims and the
                DVE runs in its 2x perf mode."""

                def __init__(self, cs, c0, wch):
                    self.cs, self.c0, self.wch = cs, c0, wch
                    self.groups = {}

                def get(self, cpos):
                    rel = cpos - self.c0
                    g0 = (rel // G) * G
                    if g0 not in self.groups:
                        gw = min(G, self.wch - g0)
                        ind = work.tile([128, 128 * G], dt.bfloat16,
                                        tag="ind", name="ind", bufs=3)
                        iv = ind[:].rearrange("p (d g) -> p d g", g=G)
                        nc.vector.tensor_tensor(
                            out=iv[:, :, :gw],
                            in0=irep[:].rearrange(
                                "p (d g) -> p d g", g=G)[:, :, :gw],
                            in1=bc(self.cs[:, g0:g0 + gw], 1, 128),
                            op=ALU.is_equal)
                        self.groups[g0] = iv
                    return self.groups[g0], rel - g0

            # ------------- T2 staging + per-piece AllGather -------------
            # Table v is built tile-by-tile (fused into main_pass(v-1)'s
            # post_tile stream); every 2 flushes completes one PIECE of the
            # local slab and fires that piece's AllGather, so collectives
            # overlap the remaining compute of the producing behavior.
            PIECE = BUCK // ncores

            def ag_piece(v, k):
                src = t2s[v][k * PIECE:(k + 1) * PIECE, :]
                if sim == "noag":
                    return
                if sim:
                    for r in range(ncores):
                        o = k * BUCK + r * PIECE
                        nc.sync.dma_start(out=t2f[v][o:o + PIECE, :],
                                          in_=src)
                else:
                    nc.gpsimd.collective_compute(
                        "AllGather", mybir.AluOpType.bypass,
                        replica_groups=[list(range(ncores))],
                        ins=[src.opt()],
                        outs=[t2f[v][k * BUCK:(k + 1) * BUCK, :].opt()])

            class Stager:
                def __init__(self, v):
                    self.v = v
                    self.s65 = None

                def stage(self, t):
                    i = t % FLUSH
                    if i == 0:
                        self.s65 = work.tile([128, FLUSH * WT], bf16,
                                             tag="s65", name="s65")
                    totsl = tot[:, t * E:(t + 1) * E]
                    nc.vector.tensor_scalar(
                        out=self.s65[:, i * WT:i * WT + E], in0=totsl,
                        scalar1=dinv_ap(self.v, t), scalar2=None,
                        op0=ALU.mult)
                    nc.scalar.copy(
                        out=self.s65[:, i * WT + E:i * WT + 2 * E],
                        in_=totsl)
                    if i == FLUSH - 1:
                        tf = t - i
                        nc.sync.dma_start(
                            out=t2s[self.v][:].rearrange(
                                "(t p) w -> p t w", p=128)[:, tf:t + 1, :],
                            in_=self.s65[:, :FLUSH * WT].rearrange(
                                "p (t w) -> p t w", w=WT))

            def assemble(v):
                st = Stager(v)
                for t in range(NT):
                    st.stage(t)

            # ---------------- main pass ----------------
            self_q = [0]
            NQ = 4

            def ag_block(v):
                # all 7 piece triggers up-front: their input flushes
                # completed during the previous pass, so these don't stall
                # the in-order Pool queue, and the transfers stream ahead
                # of the bucket-ordered gathers that consume them.
                for k in range(NBK):
                    ag_piece(v, k)

            def main_pass(b):
                ag_block(b)
                stg = Stager(b + 1)
                for w in range(NW):
                    segs = windows_md[b][w]
                    g0 = min(st for (_, st, _) in segs)
                    wch = sum(n for (_, _, n) in segs)
                    t0w = w * WTL
                    c0 = int(cstart_md[b][t0w])
                    cs8 = small.tile([128, wch_max], dt.uint8, tag="cs8",
                                     name="cs8", bufs=2)
                    nc.sync.dma_start(out=cs8[:, :wch],
                                      in_=col_in[b][:, c0:c0 + wch])
                    cs = small.tile([128, wch_max], bf16, tag="cs", name="cs",
                                    bufs=3)
                    nc.vector.tensor_copy(out=cs[:, :wch], in_=cs8[:, :wch])
                    ixs = small.tile([128, wch_max * 8], i16, tag="ixs",
                                     name="ixs", bufs=2)
                    nc.sync.dma_start(
                        out=ixs[:, :wch * 8],
                        in_=idxr[b][:, g0 * 8:(g0 + wch) * 8])
                    gat = work.tile([128, wch_max * 128], bf16, tag="gat",
                                    name="gat")
                    gv = gat[:].rearrange("p (c e) -> p c e", e=128)
                    for (be, st, n) in segs:
                        for o in range(0, n, 8):
                            if sim == "nogather":
                                break
                            m = min(8, n - o)
                            so = st - g0 + o
                            nc.gpsimd.dma_gather(
                                out_ap=gv[:, so:so + m, :],
                                in_ap=t2f[b][be * BUCK:(be + 1) * BUCK, :],
                                idxs_ap=ixs[:, so * 8:(so + m) * 8],
                                num_idxs=m * 128,
                                num_idxs_reg=m * 128,
                                elem_size=WT,
                                queue_num=self_q[0] % NQ)
                            self_q[0] += 1
                    bld = IndBuilder(cs, c0, wch)
                    for t in range(t0w, min(t0w + WTL, NT)):
                        plist = tiles_md[b][t]
                        xt_ps = ppx.tile([E, 128], f32, tag="xt", name="xt")
                        for j, pos in enumerate(plist):
                            iv, r = bld.get(int(cstart_md[b][t]) + j)
                            nc.tensor.matmul(
                                out=xt_ps[:],
                                lhsT=gv[:, pos - g0, 0:E],
                                rhs=iv[:, :, r],
                                start=(j == 0), stop=(j == len(plist) - 1))
                        post_tile(b, t, xt_ps)
                        stg.stage(t)

            def post_tile(b, t, xt_ps):
                xts = small.tile([E, 128], bf16, tag="xts", name="xts")
                nc.scalar.copy(out=xts[:], in_=xt_ps[:])
                y_ps = ppy.tile([128, E], f32, tag="y", name="y")
                nc.tensor.matmul(out=y_ps[:], lhsT=xts[:],
                                 rhs=wsb[:, b * E:(b + 1) * E],
                                 start=True, stop=True)
                z = small.tile([128, E], f32, tag="z", name="z")
                ss = small.tile([128, 1], f32, tag="ss", name="ss")
                # z = y*dinv_col + bias
                nc.vector.scalar_tensor_tensor(
                    out=z[:], in0=y_ps[:], scalar=dinv_ap(b, t),
                    in1=bbsb[:, b * E:(b + 1) * E],
                    op0=ALU.mult, op1=ALU.add)
                sq = small.tile([128, E], f32, tag="sq", name="sq")
                nc.scalar.activation(out=sq[:], in_=z[:], func=AF.Square,
                                     accum_out=ss[:])
                # sqrt(ss + 1e-24) ~= max(sqrt(ss), 1e-12)
                nc.scalar.activation(out=ss[:], in_=ss[:], func=AF.Sqrt,
                                     bias=epsb[:, 0:1])
                rin = small.tile([128, 1], f32, tag="rin", name="rin")
                nc.vector.reciprocal(out=rin[:], in_=ss[:])
                totsl = tot[:, t * E:(t + 1) * E]
                # tot += z * rin
                nc.vector.scalar_tensor_tensor(
                    out=totsl, in0=z[:], scalar=rin[:, 0:1], in1=totsl,
                    op0=ALU.mult, op1=ALU.add)

            # ---------------- loss ----------------
            LOG1P_C = [2.4139025189026897e-09, 0.9999996692324197,
                       -0.499988759640371, 0.3331669190104936,
                       -0.2486582066434577, 0.19337637102999028,
                       -0.14517645896753417, 0.09470379566439587,
                       -0.04713346504062944, 0.015145372148722138,
                       -0.002288060381570317]

            def loss_pass(b):
                gs = []
                for k in range(3):
                    gk = small.tile([128, BJ * WT], bf16, tag=f"bg{k}",
                                    name=f"bg{k}")
                    gkv = gk[:].rearrange("p (j w) -> p j w", w=WT)
                    for j in range(BJ):
                        o = (b * 3 + k) * BJ + j
                        nc.gpsimd.indirect_dma_start(
                            out=gkv[:, j, :],
                            out_offset=None,
                            in_=t2f[b + 1][:],
                            in_offset=bass.IndirectOffsetOnAxis(
                                ap=bidx[:, o:o + 1], axis=0))
                    gs.append(gkv)
                prod = small.tile([128, BJ * E], f32, tag="prod", name="prod")
                pv = prod[:].rearrange("p (j e) -> p j e", e=E)
                sco = small.tile([128, 2 * BJ], f32, tag="sco", name="sco")
                for k in range(2):
                    nc.vector.tensor_tensor(out=pv, in0=gs[0][:, :, E:2 * E],
                                            in1=gs[k + 1][:, :, E:2 * E],
                                            op=ALU.mult)
                    nc.vector.tensor_reduce(
                        out=sco[:, k * BJ:(k + 1) * BJ], in_=pv,
                        axis=mybir.AxisListType.X, op=ALU.add)
                dd = small.tile([128, BJ], f32, tag="dd", name="dd")
                nc.vector.tensor_tensor(out=dd[:], in0=sco[:, 0:BJ],
                                        in1=sco[:, BJ:2 * BJ],
                                        op=ALU.subtract)
                aab = small.tile([128, BJ], f32, tag="aab", name="aab")
                nc.vector.tensor_scalar(out=aab[:], in0=dd[:], scalar1=-1.0,
                                        scalar2=None, op0=ALU.mult)
                nc.vector.tensor_tensor(out=aab[:], in0=aab[:], in1=dd[:],
                                        op=ALU.max)
                zex = small.tile([128, BJ], f32, tag="zex", name="zex")
                nc.scalar.activation(out=zex[:], in_=aab[:], func=AF.Exp,
                                     scale=-1.0)
                pol = small.tile([128, BJ], f32, tag="pol", name="pol")
                nc.vector.tensor_scalar(out=pol[:], in0=zex[:],
                                        scalar1=LOG1P_C[10],
                                        scalar2=LOG1P_C[9],
                                        op0=ALU.mult, op1=ALU.add)
                for k in range(8, -1, -1):
                    nc.vector.tensor_tensor(out=pol[:], in0=pol[:],
                                            in1=zex[:], op=ALU.mult)
                    nc.vector.tensor_scalar(out=pol[:], in0=pol[:],
                                            scalar1=LOG1P_C[k], scalar2=None,
                                            op0=ALU.add)
                nc.vector.tensor_scalar(out=dd[:], in0=dd[:], scalar1=-1.0,
                                        scalar2=0.0, op0=ALU.mult,
                                        op1=ALU.max)
                nc.vector.tensor_tensor(out=pol[:], in0=pol[:], in1=dd[:],
                                        op=ALU.add)
                nc.vector.tensor_reduce(out=blacc[:, b:b + 1], in_=pol[:],
                                        axis=mybir.AxisListType.X,
                                        op=ALU.add)

            # ================= program =================
            assemble(0)       # stages table 0 (no AGs)
            main_pass(0)      # AG(0) block, then gathers; stages table 1
            main_pass(1)      # AG(1) block first; stages table 2
            loss_pass(0)      # table 1 complete by now
            main_pass(2)      # AG(2) block first; stages table 3
            loss_pass(1)
            ag_block(3)
            loss_pass(2)

            # ---------------- final combine ----------------
            pack = small.tile([128, 2], f32, tag="pack", name="pack")
            nc.vector.tensor_reduce(out=pack[:, 0:1], in_=blacc[:],
                                    axis=mybir.AxisListType.X, op=ALU.add)
            nc.vector.tensor_reduce(out=pack[:, 1:2], in_=racc[:, :NREG],
                                    axis=mybir.AxisListType.X, op=ALU.add)
            fin_ps = ppy.tile([1, 2], f32, tag="fin", name="fin", bufs=1)
            nc.tensor.matmul(out=fin_ps[:], lhsT=onesf[:], rhs=pack[:],
                             start=True, stop=True)
            fin = small.tile([1, 2], f32, tag="fins", name="fins")
            nc.vector.tensor_copy(out=fin[:], in_=fin_ps[:])
            nc.sync.dma_start(out=lag_i[:], in_=fin[:])
            all_gather(lag_i, lag_o, ncores)
            lsb = small.tile([1, 2 * ncores], f32, tag="lsb", name="lsb")
            nc.sync.dma_start(
                out=lsb[:],
                in_=lag_o[:].rearrange("(o a) b -> o (a b)", o=1))
            bl = small.tile([1, 2], f32, tag="bl", name="bl")
            lv = lsb[:].rearrange("p (a b) -> p a b", b=2)
            nc.vector.tensor_reduce(out=bl[:, 0:1], in_=lv[:, :, 0:1],
                                    axis=mybir.AxisListType.XY, op=ALU.add)
            nc.vector.tensor_reduce(out=bl[:, 1:2], in_=lv[:, :, 1:2],
                                    axis=mybir.AxisListType.XY, op=ALU.add)
            res = small.tile([1, 1], f32, tag="res", name="res")
            nc.vector.tensor_scalar(out=res[:], in0=bl[:, 1:2],
                                    scalar1=cfg["reg_weight"] * 0.5,
                                    scalar2=None, op0=ALU.mult)
            nc.vector.tensor_tensor(out=res[:], in0=res[:], in1=bl[:, 0:1],
                                    op=ALU.add)
            nc.vector.tensor_scalar(out=res[:], in0=res[:],
                                    scalar1=1.0 / cfg["batch"],
                                    scalar2=None, op0=ALU.mult)
            nc.sync.dma_start(out=loss_out, in_=res[:])

    nc.compile()
    return nc


# ---------------------------------------------------------------------------
# Entry point
# ---------------------------------------------------------------------------
LAST_RESULTS = None


def kernel(**inputs) -> np.ndarray:
    global LAST_RESULTS
    cfg = FULL_CFG
    edges = np.asarray(inputs["edges"])
    arrs = make_schedule_and_arrays(edges, cfg)
    sched = arrs[0]
    in_maps = make_inputs_per_core(inputs, cfg, arrs)
    nc = build_program(cfg, sched)

    import os
    os.environ["BASS_NEVER_TRACE"] = "1"  # axon NTFF hook absent here
    from concourse import bass_utils
    res = bass_utils.run_bass_kernel_spmd(
        nc, in_maps, core_ids=list(range(cfg["ncores"])))
    LAST_RESULTS = res
    out = res.results[0]["loss"]
    return np.float32(out.reshape(-1)[0])



# revision 51
# speedup vs baseline: 1.1464x; 1.0162x over previous

# CRGCN multi-behavior GCN forward loss on 8 Trainium2 NeuronCores.
#
# Strategy (graph/data parallel, dest-node sharding):
#  - Nodes (users+items, 200000 -> padded 200704) are sharded row-wise across
#    8 cores (25088 = 196*128 nodes/core). Edges are partitioned by the shard
#    of their destination (col) node on the host, bucketed by (128-dest tile,
#    source bucket of 28672 rows) and padded so every 128-edge chunk maps to
#    one dest tile and one source bucket. The chunk schedule is the max over
#    cores so a single SPMD program fits all 8 cores.
#  - Per behavior each core holds a bf16 table T2 = [dinv*total | total]
#    ([200704, 128], 256B rows) for ALL nodes, produced by AllGather of
#    per-shard slabs. Message pass: dma_gather (int16 in-bucket indices) of
#    T2 rows for edge sources; a 0/1 one-hot (edge x dest-in-tile) built on
#    DVE from edge cols; PE matmul contracts edges, accumulating
#    S^T[feat, dest] = sum_e dinv[r_e]*total[r_e] x onehot in PSUM per dest
#    tile; then S @ W, *dinv[d], +b, l2-normalize, residual-accumulate into
#    the SBUF-resident fp32 total shard.
#  - deg (in-degree) is a one-hot x ones matmul (bf16, exact), per behavior,
#    from the same col data.
#  - BPR loss: batch rows sharded across cores; u/pos/neg rows fetched with
#    per-partition indirect DMA from the raw-total half of T2; dots +
#    softplus(-d) (relu + log1p poly) on-device; partials AllGathered so all
#    cores emit the identical final scalar.

import sys

sys.path.insert(0, "/opt/trn_rl_repo")

import dataclasses
import numpy as np

# ---------------- problem constants (hardcoded; kernel.py is standalone) ---
N_USERS = 100000
N_ITEMS = 100000
N_NODES = 200000
EMBED = 64
N_BEH = 3
BATCH = 4096
REG_WEIGHT = 1e-4
NCORES = 8

FULL_CFG = dict(
    ncores=NCORES,
    embed=EMBED,
    nbeh=N_BEH,
    shard=25088,          # 196 * 128
    nt=196,               # dest tiles per shard
    wt=128,               # T2 row width in bf16 elems (256B)
    nbuck=7,              # source buckets
    bucket=28672,         # rows per bucket (7 * 28672 = 200704)
    wtiles=8,             # dest tiles per gather window
    g=32,                 # chunks per one-hot build group
    flush=14,             # tiles per T2 staging flush (196 = 14*14)
    batch=BATCH,
    batch_per_core=BATCH // NCORES,   # 512
    n_nodes=N_NODES,
    reg_weight=REG_WEIGHT,
)


# ---------------------------------------------------------------------------
# Host-side preprocessing
# ---------------------------------------------------------------------------
def make_schedule_and_arrays(edges, cfg):
    """edges: [NB, 2, E]. Builds the (window, bucket, tile)-ordered common
    chunk schedule and the per-core col/idx arrays."""
    ncores = cfg["ncores"]
    NT = cfg["nt"]
    NB = cfg["nbeh"]
    NBK = cfg["nbuck"]
    BUCK = cfg["bucket"]
    WT = cfg["wtiles"]
    NW = (NT + WT - 1) // WT

    sched = {"C": [], "cells": [], "tiles": [], "windows": [],
             "tile_cstart": []}
    cols_arr = [[None] * NB for _ in range(ncores)]
    idx_arr = [[None] * NB for _ in range(ncores)]
    dinv_arr = np.zeros((ncores, NB, 128, NT), dtype=np.float32)

    n_nodes = cfg["n_nodes"]
    shard = cfg["shard"]
    for b in range(NB):
        row = np.asarray(edges[b, 0], dtype=np.int64)
        col = np.asarray(edges[b, 1], dtype=np.int64)
        # host-side in-degree -> dinv per core shard, [128, NT] layout
        deg = np.bincount(col, minlength=n_nodes).astype(np.float32)
        dinv_g = np.where(deg > 0,
                          1.0 / np.sqrt(np.maximum(deg, 1.0)),
                          0.0).astype(np.float32)
        dinv_pad = np.zeros(ncores * shard, dtype=np.float32)
        dinv_pad[:n_nodes] = dinv_g
        for s in range(ncores):
            dinv_arr[s, b] = dinv_pad[s * shard:(s + 1) * shard].reshape(
                NT, 128).T
        gt = col >> 7                       # global dest tile
        s_of = gt // NT                     # owning core
        t_of = gt - s_of * NT               # local dest tile
        # permuted table layout: bucket k holds piece k (PIECE local rows)
        # of every core's shard, so AllGather k is per-rank contiguous:
        # pos(n) = beta*BUCK + (n//SH)*PIECE + (n%SH)%PIECE,
        # beta = (n%SH)//PIECE
        PIECE = BUCK // ncores
        r_loc = row % shard
        beta = r_loc // PIECE               # source bucket (= piece id)
        # per (core, tile, bucket) counts
        cellkey = (s_of * NT + t_of) * NBK + beta
        cnt = np.bincount(cellkey, minlength=ncores * NT * NBK).reshape(
            ncores, NT, NBK)
        K_cell = -(-cnt.max(axis=0) // 128)           # [NT, NBK]
        empty_t = K_cell.sum(axis=1) == 0
        K_cell[empty_t, 0] = 1

        # gather order: (window, bucket, tile); consumption order:
        # (window, tile, bucket). Chunks get positions in both orders.
        C = int(K_cell.sum())
        cell_start = {}      # gather-order chunk start per cell
        cell_cstart = {}     # consumption-order chunk start per cell
        pos = 0
        for w in range(NW):
            ts = range(w * WT, min((w + 1) * WT, NT))
            for be in range(NBK):
                for t in ts:
                    if K_cell[t, be]:
                        cell_start[(t, be)] = pos
                        pos += int(K_cell[t, be])
        assert pos == C
        cpos = 0
        tile_cstart = np.zeros(NT + 1, dtype=np.int64)
        for w in range(NW):
            ts = range(w * WT, min((w + 1) * WT, NT))
            for t in ts:
                tile_cstart[t] = cpos
                for be in range(NBK):
                    if K_cell[t, be]:
                        cell_cstart[(t, be)] = cpos
                        cpos += int(K_cell[t, be])
        tile_cstart[NT] = cpos
        assert cpos == C

        # per-tile consumption: ordered chunk positions + total K per tile
        tiles = []
        for t in range(NT):
            plist = []
            for be in range(NBK):
                if K_cell[t, be]:
                    st = cell_start[(t, be)]
                    plist.extend(range(st, st + int(K_cell[t, be])))
            tiles.append(plist)

        # per-window gather segments: (bucket, pos_start, n_chunks)
        windows = []
        for w in range(NW):
            ts = range(w * WT, min((w + 1) * WT, NT))
            segs = []
            for be in range(NBK):
                n = int(sum(K_cell[t, be] for t in ts))
                if n:
                    st = min(cell_start[(t, be)] for t in ts
                             if K_cell[t, be])
                    segs.append((be, st, n))
            windows.append(segs)

        sched["C"].append(C)
        sched["cells"].append((K_cell, cell_start))
        sched["tiles"].append(tiles)
        sched["windows"].append(windows)
        sched["tile_cstart"].append(tile_cstart)

        # ------------- per-core arrays -------------
        starts_np = np.zeros((NT, NBK), dtype=np.int64)
        for (t, be), st in cell_start.items():
            starts_np[t, be] = st
        cstarts_np = np.zeros((NT, NBK), dtype=np.int64)
        for (t, be), st in cell_cstart.items():
            cstarts_np[t, be] = st
        for s in range(ncores):
            colv = np.full(C * 128, 128.0, dtype=np.float32)
            rowv = np.zeros(C * 128, dtype=np.int64)   # in-bucket idx
            sel = s_of == s
            r_s = row[sel]
            c_s = col[sel]
            t_s = t_of[sel]
            be_s = beta[sel]
            key = t_s * NBK + be_s
            order = np.argsort(key, kind="stable")
            r_s, c_s, t_s, be_s, key = (r_s[order], c_s[order], t_s[order],
                                        be_s[order], key[order])
            seg_start = np.searchsorted(key, np.arange(NT * NBK))
            within = np.arange(len(key)) - seg_start[key]
            dst = starts_np[t_s, be_s] * 128 + within
            cdst = cstarts_np[t_s, be_s] * 128 + within
            colv[cdst] = (c_s & 127).astype(np.float32)
            rowv[dst] = (r_s // shard) * (BUCK // ncores) \
                + (r_s % shard) % (BUCK // ncores)
            cols_arr[s][b] = np.ascontiguousarray(
                colv.reshape(C, 128).T).astype(np.uint8)   # [128, C]
            # idx16: [128, C*8]; gather element i -> [i%16 (+16k), off+i//16]
            iv = rowv.reshape(C * 128)
            i16 = np.zeros((16, C * 8), dtype=np.int16)
            ii = np.arange(C * 128)
            i16[ii % 16, ii // 16] = iv.astype(np.int16)
            idx_arr[s][b] = np.ascontiguousarray(i16)   # [16, C*8]

    return sched, cols_arr, idx_arr, dinv_arr


def make_inputs_per_core(inputs, cfg, sched_arrays):
    import ml_dtypes

    ncores = cfg["ncores"]
    SH = cfg["shard"]
    E = cfg["embed"]
    NB = cfg["nbeh"]
    BPC = cfg["batch_per_core"]
    BJ = BPC // 128
    n_nodes = cfg["n_nodes"]
    n_users = n_nodes // 2

    sched, cols_arr, idx_arr, dinv_arr = sched_arrays

    user_emb = np.asarray(inputs["user_emb"], dtype=np.float32)
    item_emb = np.asarray(inputs["item_emb"], dtype=np.float32)
    gcn_weight = np.asarray(inputs["gcn_weight"], dtype=np.float32)
    gcn_bias = np.asarray(inputs["gcn_bias"], dtype=np.float32)
    batch_data = np.asarray(inputs["batch_data"], dtype=np.int64)

    total0 = np.concatenate([user_emb, item_emb], axis=0)

    G = cfg["g"]
    iotar = np.repeat(np.arange(128, dtype=np.float32), G)[None, :].astype(
        ml_dtypes.bfloat16)                       # [1, 128*G], j//G values
    w_bf = gcn_weight.astype(ml_dtypes.bfloat16)
    bb = np.tile(gcn_bias[:, None, :], (1, 128, 1)).astype(np.float32)

    in_maps = []
    for s in range(ncores):
        lo = s * SH
        hi = min((s + 1) * SH, n_nodes)
        init_shard = np.zeros((SH, E), dtype=ml_dtypes.bfloat16)
        if hi > lo:
            init_shard[: hi - lo] = total0[lo:hi].astype(ml_dtypes.bfloat16)

        PIECE = cfg["bucket"] // ncores

        def pos(n):
            # node id -> permuted table row (see make_schedule_and_arrays)
            r = n % SH
            return (r // PIECE) * cfg["bucket"] + (n // SH) * PIECE \
                + r % PIECE

        bidx = np.zeros((NB * 3, 128, BJ), dtype=np.int32)
        rs = slice(s * BPC, (s + 1) * BPC)
        for b in range(NB):
            u = pos(batch_data[rs, b, 0].astype(np.int32))
            p = pos(batch_data[rs, b, 1].astype(np.int32) + n_users)
            n = pos(batch_data[rs, b, 2].astype(np.int32) + n_users)
            for k, v in enumerate((u, p, n)):
                bidx[b * 3 + k] = v.reshape(BJ, 128).T

        m = {
            "init_shard": init_shard,
            "iotar_in": iotar,
            "w_in": w_bf,
            "bb_in": bb,
            "bidx_in": bidx,
            "dinv_in": dinv_arr[s],
        }
        for b in range(NB):
            m[f"col{b}"] = cols_arr[s][b]
            m[f"idx{b}"] = idx_arr[s][b]
        in_maps.append(m)
    return in_maps


# ---------------------------------------------------------------------------
# Device program
# ---------------------------------------------------------------------------
def build_program(cfg, sched, sim=False):
    from concourse import bass, bacc, mybir, tile

    dt = mybir.dt
    AF = mybir.ActivationFunctionType
    ALU = mybir.AluOpType

    ncores = cfg["ncores"]
    NT = cfg["nt"]
    SH = cfg["shard"]
    NTOT = SH * ncores
    E = cfg["embed"]
    WT = cfg["wt"]            # 128 table cols
    NBK = cfg["nbuck"]
    BUCK = cfg["bucket"]
    WTL = cfg["wtiles"]
    G = cfg["g"]
    FLUSH = cfg["flush"]
    BPC = cfg["batch_per_core"]
    BJ = BPC // 128
    NB = cfg["nbeh"]
    NV = NB + 1
    NW = (NT + WTL - 1) // WTL

    C = sched["C"]
    tiles_md = sched["tiles"]
    windows_md = sched["windows"]
    cstart_md = sched["tile_cstart"]

    # max chunks in any window (for the staging tile size)
    wch_max = 0
    for b in range(NB):
        for w in range(NW):
            wch = sum(n for (_, _, n) in windows_md[b][w])
            wch_max = max(wch_max, wch)

    def bc(ap, where, n):
        newap = list(ap.ap)
        newap.insert(where, [0, n])
        return dataclasses.replace(ap, ap=newap)

    nc = bacc.Bacc("TRN2", target_bir_lowering=False, debug=False,
                   num_devices=1 if sim is True else ncores,
                   num_swdge_queues=4)

    def all_gather(src_tile, dst_tile, nrep):
        # sim mode: stand in for the collective with local HBM->HBM copies
        # of the same receive volume so TimelineSim can run (single-core,
        # no collectives) with comparable DMA load + dependencies.
        if sim is True:
            n = src_tile.shape[0]
            for r in range(nrep):
                nc.sync.dma_start(out=dst_tile[r * n:(r + 1) * n, :],
                                  in_=src_tile[:])
        else:
            nc.gpsimd.collective_compute(
                "AllGather", mybir.AluOpType.bypass,
                replica_groups=[list(range(nrep))],
                ins=[src_tile[:].opt()], outs=[dst_tile[:].opt()])

    f32, bf16, i32, i16 = dt.float32, dt.bfloat16, dt.int32, dt.int16
    shared = "Local"

    init_in = nc.dram_tensor("init_shard", [SH, E], bf16,
                             kind="ExternalInput").ap()
    iotar_in = nc.dram_tensor("iotar_in", [1, 128 * G], bf16,
                              kind="ExternalInput").ap()
    dinv_in = nc.dram_tensor("dinv_in", [NB, 128, NT], f32,
                             kind="ExternalInput").ap()
    w_in = nc.dram_tensor("w_in", [NB, E, E], bf16, kind="ExternalInput").ap()
    bb_in = nc.dram_tensor("bb_in", [NB, 128, E], f32,
                           kind="ExternalInput").ap()
    bidx_in = nc.dram_tensor("bidx_in", [NB * 3, 128, BJ], i32,
                             kind="ExternalInput").ap()
    col_in = [nc.dram_tensor(f"col{b}", [128, C[b]], dt.uint8,
                             kind="ExternalInput").ap() for b in range(NB)]
    idx16_in = [nc.dram_tensor(f"idx{b}", [16, C[b] * 8], i16,
                              kind="ExternalInput").ap() for b in range(NB)]
    loss_out = nc.dram_tensor("loss", [1, 1], f32, kind="ExternalOutput").ap()

    with tile.TileContext(nc) as tc:
        with (
            tc.tile_pool(name="dram", bufs=1, space="DRAM") as dpool,
            tc.tile_pool(name="pers", bufs=1) as pers,
            tc.tile_pool(name="work", bufs=2) as work,
            tc.tile_pool(name="small", bufs=4) as small,
            tc.tile_pool(name="ppx", bufs=3, space="PSUM") as ppx,
            tc.tile_pool(name="ppy", bufs=2, space="PSUM") as ppy,
        ):
            t2s = [dpool.tile([SH, WT], bf16, tag=f"t2s{v}",
                              name=f"t2s{v}") for v in range(NV)]
            idxr = [dpool.tile([128, C[b] * 8], i16, tag=f"idxr{b}",
                               name=f"idxr{b}") for b in range(NB)]
            t2f = [dpool.tile([NTOT, WT], bf16, tag=f"t2f{v}",
                              name=f"t2f{v}", addr_space=shared)
                   for v in range(NV)]
            lag_i = dpool.tile([1, 2], f32, tag="lag_i", name="lag_i")
            lag_o = dpool.tile([ncores, 2], f32, tag="lag_o", name="lag_o",
                               addr_space=shared)

            tot = pers.tile([128, NT * E], f32, tag="tot", name="tot")
            irep = pers.tile([128, 128 * G], bf16, tag="irep", name="irep")
            wsb = pers.tile([E, NB * E], bf16, tag="wsb", name="wsb")
            bbsb = pers.tile([128, NB * E], f32, tag="bbsb", name="bbsb")
            bidx = pers.tile([128, NB * 3 * BJ], i32, tag="bidx", name="bidx")
            dinvsb = pers.tile([128, NB * NT], f32, tag="dinvsb",
                               name="dinvsb")
            dinv3z = pers.tile([128, 1], f32, tag="dinv3z", name="dinv3z")
            onesf = pers.tile([128, 1], f32, tag="onesf", name="onesf")
            racc = pers.tile([128, 16], f32, tag="racc", name="racc")
            blacc = pers.tile([128, NB], f32, tag="blacc", name="blacc")

            nc.sync.dma_start(
                out=irep[:].rearrange("p (a x) -> p a x", a=1),
                in_=bc(iotar_in, 0, 128))
            nc.sync.dma_start(
                out=dinvsb[:].rearrange("p (b t) -> p b t", b=NB),
                in_=dinv_in.rearrange("b p t -> p b t"))
            nc.sync.dma_start(
                out=wsb[:].rearrange("k (b e) -> k b e", b=NB),
                in_=w_in.rearrange("b k e -> k b e"))
            nc.sync.dma_start(
                out=bbsb[:].rearrange("p (b e) -> p b e", b=NB),
                in_=bb_in.rearrange("b p e -> p b e"))
            nc.sync.dma_start(
                out=bidx[:].rearrange("p (a j) -> p a j", a=NB * 3),
                in_=bidx_in.rearrange("a p j -> p a j"))
            for b in range(NB):
                nc.sync.dma_start(
                    out=idxr[b][:].rearrange("(a q) x -> a q x", a=8),
                    in_=bc(idx16_in[b], 0, 8))
            totb = work.tile([128, NT * E], bf16, tag="totb", name="totb",
                             bufs=1)
            nc.sync.dma_start(
                out=totb[:].rearrange("p (t e) -> p t e", e=E),
                in_=init_in.rearrange("(t p) e -> p t e", p=128))
            nc.vector.tensor_copy(out=tot[:], in_=totb[:])
            nc.vector.memset(onesf[:], 1.0)
            nc.vector.memset(dinv3z[:], 0.0)
            epsb = pers.tile([128, 1], f32, tag="epsb", name="epsb")
            nc.vector.memset(epsb[:], 1e-24)

            def dinv_ap(v, t):
                # per-partition dinv scalar for (behavior v, dest tile t)
                if v < NB:
                    return dinvsb[:, v * NT + t:v * NT + t + 1]
                return dinv3z[:, 0:1]

            # reg term: sum of squares of the initial embeddings
            NREG = (NT * E + 1023) // 1024
            sqd = pers.tile([128, 1024], f32, tag="sqd", name="sqd")
            for i in range(NREG):
                sl = slice(i * 1024, min((i + 1) * 1024, NT * E))
                nc.scalar.activation(out=sqd[:, : sl.stop - sl.start],
                                     in_=tot[:, sl], func=AF.Square,
                                     accum_out=racc[:, i:i + 1])

            # ------- lazy consumption-ordered one-hot group builder -------
            class IndBuilder:
                """Builds one-hot groups for consumption positions
                [c0, c0+wch) on demand, in order, so only a few groups are
                live at once. Layout [p, d, g] (g innermost) so every
                operand of the is_equal has stride-1 innermost dims and the
                DVE runs in its 2x perf mode."""

                def __init__(self, cs, c0, wch):
                    self.cs, self.c0, self.wch = cs, c0, wch
                    self.groups = {}

                def get(self, cpos):
                    rel = cpos - self.c0
                    g0 = (rel // G) * G
                    if g0 not in self.groups:
                        gw = min(G, self.wch - g0)
                        ind = work.tile([128, 128 * G], dt.bfloat16,
                                        tag="ind", name="ind", bufs=3)
                        iv = ind[:].rearrange("p (d g) -> p d g", g=G)
                        nc.vector.tensor_tensor(
                            out=iv[:, :, :gw],
                            in0=irep[:].rearrange(
                                "p (d g) -> p d g", g=G)[:, :, :gw],
                            in1=bc(self.cs[:, g0:g0 + gw], 1, 128),
                            op=ALU.is_equal)
                        self.groups[g0] = iv
                    return self.groups[g0], rel - g0

            # ------------- T2 staging + per-piece AllGather -------------
            # Table v is built tile-by-tile (fused into main_pass(v-1)'s
            # post_tile stream); every 2 flushes completes one PIECE of the
            # local slab and fires that piece's AllGather, so collectives
            # overlap the remaining compute of the producing behavior.
            PIECE = BUCK // ncores

            def ag_piece(v, k):
                src = t2s[v][k * PIECE:(k + 1) * PIECE, :]
                if sim == "noag":
                    return
                if sim:
                    for r in range(ncores):
                        o = k * BUCK + r * PIECE
                        nc.sync.dma_start(out=t2f[v][o:o + PIECE, :],
                                          in_=src)
                else:
                    nc.gpsimd.collective_compute(
                        "AllGather", mybir.AluOpType.bypass,
                        replica_groups=[list(range(ncores))],
                        ins=[src.opt()],
                        outs=[t2f[v][k * BUCK:(k + 1) * BUCK, :].opt()])

            class Stager:
                def __init__(self, v):
                    self.v = v
                    self.s65 = None

                def stage(self, t):
                    i = t % FLUSH
                    if i == 0:
                        self.s65 = work.tile([128, FLUSH * WT], bf16,
                                             tag="s65", name="s65")
                    totsl = tot[:, t * E:(t + 1) * E]
                    nc.vector.tensor_scalar(
                        out=self.s65[:, i * WT:i * WT + E], in0=totsl,
                        scalar1=dinv_ap(self.v, t), scalar2=None,
                        op0=ALU.mult)
                    nc.scalar.copy(
                        out=self.s65[:, i * WT + E:i * WT + 2 * E],
                        in_=totsl)
                    if i == FLUSH - 1:
                        tf = t - i
                        nc.sync.dma_start(
                            out=t2s[self.v][:].rearrange(
                                "(t p) w -> p t w", p=128)[:, tf:t + 1, :],
                            in_=self.s65[:, :FLUSH * WT].rearrange(
                                "p (t w) -> p t w", w=WT))

            def assemble(v):
                st = Stager(v)
                for t in range(NT):
                    st.stage(t)

            # ---------------- main pass ----------------
            self_q = [0]
            NQ = 4

            def ag_block(v):
                # all 7 piece triggers up-front: their input flushes
                # completed during the previous pass, so these don't stall
                # the in-order Pool queue, and the transfers stream ahead
                # of the bucket-ordered gathers that consume them.
                for k in range(NBK):
                    ag_piece(v, k)

            def main_pass(b):
                ag_block(b)
                stg = Stager(b + 1)
                for w in range(NW):
                    segs = windows_md[b][w]
                    g0 = min(st for (_, st, _) in segs)
                    wch = sum(n for (_, _, n) in segs)
                    t0w = w * WTL
                    c0 = int(cstart_md[b][t0w])
                    cs8 = small.tile([128, wch_max], dt.uint8, tag="cs8",
                                     name="cs8", bufs=2)
                    nc.sync.dma_start(out=cs8[:, :wch],
                                      in_=col_in[b][:, c0:c0 + wch])
                    cs = small.tile([128, wch_max], bf16, tag="cs", name="cs",
                                    bufs=3)
                    nc.vector.tensor_copy(out=cs[:, :wch], in_=cs8[:, :wch])
                    ixs = small.tile([128, wch_max * 8], i16, tag="ixs",
                                     name="ixs", bufs=2)
                    nc.sync.dma_start(
                        out=ixs[:, :wch * 8],
                        in_=idxr[b][:, g0 * 8:(g0 + wch) * 8])
                    gat = work.tile([128, wch_max * 128], bf16, tag="gat",
                                    name="gat")
                    gv = gat[:].rearrange("p (c e) -> p c e", e=128)
                    for (be, st, n) in segs:
                        for o in range(0, n, 8):
                            if sim == "nogather":
                                break
                            m = min(8, n - o)
                            so = st - g0 + o
                            nc.gpsimd.dma_gather(
                                out_ap=gv[:, so:so + m, :],
                                in_ap=t2f[b][be * BUCK:(be + 1) * BUCK, :],
                                idxs_ap=ixs[:, so * 8:(so + m) * 8],
                                num_idxs=m * 128,
                                num_idxs_reg=m * 128,
                                elem_size=WT,
                                single_packet=False,
                                queue_num=self_q[0] % NQ)
                            self_q[0] += 1
                    bld = IndBuilder(cs, c0, wch)
                    for t in range(t0w, min(t0w + WTL, NT)):
                        plist = tiles_md[b][t]
                        xt_ps = ppx.tile([E, 128], f32, tag="xt", name="xt")
                        for j, pos in enumerate(plist):
                            iv, r = bld.get(int(cstart_md[b][t]) + j)
                            nc.tensor.matmul(
                                out=xt_ps[:],
                                lhsT=gv[:, pos - g0, 0:E],
                                rhs=iv[:, :, r],
                                start=(j == 0), stop=(j == len(plist) - 1))
                        post_tile(b, t, xt_ps)
                        stg.stage(t)

            def post_tile(b, t, xt_ps):
                xts = small.tile([E, 128], bf16, tag="xts", name="xts")
                nc.scalar.copy(out=xts[:], in_=xt_ps[:])
                y_ps = ppy.tile([128, E], f32, tag="y", name="y")
                nc.tensor.matmul(out=y_ps[:], lhsT=xts[:],
                                 rhs=wsb[:, b * E:(b + 1) * E],
                                 start=True, stop=True)
                z = small.tile([128, E], f32, tag="z", name="z")
                ss = small.tile([128, 1], f32, tag="ss", name="ss")
                # z = y*dinv_col + bias
                nc.vector.scalar_tensor_tensor(
                    out=z[:], in0=y_ps[:], scalar=dinv_ap(b, t),
                    in1=bbsb[:, b * E:(b + 1) * E],
                    op0=ALU.mult, op1=ALU.add)
                sq = small.tile([128, E], f32, tag="sq", name="sq")
                nc.scalar.activation(out=sq[:], in_=z[:], func=AF.Square,
                                     accum_out=ss[:])
                # sqrt(ss + 1e-24) ~= max(sqrt(ss), 1e-12)
                nc.scalar.activation(out=ss[:], in_=ss[:], func=AF.Sqrt,
                                     bias=epsb[:, 0:1])
                rin = small.tile([128, 1], f32, tag="rin", name="rin")
                nc.vector.reciprocal(out=rin[:], in_=ss[:])
                totsl = tot[:, t * E:(t + 1) * E]
                # tot += z * rin
                nc.vector.scalar_tensor_tensor(
                    out=totsl, in0=z[:], scalar=rin[:, 0:1], in1=totsl,
                    op0=ALU.mult, op1=ALU.add)

            # ---------------- loss ----------------
            LOG1P_C = [2.4139025189026897e-09, 0.9999996692324197,
                       -0.499988759640371, 0.3331669190104936,
                       -0.2486582066434577, 0.19337637102999028,
                       -0.14517645896753417, 0.09470379566439587,
                       -0.04713346504062944, 0.015145372148722138,
                       -0.002288060381570317]

            def loss_pass(b):
                gs = []
                for k in range(3):
                    gk = small.tile([128, BJ * WT], bf16, tag=f"bg{k}",
                                    name=f"bg{k}")
                    gkv = gk[:].rearrange("p (j w) -> p j w", w=WT)
                    for j in range(BJ):
                        o = (b * 3 + k) * BJ + j
                        nc.gpsimd.indirect_dma_start(
                            out=gkv[:, j, :],
                            out_offset=None,
                            in_=t2f[b + 1][:],
                            in_offset=bass.IndirectOffsetOnAxis(
                                ap=bidx[:, o:o + 1], axis=0))
                    gs.append(gkv)
                prod = small.tile([128, BJ * E], f32, tag="prod", name="prod")
                pv = prod[:].rearrange("p (j e) -> p j e", e=E)
                sco = small.tile([128, 2 * BJ], f32, tag="sco", name="sco")
                for k in range(2):
                    nc.vector.tensor_tensor(out=pv, in0=gs[0][:, :, E:2 * E],
                                            in1=gs[k + 1][:, :, E:2 * E],
                                            op=ALU.mult)
                    nc.vector.tensor_reduce(
                        out=sco[:, k * BJ:(k + 1) * BJ], in_=pv,
                        axis=mybir.AxisListType.X, op=ALU.add)
                dd = small.tile([128, BJ], f32, tag="dd", name="dd")
                nc.vector.tensor_tensor(out=dd[:], in0=sco[:, 0:BJ],
                                        in1=sco[:, BJ:2 * BJ],
                                        op=ALU.subtract)
                aab = small.tile([128, BJ], f32, tag="aab", name="aab")
                nc.vector.tensor_scalar(out=aab[:], in0=dd[:], scalar1=-1.0,
                                        scalar2=None, op0=ALU.mult)
                nc.vector.tensor_tensor(out=aab[:], in0=aab[:], in1=dd[:],
                                        op=ALU.max)
                zex = small.tile([128, BJ], f32, tag="zex", name="zex")
                nc.scalar.activation(out=zex[:], in_=aab[:], func=AF.Exp,
                                     scale=-1.0)
                pol = small.tile([128, BJ], f32, tag="pol", name="pol")
                nc.vector.tensor_scalar(out=pol[:], in0=zex[:],
                                        scalar1=LOG1P_C[10],
                                        scalar2=LOG1P_C[9],
                                        op0=ALU.mult, op1=ALU.add)
                for k in range(8, -1, -1):
                    nc.vector.tensor_tensor(out=pol[:], in0=pol[:],
                                            in1=zex[:], op=ALU.mult)
                    nc.vector.tensor_scalar(out=pol[:], in0=pol[:],
                                            scalar1=LOG1P_C[k], scalar2=None,
                                            op0=ALU.add)
                nc.vector.tensor_scalar(out=dd[:], in0=dd[:], scalar1=-1.0,
                                        scalar2=0.0, op0=ALU.mult,
                                        op1=ALU.max)
                nc.vector.tensor_tensor(out=pol[:], in0=pol[:], in1=dd[:],
                                        op=ALU.add)
                nc.vector.tensor_reduce(out=blacc[:, b:b + 1], in_=pol[:],
                                        axis=mybir.AxisListType.X,
                                        op=ALU.add)

            # ================= program =================
            assemble(0)       # stages table 0 (no AGs)
            main_pass(0)      # AG(0) block, then gathers; stages table 1
            main_pass(1)      # AG(1) block first; stages table 2
            loss_pass(0)      # table 1 complete by now
            main_pass(2)      # AG(2) block first; stages table 3
            loss_pass(1)
            ag_block(3)
            loss_pass(2)

            # ---------------- final combine ----------------
            pack = small.tile([128, 2], f32, tag="pack", name="pack")
            nc.vector.tensor_reduce(out=pack[:, 0:1], in_=blacc[:],
                                    axis=mybir.AxisListType.X, op=ALU.add)
            nc.vector.tensor_reduce(out=pack[:, 1:2], in_=racc[:, :NREG],
                                    axis=mybir.AxisListType.X, op=ALU.add)
            fin_ps = ppy.tile([1, 2], f32, tag="fin", name="fin", bufs=1)
            nc.tensor.matmul(out=fin_ps[:], lhsT=onesf[:], rhs=pack[:],
                             start=True, stop=True)
            fin = small.tile([1, 2], f32, tag="fins", name="fins")
            nc.vector.tensor_copy(out=fin[:], in_=fin_ps[:])
            nc.sync.dma_start(out=lag_i[:], in_=fin[:])
            all_gather(lag_i, lag_o, ncores)
            lsb = small.tile([1, 2 * ncores], f32, tag="lsb", name="lsb")
            nc.sync.dma_start(
                out=lsb[:],
                in_=lag_o[:].rearrange("(o a) b -> o (a b)", o=1))
            bl = small.tile([1, 2], f32, tag="bl", name="bl")
            lv = lsb[:].rearrange("p (a b) -> p a b", b=2)
            nc.vector.tensor_reduce(out=bl[:, 0:1], in_=lv[:, :, 0:1],
                                    axis=mybir.AxisListType.XY, op=ALU.add)
            nc.vector.tensor_reduce(out=bl[:, 1:2], in_=lv[:, :, 1:2],
                                    axis=mybir.AxisListType.XY, op=ALU.add)
            res = small.tile([1, 1], f32, tag="res", name="res")
            nc.vector.tensor_scalar(out=res[:], in0=bl[:, 1:2],
                                    scalar1=cfg["reg_weight"] * 0.5,
                                    scalar2=None, op0=ALU.mult)
            nc.vector.tensor_tensor(out=res[:], in0=res[:], in1=bl[:, 0:1],
                                    op=ALU.add)
            nc.vector.tensor_scalar(out=res[:], in0=res[:],
                                    scalar1=1.0 / cfg["batch"],
                                    scalar2=None, op0=ALU.mult)
            nc.sync.dma_start(out=loss_out, in_=res[:])

    nc.compile()
    return nc


# ---------------------------------------------------------------------------
# Entry point
# ---------------------------------------------------------------------------
LAST_RESULTS = None


def kernel(**inputs) -> np.ndarray:
    global LAST_RESULTS
    cfg = FULL_CFG
    edges = np.asarray(inputs["edges"])
    arrs = make_schedule_and_arrays(edges, cfg)
    sched = arrs[0]
    in_maps = make_inputs_per_core(inputs, cfg, arrs)
    nc = build_program(cfg, sched)

    import os
    os.environ["BASS_NEVER_TRACE"] = "1"  # axon NTFF hook absent here
    from concourse import bass_utils
    res = bass_utils.run_bass_kernel_spmd(
        nc, in_maps, core_ids=list(range(cfg["ncores"])))
    LAST_RESULTS = res
    out = res.results[0]["loss"]
    return np.float32(out.reshape(-1)[0])



# revision 53
# speedup vs baseline: 1.1477x; 1.0012x over previous

# CRGCN multi-behavior GCN forward loss on 8 Trainium2 NeuronCores.
#
# Strategy (graph/data parallel, dest-node sharding):
#  - Nodes (users+items, 200000 -> padded 200704) are sharded row-wise across
#    8 cores (25088 = 196*128 nodes/core). Edges are partitioned by the shard
#    of their destination (col) node on the host, bucketed by (128-dest tile,
#    source bucket of 28672 rows) and padded so every 128-edge chunk maps to
#    one dest tile and one source bucket. The chunk schedule is the max over
#    cores so a single SPMD program fits all 8 cores.
#  - Per behavior each core holds a bf16 table T2 = [dinv*total | total]
#    ([200704, 128], 256B rows) for ALL nodes, produced by AllGather of
#    per-shard slabs. Message pass: dma_gather (int16 in-bucket indices) of
#    T2 rows for edge sources; a 0/1 one-hot (edge x dest-in-tile) built on
#    DVE from edge cols; PE matmul contracts edges, accumulating
#    S^T[feat, dest] = sum_e dinv[r_e]*total[r_e] x onehot in PSUM per dest
#    tile; then S @ W, *dinv[d], +b, l2-normalize, residual-accumulate into
#    the SBUF-resident fp32 total shard.
#  - deg (in-degree) is a one-hot x ones matmul (bf16, exact), per behavior,
#    from the same col data.
#  - BPR loss: batch rows sharded across cores; u/pos/neg rows fetched with
#    per-partition indirect DMA from the raw-total half of T2; dots +
#    softplus(-d) (relu + log1p poly) on-device; partials AllGathered so all
#    cores emit the identical final scalar.

import sys

sys.path.insert(0, "/opt/trn_rl_repo")

import dataclasses
import numpy as np

# ---------------- problem constants (hardcoded; kernel.py is standalone) ---
N_USERS = 100000
N_ITEMS = 100000
N_NODES = 200000
EMBED = 64
N_BEH = 3
BATCH = 4096
REG_WEIGHT = 1e-4
NCORES = 8

FULL_CFG = dict(
    ncores=NCORES,
    embed=EMBED,
    nbeh=N_BEH,
    shard=25088,          # 196 * 128
    nt=196,               # dest tiles per shard
    wt=128,               # T2 row width in bf16 elems (256B)
    nbuck=7,              # source buckets
    bucket=28672,         # rows per bucket (7 * 28672 = 200704)
    wtiles=8,             # dest tiles per gather window
    g=32,                 # chunks per one-hot build group
    flush=14,             # tiles per T2 staging flush (196 = 14*14)
    batch=BATCH,
    batch_per_core=BATCH // NCORES,   # 512
    n_nodes=N_NODES,
    reg_weight=REG_WEIGHT,
)


# ---------------------------------------------------------------------------
# Host-side preprocessing
# ---------------------------------------------------------------------------
def make_schedule_and_arrays(edges, cfg):
    """edges: [NB, 2, E]. Builds the (window, bucket, tile)-ordered common
    chunk schedule and the per-core col/idx arrays."""
    ncores = cfg["ncores"]
    NT = cfg["nt"]
    NB = cfg["nbeh"]
    NBK = cfg["nbuck"]
    BUCK = cfg["bucket"]
    WT = cfg["wtiles"]
    NW = (NT + WT - 1) // WT

    sched = {"C": [], "cells": [], "tiles": [], "windows": [],
             "tile_cstart": []}
    cols_arr = [[None] * NB for _ in range(ncores)]
    idx_arr = [[None] * NB for _ in range(ncores)]
    dinv_arr = np.zeros((ncores, NB, 128, NT), dtype=np.float32)

    n_nodes = cfg["n_nodes"]
    shard = cfg["shard"]
    for b in range(NB):
        row = np.asarray(edges[b, 0], dtype=np.int64)
        col = np.asarray(edges[b, 1], dtype=np.int64)
        # host-side in-degree -> dinv per core shard, [128, NT] layout
        deg = np.bincount(col, minlength=n_nodes).astype(np.float32)
        dinv_g = np.where(deg > 0,
                          1.0 / np.sqrt(np.maximum(deg, 1.0)),
                          0.0).astype(np.float32)
        dinv_pad = np.zeros(ncores * shard, dtype=np.float32)
        dinv_pad[:n_nodes] = dinv_g
        for s in range(ncores):
            dinv_arr[s, b] = dinv_pad[s * shard:(s + 1) * shard].reshape(
                NT, 128).T
        gt = col >> 7                       # global dest tile
        s_of = gt // NT                     # owning core
        t_of = gt - s_of * NT               # local dest tile
        # permuted table layout: bucket k holds piece k (PIECE local rows)
        # of every core's shard, so AllGather k is per-rank contiguous:
        # pos(n) = beta*BUCK + (n//SH)*PIECE + (n%SH)%PIECE,
        # beta = (n%SH)//PIECE
        PIECE = BUCK // ncores
        r_loc = row % shard
        beta = r_loc // PIECE               # source bucket (= piece id)
        # per (core, tile, bucket) counts
        cellkey = (s_of * NT + t_of) * NBK + beta
        cnt = np.bincount(cellkey, minlength=ncores * NT * NBK).reshape(
            ncores, NT, NBK)
        K_cell = -(-cnt.max(axis=0) // 128)           # [NT, NBK]
        empty_t = K_cell.sum(axis=1) == 0
        K_cell[empty_t, 0] = 1

        # gather order: (window, bucket, tile); consumption order:
        # (window, tile, bucket). Chunks get positions in both orders.
        C = int(K_cell.sum())
        cell_start = {}      # gather-order chunk start per cell
        cell_cstart = {}     # consumption-order chunk start per cell
        pos = 0
        for w in range(NW):
            ts = range(w * WT, min((w + 1) * WT, NT))
            for be in range(NBK):
                for t in ts:
                    if K_cell[t, be]:
                        cell_start[(t, be)] = pos
                        pos += int(K_cell[t, be])
        assert pos == C
        cpos = 0
        tile_cstart = np.zeros(NT + 1, dtype=np.int64)
        for w in range(NW):
            ts = range(w * WT, min((w + 1) * WT, NT))
            for t in ts:
                tile_cstart[t] = cpos
                for be in range(NBK):
                    if K_cell[t, be]:
                        cell_cstart[(t, be)] = cpos
                        cpos += int(K_cell[t, be])
        tile_cstart[NT] = cpos
        assert cpos == C

        # per-tile consumption: ordered chunk positions + total K per tile
        tiles = []
        for t in range(NT):
            plist = []
            for be in range(NBK):
                if K_cell[t, be]:
                    st = cell_start[(t, be)]
                    plist.extend(range(st, st + int(K_cell[t, be])))
            tiles.append(plist)

        # per-window gather segments: (bucket, pos_start, n_chunks)
        windows = []
        for w in range(NW):
            ts = range(w * WT, min((w + 1) * WT, NT))
            segs = []
            for be in range(NBK):
                n = int(sum(K_cell[t, be] for t in ts))
                if n:
                    st = min(cell_start[(t, be)] for t in ts
                             if K_cell[t, be])
                    segs.append((be, st, n))
            windows.append(segs)

        sched["C"].append(C)
        sched["cells"].append((K_cell, cell_start))
        sched["tiles"].append(tiles)
        sched["windows"].append(windows)
        sched["tile_cstart"].append(tile_cstart)

        # ------------- per-core arrays -------------
        starts_np = np.zeros((NT, NBK), dtype=np.int64)
        for (t, be), st in cell_start.items():
            starts_np[t, be] = st
        cstarts_np = np.zeros((NT, NBK), dtype=np.int64)
        for (t, be), st in cell_cstart.items():
            cstarts_np[t, be] = st
        for s in range(ncores):
            colv = np.full(C * 128, 128.0, dtype=np.float32)
            rowv = np.zeros(C * 128, dtype=np.int64)   # in-bucket idx
            sel = s_of == s
            r_s = row[sel]
            c_s = col[sel]
            t_s = t_of[sel]
            be_s = beta[sel]
            key = t_s * NBK + be_s
            order = np.argsort(key, kind="stable")
            r_s, c_s, t_s, be_s, key = (r_s[order], c_s[order], t_s[order],
                                        be_s[order], key[order])
            seg_start = np.searchsorted(key, np.arange(NT * NBK))
            within = np.arange(len(key)) - seg_start[key]
            dst = starts_np[t_s, be_s] * 128 + within
            cdst = cstarts_np[t_s, be_s] * 128 + within
            colv[cdst] = (c_s & 127).astype(np.float32)
            rowv[dst] = (r_s // shard) * (BUCK // ncores) \
                + (r_s % shard) % (BUCK // ncores)
            cols_arr[s][b] = np.ascontiguousarray(
                colv.reshape(C, 128).T).astype(np.uint8)   # [128, C]
            # idx16: [128, C*8]; gather element i -> [i%16 (+16k), off+i//16]
            iv = rowv.reshape(C * 128)
            i16 = np.zeros((16, C * 8), dtype=np.int16)
            ii = np.arange(C * 128)
            i16[ii % 16, ii // 16] = iv.astype(np.int16)
            idx_arr[s][b] = np.ascontiguousarray(i16)   # [16, C*8]

    return sched, cols_arr, idx_arr, dinv_arr


def make_inputs_per_core(inputs, cfg, sched_arrays):
    import ml_dtypes

    ncores = cfg["ncores"]
    SH = cfg["shard"]
    E = cfg["embed"]
    NB = cfg["nbeh"]
    BPC = cfg["batch_per_core"]
    BJ = BPC // 128
    n_nodes = cfg["n_nodes"]
    n_users = n_nodes // 2

    sched, cols_arr, idx_arr, dinv_arr = sched_arrays

    user_emb = np.asarray(inputs["user_emb"], dtype=np.float32)
    item_emb = np.asarray(inputs["item_emb"], dtype=np.float32)
    gcn_weight = np.asarray(inputs["gcn_weight"], dtype=np.float32)
    gcn_bias = np.asarray(inputs["gcn_bias"], dtype=np.float32)
    batch_data = np.asarray(inputs["batch_data"], dtype=np.int64)

    total0 = np.concatenate([user_emb, item_emb], axis=0)

    G = cfg["g"]
    iotar = np.repeat(np.arange(128, dtype=np.float32), G)[None, :].astype(
        ml_dtypes.bfloat16)                       # [1, 128*G], j//G values
    w_bf = gcn_weight.astype(ml_dtypes.bfloat16)
    bb = np.tile(gcn_bias[:, None, :], (1, 128, 1)).astype(np.float32)

    in_maps = []
    for s in range(ncores):
        lo = s * SH
        hi = min((s + 1) * SH, n_nodes)
        init_shard = np.zeros((SH, E), dtype=ml_dtypes.bfloat16)
        if hi > lo:
            init_shard[: hi - lo] = total0[lo:hi].astype(ml_dtypes.bfloat16)

        PIECE = cfg["bucket"] // ncores

        def pos(n):
            # node id -> permuted table row (see make_schedule_and_arrays)
            r = n % SH
            return (r // PIECE) * cfg["bucket"] + (n // SH) * PIECE \
                + r % PIECE

        bidx = np.zeros((NB * 3, 128, BJ), dtype=np.int32)
        rs = slice(s * BPC, (s + 1) * BPC)
        for b in range(NB):
            u = pos(batch_data[rs, b, 0].astype(np.int32))
            p = pos(batch_data[rs, b, 1].astype(np.int32) + n_users)
            n = pos(batch_data[rs, b, 2].astype(np.int32) + n_users)
            for k, v in enumerate((u, p, n)):
                bidx[b * 3 + k] = v.reshape(BJ, 128).T

        m = {
            "init_shard": init_shard,
            "iotar_in": iotar,
            "w_in": w_bf,
            "bb_in": bb,
            "bidx_in": bidx,
            "dinv_in": dinv_arr[s],
        }
        for b in range(NB):
            m[f"col{b}"] = cols_arr[s][b]
            m[f"idx{b}"] = idx_arr[s][b]
        in_maps.append(m)
    return in_maps


# ---------------------------------------------------------------------------
# Device program
# ---------------------------------------------------------------------------
def build_program(cfg, sched, sim=False):
    from concourse import bass, bacc, mybir, tile

    dt = mybir.dt
    AF = mybir.ActivationFunctionType
    ALU = mybir.AluOpType

    ncores = cfg["ncores"]
    NT = cfg["nt"]
    SH = cfg["shard"]
    NTOT = SH * ncores
    E = cfg["embed"]
    WT = cfg["wt"]            # 128 table cols
    NBK = cfg["nbuck"]
    BUCK = cfg["bucket"]
    WTL = cfg["wtiles"]
    G = cfg["g"]
    FLUSH = cfg["flush"]
    BPC = cfg["batch_per_core"]
    BJ = BPC // 128
    NB = cfg["nbeh"]
    NV = NB + 1
    NW = (NT + WTL - 1) // WTL

    C = sched["C"]
    tiles_md = sched["tiles"]
    windows_md = sched["windows"]
    cstart_md = sched["tile_cstart"]

    # max chunks in any window (for the staging tile size)
    wch_max = 0
    for b in range(NB):
        for w in range(NW):
            wch = sum(n for (_, _, n) in windows_md[b][w])
            wch_max = max(wch_max, wch)

    def bc(ap, where, n):
        newap = list(ap.ap)
        newap.insert(where, [0, n])
        return dataclasses.replace(ap, ap=newap)

    nc = bacc.Bacc("TRN2", target_bir_lowering=False, debug=False,
                   num_devices=1 if sim is True else ncores,
                   num_swdge_queues=4, dynamic_dma_scratch_size=24576)

    def all_gather(src_tile, dst_tile, nrep):
        # sim mode: stand in for the collective with local HBM->HBM copies
        # of the same receive volume so TimelineSim can run (single-core,
        # no collectives) with comparable DMA load + dependencies.
        if sim is True:
            n = src_tile.shape[0]
            for r in range(nrep):
                nc.sync.dma_start(out=dst_tile[r * n:(r + 1) * n, :],
                                  in_=src_tile[:])
        else:
            nc.gpsimd.collective_compute(
                "AllGather", mybir.AluOpType.bypass,
                replica_groups=[list(range(nrep))],
                ins=[src_tile[:].opt()], outs=[dst_tile[:].opt()])

    f32, bf16, i32, i16 = dt.float32, dt.bfloat16, dt.int32, dt.int16
    shared = "Local"

    init_in = nc.dram_tensor("init_shard", [SH, E], bf16,
                             kind="ExternalInput").ap()
    iotar_in = nc.dram_tensor("iotar_in", [1, 128 * G], bf16,
                              kind="ExternalInput").ap()
    dinv_in = nc.dram_tensor("dinv_in", [NB, 128, NT], f32,
                             kind="ExternalInput").ap()
    w_in = nc.dram_tensor("w_in", [NB, E, E], bf16, kind="ExternalInput").ap()
    bb_in = nc.dram_tensor("bb_in", [NB, 128, E], f32,
                           kind="ExternalInput").ap()
    bidx_in = nc.dram_tensor("bidx_in", [NB * 3, 128, BJ], i32,
                             kind="ExternalInput").ap()
    col_in = [nc.dram_tensor(f"col{b}", [128, C[b]], dt.uint8,
                             kind="ExternalInput").ap() for b in range(NB)]
    idx16_in = [nc.dram_tensor(f"idx{b}", [16, C[b] * 8], i16,
                              kind="ExternalInput").ap() for b in range(NB)]
    loss_out = nc.dram_tensor("loss", [1, 1], f32, kind="ExternalOutput").ap()

    with tile.TileContext(nc) as tc:
        with (
            tc.tile_pool(name="dram", bufs=1, space="DRAM") as dpool,
            tc.tile_pool(name="pers", bufs=1) as pers,
            tc.tile_pool(name="work", bufs=2) as work,
            tc.tile_pool(name="small", bufs=4) as small,
            tc.tile_pool(name="ppx", bufs=3, space="PSUM") as ppx,
            tc.tile_pool(name="ppy", bufs=2, space="PSUM") as ppy,
        ):
            t2s = [dpool.tile([SH, WT], bf16, tag=f"t2s{v}",
                              name=f"t2s{v}") for v in range(NV)]
            idxr = [dpool.tile([128, C[b] * 8], i16, tag=f"idxr{b}",
                               name=f"idxr{b}") for b in range(NB)]
            t2f = [dpool.tile([NTOT, WT], bf16, tag=f"t2f{v}",
                              name=f"t2f{v}", addr_space=shared)
                   for v in range(NV)]
            lag_i = dpool.tile([1, 2], f32, tag="lag_i", name="lag_i")
            lag_o = dpool.tile([ncores, 2], f32, tag="lag_o", name="lag_o",
                               addr_space=shared)

            tot = pers.tile([128, NT * E], f32, tag="tot", name="tot")
            irep = pers.tile([128, 128 * G], bf16, tag="irep", name="irep")
            wsb = pers.tile([E, NB * E], bf16, tag="wsb", name="wsb")
            bbsb = pers.tile([128, NB * E], f32, tag="bbsb", name="bbsb")
            bidx = pers.tile([128, NB * 3 * BJ], i32, tag="bidx", name="bidx")
            dinvsb = pers.tile([128, NB * NT], f32, tag="dinvsb",
                               name="dinvsb")
            dinv3z = pers.tile([128, 1], f32, tag="dinv3z", name="dinv3z")
            onesf = pers.tile([128, 1], f32, tag="onesf", name="onesf")
            racc = pers.tile([128, 16], f32, tag="racc", name="racc")
            blacc = pers.tile([128, NB], f32, tag="blacc", name="blacc")

            nc.sync.dma_start(
                out=irep[:].rearrange("p (a x) -> p a x", a=1),
                in_=bc(iotar_in, 0, 128))
            nc.sync.dma_start(
                out=dinvsb[:].rearrange("p (b t) -> p b t", b=NB),
                in_=dinv_in.rearrange("b p t -> p b t"))
            nc.sync.dma_start(
                out=wsb[:].rearrange("k (b e) -> k b e", b=NB),
                in_=w_in.rearrange("b k e -> k b e"))
            nc.sync.dma_start(
                out=bbsb[:].rearrange("p (b e) -> p b e", b=NB),
                in_=bb_in.rearrange("b p e -> p b e"))
            nc.sync.dma_start(
                out=bidx[:].rearrange("p (a j) -> p a j", a=NB * 3),
                in_=bidx_in.rearrange("a p j -> p a j"))
            for b in range(NB):
                nc.sync.dma_start(
                    out=idxr[b][:].rearrange("(a q) x -> a q x", a=8),
                    in_=bc(idx16_in[b], 0, 8))
            totb = work.tile([128, wch_max * 128], bf16, tag="gat",
                             name="totb")
            nc.sync.dma_start(
                out=totb[:, :NT * E].rearrange("p (t e) -> p t e", e=E),
                in_=init_in.rearrange("(t p) e -> p t e", p=128))
            nc.vector.tensor_copy(out=tot[:], in_=totb[:, :NT * E])
            nc.vector.memset(onesf[:], 1.0)
            nc.vector.memset(dinv3z[:], 0.0)
            epsb = pers.tile([128, 1], f32, tag="epsb", name="epsb")
            nc.vector.memset(epsb[:], 1e-24)

            def dinv_ap(v, t):
                # per-partition dinv scalar for (behavior v, dest tile t)
                if v < NB:
                    return dinvsb[:, v * NT + t:v * NT + t + 1]
                return dinv3z[:, 0:1]

            # reg term: sum of squares of the initial embeddings
            NREG = (NT * E + 1023) // 1024
            sqd = pers.tile([128, 1024], f32, tag="sqd", name="sqd")
            for i in range(NREG):
                sl = slice(i * 1024, min((i + 1) * 1024, NT * E))
                nc.scalar.activation(out=sqd[:, : sl.stop - sl.start],
                                     in_=tot[:, sl], func=AF.Square,
                                     accum_out=racc[:, i:i + 1])

            # ------- lazy consumption-ordered one-hot group builder -------
            class IndBuilder:
                """Builds one-hot groups for consumption positions
                [c0, c0+wch) on demand, in order, so only a few groups are
                live at once. Layout [p, d, g] (g innermost) so every
                operand of the is_equal has stride-1 innermost dims and the
                DVE runs in its 2x perf mode."""

                def __init__(self, cs, c0, wch):
                    self.cs, self.c0, self.wch = cs, c0, wch
                    self.groups = {}

                def get(self, cpos):
                    rel = cpos - self.c0
                    g0 = (rel // G) * G
                    if g0 not in self.groups:
                        gw = min(G, self.wch - g0)
                        ind = work.tile([128, 128 * G], dt.bfloat16,
                                        tag="ind", name="ind", bufs=3)
                        iv = ind[:].rearrange("p (d g) -> p d g", g=G)
                        nc.vector.tensor_tensor(
                            out=iv[:, :, :gw],
                            in0=irep[:].rearrange(
                                "p (d g) -> p d g", g=G)[:, :, :gw],
                            in1=bc(self.cs[:, g0:g0 + gw], 1, 128),
                            op=ALU.is_equal)
                        self.groups[g0] = iv
                    return self.groups[g0], rel - g0

            # ------------- T2 staging + per-piece AllGather -------------
            # Table v is built tile-by-tile (fused into main_pass(v-1)'s
            # post_tile stream); every 2 flushes completes one PIECE of the
            # local slab and fires that piece's AllGather, so collectives
            # overlap the remaining compute of the producing behavior.
            PIECE = BUCK // ncores

            def ag_piece(v, k):
                src = t2s[v][k * PIECE:(k + 1) * PIECE, :]
                if sim == "noag":
                    return
                if sim:
                    for r in range(ncores):
                        o = k * BUCK + r * PIECE
                        nc.sync.dma_start(out=t2f[v][o:o + PIECE, :],
                                          in_=src)
                else:
                    nc.gpsimd.collective_compute(
                        "AllGather", mybir.AluOpType.bypass,
                        replica_groups=[list(range(ncores))],
                        ins=[src.opt()],
                        outs=[t2f[v][k * BUCK:(k + 1) * BUCK, :].opt()])

            class Stager:
                def __init__(self, v):
                    self.v = v
                    self.s65 = None

                def stage(self, t):
                    i = t % FLUSH
                    if i == 0:
                        self.s65 = work.tile([128, FLUSH * WT], bf16,
                                             tag="s65", name="s65")
                    totsl = tot[:, t * E:(t + 1) * E]
                    nc.vector.tensor_scalar(
                        out=self.s65[:, i * WT:i * WT + E], in0=totsl,
                        scalar1=dinv_ap(self.v, t), scalar2=None,
                        op0=ALU.mult)
                    nc.scalar.copy(
                        out=self.s65[:, i * WT + E:i * WT + 2 * E],
                        in_=totsl)
                    if i == FLUSH - 1:
                        tf = t - i
                        nc.sync.dma_start(
                            out=t2s[self.v][:].rearrange(
                                "(t p) w -> p t w", p=128)[:, tf:t + 1, :],
                            in_=self.s65[:, :FLUSH * WT].rearrange(
                                "p (t w) -> p t w", w=WT))

            def assemble(v):
                st = Stager(v)
                for t in range(NT):
                    st.stage(t)

            # ---------------- main pass ----------------
            self_q = [0]
            NQ = 4

            def ag_block(v):
                # all 7 piece triggers up-front: their input flushes
                # completed during the previous pass, so these don't stall
                # the in-order Pool queue, and the transfers stream ahead
                # of the bucket-ordered gathers that consume them.
                for k in range(NBK):
                    ag_piece(v, k)

            def main_pass(b):
                ag_block(b)
                stg = Stager(b + 1)
                for w in range(NW):
                    segs = windows_md[b][w]
                    g0 = min(st for (_, st, _) in segs)
                    wch = sum(n for (_, _, n) in segs)
                    t0w = w * WTL
                    c0 = int(cstart_md[b][t0w])
                    cs8 = small.tile([128, wch_max], dt.uint8, tag="cs8",
                                     name="cs8", bufs=2)
                    nc.sync.dma_start(out=cs8[:, :wch],
                                      in_=col_in[b][:, c0:c0 + wch])
                    cs = small.tile([128, wch_max], bf16, tag="cs", name="cs",
                                    bufs=3)
                    nc.vector.tensor_copy(out=cs[:, :wch], in_=cs8[:, :wch])
                    ixs = small.tile([128, wch_max * 8], i16, tag="ixs",
                                     name="ixs", bufs=2)
                    nc.sync.dma_start(
                        out=ixs[:, :wch * 8],
                        in_=idxr[b][:, g0 * 8:(g0 + wch) * 8])
                    gat = work.tile([128, wch_max * 128], bf16, tag="gat",
                                    name="gat")
                    gv = gat[:].rearrange("p (c e) -> p c e", e=128)
                    for (be, st, n) in segs:
                        for o in range(0, n, 12):
                            if sim == "nogather":
                                break
                            m = min(12, n - o)
                            so = st - g0 + o
                            nc.gpsimd.dma_gather(
                                out_ap=gv[:, so:so + m, :],
                                in_ap=t2f[b][be * BUCK:(be + 1) * BUCK, :],
                                idxs_ap=ixs[:, so * 8:(so + m) * 8],
                                num_idxs=m * 128,
                                num_idxs_reg=m * 128,
                                elem_size=WT,
                                single_packet=False,
                                queue_num=self_q[0] % NQ)
                            self_q[0] += 1
                    bld = IndBuilder(cs, c0, wch)
                    for t in range(t0w, min(t0w + WTL, NT)):
                        plist = tiles_md[b][t]
                        xt_ps = ppx.tile([E, 128], f32, tag="xt", name="xt")
                        for j, pos in enumerate(plist):
                            iv, r = bld.get(int(cstart_md[b][t]) + j)
                            nc.tensor.matmul(
                                out=xt_ps[:],
                                lhsT=gv[:, pos - g0, 0:E],
                                rhs=iv[:, :, r],
                                start=(j == 0), stop=(j == len(plist) - 1))
                        post_tile(b, t, xt_ps)
                        stg.stage(t)

            def post_tile(b, t, xt_ps):
                xts = small.tile([E, 128], bf16, tag="xts", name="xts")
                nc.scalar.copy(out=xts[:], in_=xt_ps[:])
                y_ps = ppy.tile([128, E], f32, tag="y", name="y")
                nc.tensor.matmul(out=y_ps[:], lhsT=xts[:],
                                 rhs=wsb[:, b * E:(b + 1) * E],
                                 start=True, stop=True)
                z = small.tile([128, E], f32, tag="z", name="z")
                ss = small.tile([128, 1], f32, tag="ss", name="ss")
                # z = y*dinv_col + bias
                nc.vector.scalar_tensor_tensor(
                    out=z[:], in0=y_ps[:], scalar=dinv_ap(b, t),
                    in1=bbsb[:, b * E:(b + 1) * E],
                    op0=ALU.mult, op1=ALU.add)
                sq = small.tile([128, E], f32, tag="sq", name="sq")
                nc.scalar.activation(out=sq[:], in_=z[:], func=AF.Square,
                                     accum_out=ss[:])
                # sqrt(ss + 1e-24) ~= max(sqrt(ss), 1e-12)
                nc.scalar.activation(out=ss[:], in_=ss[:], func=AF.Sqrt,
                                     bias=epsb[:, 0:1])
                rin = small.tile([128, 1], f32, tag="rin", name="rin")
                nc.vector.reciprocal(out=rin[:], in_=ss[:])
                totsl = tot[:, t * E:(t + 1) * E]
                # tot += z * rin
                nc.vector.scalar_tensor_tensor(
                    out=totsl, in0=z[:], scalar=rin[:, 0:1], in1=totsl,
                    op0=ALU.mult, op1=ALU.add)

            # ---------------- loss ----------------
            LOG1P_C = [2.4139025189026897e-09, 0.9999996692324197,
                       -0.499988759640371, 0.3331669190104936,
                       -0.2486582066434577, 0.19337637102999028,
                       -0.14517645896753417, 0.09470379566439587,
                       -0.04713346504062944, 0.015145372148722138,
                       -0.002288060381570317]

            def loss_pass(b):
                gs = []
                for k in range(3):
                    gk = small.tile([128, BJ * WT], bf16, tag=f"bg{k}",
                                    name=f"bg{k}")
                    gkv = gk[:].rearrange("p (j w) -> p j w", w=WT)
                    for j in range(BJ):
                        o = (b * 3 + k) * BJ + j
                        nc.gpsimd.indirect_dma_start(
                            out=gkv[:, j, :],
                            out_offset=None,
                            in_=t2f[b + 1][:],
                            in_offset=bass.IndirectOffsetOnAxis(
                                ap=bidx[:, o:o + 1], axis=0))
                    gs.append(gkv)
                prod = small.tile([128, BJ * E], f32, tag="prod", name="prod")
                pv = prod[:].rearrange("p (j e) -> p j e", e=E)
                sco = small.tile([128, 2 * BJ], f32, tag="sco", name="sco")
                for k in range(2):
                    nc.vector.tensor_tensor(out=pv, in0=gs[0][:, :, E:2 * E],
                                            in1=gs[k + 1][:, :, E:2 * E],
                                            op=ALU.mult)
                    nc.vector.tensor_reduce(
                        out=sco[:, k * BJ:(k + 1) * BJ], in_=pv,
                        axis=mybir.AxisListType.X, op=ALU.add)
                dd = small.tile([128, BJ], f32, tag="dd", name="dd")
                nc.vector.tensor_tensor(out=dd[:], in0=sco[:, 0:BJ],
                                        in1=sco[:, BJ:2 * BJ],
                                        op=ALU.subtract)
                aab = small.tile([128, BJ], f32, tag="aab", name="aab")
                nc.vector.tensor_scalar(out=aab[:], in0=dd[:], scalar1=-1.0,
                                        scalar2=None, op0=ALU.mult)
                nc.vector.tensor_tensor(out=aab[:], in0=aab[:], in1=dd[:],
                                        op=ALU.max)
                zex = small.tile([128, BJ], f32, tag="zex", name="zex")
                nc.scalar.activation(out=zex[:], in_=aab[:], func=AF.Exp,
                                     scale=-1.0)
                pol = small.tile([128, BJ], f32, tag="pol", name="pol")
                nc.vector.tensor_scalar(out=pol[:], in0=zex[:],
                                        scalar1=LOG1P_C[10],
                                        scalar2=LOG1P_C[9],
                                        op0=ALU.mult, op1=ALU.add)
                for k in range(8, -1, -1):
                    nc.vector.tensor_tensor(out=pol[:], in0=pol[:],
                                            in1=zex[:], op=ALU.mult)
                    nc.vector.tensor_scalar(out=pol[:], in0=pol[:],
                                            scalar1=LOG1P_C[k], scalar2=None,
                                            op0=ALU.add)
                nc.vector.tensor_scalar(out=dd[:], in0=dd[:], scalar1=-1.0,
                                        scalar2=0.0, op0=ALU.mult,
                                        op1=ALU.max)
                nc.vector.tensor_tensor(out=pol[:], in0=pol[:], in1=dd[:],
                                        op=ALU.add)
                nc.vector.tensor_reduce(out=blacc[:, b:b + 1], in_=pol[:],
                                        axis=mybir.AxisListType.X,
                                        op=ALU.add)

            # ================= program =================
            assemble(0)       # stages table 0 (no AGs)
            main_pass(0)      # AG(0) block, then gathers; stages table 1
            main_pass(1)      # AG(1) block first; stages table 2
            loss_pass(0)      # table 1 complete by now
            main_pass(2)      # AG(2) block first; stages table 3
            loss_pass(1)
            ag_block(3)
            loss_pass(2)

            # ---------------- final combine ----------------
            pack = small.tile([128, 2], f32, tag="pack", name="pack")
            nc.vector.tensor_reduce(out=pack[:, 0:1], in_=blacc[:],
                                    axis=mybir.AxisListType.X, op=ALU.add)
            nc.vector.tensor_reduce(out=pack[:, 1:2], in_=racc[:, :NREG],
                                    axis=mybir.AxisListType.X, op=ALU.add)
            fin_ps = ppy.tile([1, 2], f32, tag="fin", name="fin", bufs=1)
            nc.tensor.matmul(out=fin_ps[:], lhsT=onesf[:], rhs=pack[:],
                             start=True, stop=True)
            fin = small.tile([1, 2], f32, tag="fins", name="fins")
            nc.vector.tensor_copy(out=fin[:], in_=fin_ps[:])
            nc.sync.dma_start(out=lag_i[:], in_=fin[:])
            all_gather(lag_i, lag_o, ncores)
            lsb = small.tile([1, 2 * ncores], f32, tag="lsb", name="lsb")
            nc.sync.dma_start(
                out=lsb[:],
                in_=lag_o[:].rearrange("(o a) b -> o (a b)", o=1))
            bl = small.tile([1, 2], f32, tag="bl", name="bl")
            lv = lsb[:].rearrange("p (a b) -> p a b", b=2)
            nc.vector.tensor_reduce(out=bl[:, 0:1], in_=lv[:, :, 0:1],
                                    axis=mybir.AxisListType.XY, op=ALU.add)
            nc.vector.tensor_reduce(out=bl[:, 1:2], in_=lv[:, :, 1:2],
                                    axis=mybir.AxisListType.XY, op=ALU.add)
            res = small.tile([1, 1], f32, tag="res", name="res")
            nc.vector.tensor_scalar(out=res[:], in0=bl[:, 1:2],
                                    scalar1=cfg["reg_weight"] * 0.5,
                                    scalar2=None, op0=ALU.mult)
            nc.vector.tensor_tensor(out=res[:], in0=res[:], in1=bl[:, 0:1],
                                    op=ALU.add)
            nc.vector.tensor_scalar(out=res[:], in0=res[:],
                                    scalar1=1.0 / cfg["batch"],
                                    scalar2=None, op0=ALU.mult)
            nc.sync.dma_start(out=loss_out, in_=res[:])

    nc.compile()
    return nc


# ---------------------------------------------------------------------------
# Entry point
# ---------------------------------------------------------------------------
LAST_RESULTS = None


def kernel(**inputs) -> np.ndarray:
    global LAST_RESULTS
    cfg = FULL_CFG
    edges = np.asarray(inputs["edges"])
    arrs = make_schedule_and_arrays(edges, cfg)
    sched = arrs[0]
    in_maps = make_inputs_per_core(inputs, cfg, arrs)
    nc = build_program(cfg, sched)

    import os
    os.environ["BASS_NEVER_TRACE"] = "1"  # axon NTFF hook absent here
    from concourse import bass_utils
    res = bass_utils.run_bass_kernel_spmd(
        nc, in_maps, core_ids=list(range(cfg["ncores"])))
    LAST_RESULTS = res
    out = res.results[0]["loss"]
    return np.float32(out.reshape(-1)[0])

